# revision 4
# baseline (speedup 1.0000x reference)
"""AutoCorrelation Bass kernel, refinement architecture (stage 2).

Per batch: correlation pipeline (projections, fwd DFT, pointwise, inv DFT)
runs in float32r (tf32-grade, 1 cyc/row on PE) and is used ONLY to select
top-M=5 candidate lags per channel plus the softmax denominator. The top-3
selection and softmax weights then come from EXACT fp32 time-domain dots
a[tau] = (1/T) sum_t q2[t] k2[t-tau], with q2/k2 from fp32 matmuls and the
circular k-shifts gathered from a DRAM table via per-partition indirect DMA.
Value path (v-proj f32r, agg/E in bf16) only affects output values (2e-2 rel
gate; flips cost ~1e-2 each so refined selection must match fp32 reference).
"""
import numpy as np

import concourse.bass as bass
import concourse.tile as tile
from concourse import bacc, mybir

dt = mybir.dt
AF = mybir.ActivationFunctionType
OP = mybir.AluOpType

P = 128
B, C, T, K = 16, 512, 2048, 3
NB = 2
NCORES = 8
F = 1152
TC = T // P
CC = C // P
FC = F // P
NE = K * C // P
TE = 1152
TEC = TE // P
TO = 1024
TOC = TO // P
H = T // 2
M = 5                     # refinement candidates per channel

_CACHE = {}


def _dft_matrices():
    """Radix-split DFT matrices (fp64 -> fp32). Same as baseline."""
    t640 = np.arange(640.0)[:, None]
    t512 = np.arange(512.0)[:, None]
    ge = np.arange(640.0)[None, :]
    go = np.arange(512.0)[None, :]
    wree = np.where((t640 <= 512) & (ge <= 512),
                    np.cos(2 * np.pi * t640 * (2 * ge) / T), 0.0).astype(np.float32)
    wreo = np.cos(2 * np.pi * t512 * (2 * go + 1) / T).astype(np.float32)
    wime = np.where(ge <= 512,
                    -np.sin(2 * np.pi * t512 * (2 * ge) / T), 0.0).astype(np.float32)
    wimo = np.where(t640 <= 512,
                    -np.sin(2 * np.pi * t640 * (2 * go + 1) / T), 0.0).astype(np.float32)

    f64 = np.arange(F, dtype=np.float64)[None, :]
    livef = f64 <= H
    w = np.where((f64 == 0) | (f64 == H), 1.0, 2.0) * livef / (T * T)
    fc_ = f64.T
    tt = np.arange(TE, dtype=np.float64)[None, :]
    cie = np.where((fc_ <= H) & (tt <= H),
                   np.cos(2 * np.pi * fc_ * tt / T) * w.T, 0.0)
    tt2 = np.arange(TO, dtype=np.float64)[None, :]
    sie = np.where(fc_ <= H,
                   -np.sin(2 * np.pi * fc_ * tt2 / T) * w.T, 0.0)

    def permrows(m):
        out = np.zeros_like(m)
        out[0:513] = m[0:1025:2]
        out[640:1152] = m[1:1024:2]
        return out

    return (wree, wreo, wime, wimo,
            permrows(cie).astype(np.float32), permrows(sie).astype(np.float32))


def _build():
    nc = bacc.Bacc("TRN2", target_bir_lowering=False, debug=False,
                   num_devices=NCORES)

    query2 = nc.dram_tensor("query2", [NB, C, T], dt.float32, kind="ExternalInput").ap()
    key2 = nc.dram_tensor("key2", [NB, C, T], dt.float32, kind="ExternalInput").ap()
    value2 = nc.dram_tensor("value2", [NB, C, T], dt.float32r, kind="ExternalInput").ap()
    Wq = nc.dram_tensor("Wq", [C, C], dt.float32, kind="ExternalInput").ap()
    Wk = nc.dram_tensor("Wk", [C, C], dt.float32, kind="ExternalInput").ap()
    Wqr = nc.dram_tensor("Wqr", [C, C], dt.float32r, kind="ExternalInput").ap()
    Wkr = nc.dram_tensor("Wkr", [C, C], dt.float32r, kind="ExternalInput").ap()
    Wvr = nc.dram_tensor("Wvr", [C, C], dt.float32r, kind="ExternalInput").ap()
    Wf = nc.dram_tensor("Wf", [K * C, C], dt.float32, kind="ExternalInput").ap()
    Wree = nc.dram_tensor("Wree", [640, 640], dt.float32r, kind="ExternalInput").ap()
    Wreo = nc.dram_tensor("Wreo", [512, 512], dt.float32r, kind="ExternalInput").ap()
    Wime = nc.dram_tensor("Wime", [512, 640], dt.float32r, kind="ExternalInput").ap()
    Wimo = nc.dram_tensor("Wimo", [640, 512], dt.float32r, kind="ExternalInput").ap()
    Cie = nc.dram_tensor("Cie", [F, TE], dt.float32r, kind="ExternalInput").ap()
    Sie = nc.dram_tensor("Sie", [F, TO], dt.float32r, kind="ExternalInput").ap()
    out2 = nc.dram_tensor("out2", [NB, C, T], dt.float32, kind="ExternalOutput").ap()

    v2 = nc.dram_tensor("v2", [NB * C, 2 * T], dt.bfloat16).ap()     # rolled-v table
    k2d = nc.dram_tensor("k2d", [NB * C, 2 * T], dt.float32).ap()    # k2 gather table

    with tile.TileContext(nc) as tc:
        from contextlib import ExitStack

        # ---- P0: resident constants ----
        es_const = ExitStack()
        cpool = es_const.enter_context(tc.tile_pool(name="consts", bufs=1, side="left"))
        cie_sb = cpool.tile([P, FC, 640], dt.float32r, tag="cie_sb")
        nc.sync.dma_start(cie_sb[:], Cie.rearrange("(n p) t -> p n t", p=P)[:, :, 0:640])
        sie_sb = cpool.tile([P, FC, 640], dt.float32r, tag="sie_sb")
        nc.sync.dma_start(sie_sb[:], Sie.rearrange("(n p) t -> p n t", p=P)[:, :, 0:640])
        wv_sb = cpool.tile([P, CC, C], dt.float32r, tag="wv_sb")
        nc.sync.dma_start(wv_sb[:], Wvr.rearrange("(n p) d -> p n d", p=P))
        wqr_sb = cpool.tile([P, CC, C], dt.float32r, tag="wqr_sb")
        nc.sync.dma_start(wqr_sb[:], Wqr.rearrange("(n p) d -> p n d", p=P))
        wkr_sb = cpool.tile([P, CC, C], dt.float32r, tag="wkr_sb")
        nc.sync.dma_start(wkr_sb[:], Wkr.rearrange("(n p) d -> p n d", p=P))

        for b in range(NB):
            # ============ P1: radix splits + f32r split-proj + v ============
            es_qk = ExitStack()
            qk_pool = es_qk.enter_context(tc.tile_pool(name=f"qk{b}", bufs=1, side="right"))
            qTee = qk_pool.tile([P, 5, C], dt.float32r, tag="qTee")
            qTeo = qk_pool.tile([P, 4, C], dt.float32r, tag="qTeo")
            qToo = qk_pool.tile([P, 4, C], dt.float32r, tag="qToo")
            qToe = qk_pool.tile([P, 5, C], dt.float32r, tag="qToe")
            kTee = qk_pool.tile([P, 5, C], dt.float32r, tag="kTee")
            kTeo = qk_pool.tile([P, 4, C], dt.float32r, tag="kTeo")
            kToo = qk_pool.tile([P, 4, C], dt.float32r, tag="kToo")
            kToe = qk_pool.tile([P, 5, C], dt.float32r, tag="kToe")

            with tc.tile_pool(name=f"a{b}", bufs=1) as ap_, \
                 tc.tile_pool(name=f"axs{b}", bufs=2) as axs, \
                 tc.tile_pool(name=f"aps{b}", bufs=3, space="PSUM") as aps:
                for name, srcx, w_sb, dsts in (
                        ("q", query2, wqr_sb, (qTee, qTeo, qToo, qToe)),
                        ("k", key2, wkr_sb, (kTee, kTeo, kToo, kToe))):
                    dee, deo, doo, doe = dsts
                    xee = ap_.tile([P, CC, 640], dt.float32r, tag="xee")
                    xeo = ap_.tile([P, CC, 512], dt.float32r, tag="xeo")
                    xoo = ap_.tile([P, CC, 512], dt.float32r, tag="xoo")
                    xoe = ap_.tile([P, CC, 640], dt.float32r, tag="xoe")
                    for cc in range(CC):
                        x_cc = axs.tile([P, T], dt.float32, tag="x_cc")
                        nc.sync.dma_start(
                            x_cc[:],
                            srcx[b].rearrange("(n p) t -> p n t", p=P)[:, cc, :])
                        ab = axs.tile([P, 2, 511], dt.float32, tag="ab")
                        x = x_cc
                        # f32r memset hits a walrus codegen bug; zero via ACT
                        nc.scalar.activation(xee[:, cc, 513:640], x[:, 0:127],
                                             AF.Copy, bias=0.0, scale=0.0)
                        nc.scalar.activation(xoe[:, cc, 513:640], x[:, 0:127],
                                             AF.Copy, bias=0.0, scale=0.0)
                        nc.scalar.activation(xoe[:, cc, 0:1], x[:, 0:1],
                                             AF.Copy, bias=0.0, scale=0.0)
                        nc.scalar.activation(xoo[:, cc, 0:1], x[:, 0:1],
                                             AF.Copy, bias=0.0, scale=0.0)
                        nc.vector.tensor_tensor(out=ab[:, 0, :], in0=x[:, 1:512],
                                                in1=x[:, T - 1:1536:-1], op=OP.add)
                        nc.vector.tensor_tensor(out=ab[:, 1, :], in0=x[:, 1023:512:-1],
                                                in1=x[:, 1025:1536], op=OP.add)
                        nc.vector.tensor_tensor(out=xee[:, cc, 1:512], in0=ab[:, 0, :],
                                                in1=ab[:, 1, :], op=OP.add)
                        nc.vector.tensor_tensor(out=xeo[:, cc, 1:512], in0=ab[:, 0, :],
                                                in1=ab[:, 1, :], op=OP.subtract)
                        nc.vector.tensor_tensor(out=ab[:, 0, :], in0=x[:, 1:512],
                                                in1=x[:, T - 1:1536:-1], op=OP.subtract)
                        nc.vector.tensor_tensor(out=ab[:, 1, :], in0=x[:, 1023:512:-1],
                                                in1=x[:, 1025:1536], op=OP.subtract)
                        nc.vector.tensor_tensor(out=xoo[:, cc, 1:512], in0=ab[:, 0, :],
                                                in1=ab[:, 1, :], op=OP.subtract)
                        nc.vector.tensor_tensor(out=xoe[:, cc, 1:512], in0=ab[:, 0, :],
                                                in1=ab[:, 1, :], op=OP.add)
                        nc.vector.tensor_tensor(out=xee[:, cc, 0:1], in0=x[:, 0:1],
                                                in1=x[:, H:H + 1], op=OP.add)
                        nc.vector.tensor_tensor(out=xeo[:, cc, 0:1], in0=x[:, 0:1],
                                                in1=x[:, H:H + 1], op=OP.subtract)
                        nc.vector.tensor_tensor(out=xee[:, cc, 512:513], in0=x[:, 512:513],
                                                in1=x[:, 1536:1537], op=OP.add)
                        nc.vector.tensor_tensor(out=xoe[:, cc, 512:513], in0=x[:, 512:513],
                                                in1=x[:, 1536:1537], op=OP.subtract)
                    for st_, dst, nch in ((xee, dee, 5), (xeo, deo, 4),
                                          (xoo, doo, 4), (xoe, doe, 5)):
                        for i in range(nch):
                            ps = aps.tile([P, C], dt.float32, tag="proj_ps")
                            for cc in range(CC):
                                nc.tensor.matmul(
                                    ps[:], st_[:, cc, bass.ts(i, P)],
                                    w_sb[:, cc, :],
                                    start=(cc == 0), stop=(cc == CC - 1))
                            nc.scalar.activation(dst[:, i, :], ps[:], AF.Copy)

            # ============ P2: forward DFT (f32r) + pointwise ============
            es_p = ExitStack()
            p_pool = es_p.enter_context(tc.tile_pool(name=f"p{b}", bufs=1, side="left"))
            pre = p_pool.tile([P, FC, C], dt.float32r, tag="pre")
            pim = p_pool.tile([P, FC, C], dt.float32r, tag="pim")
            with tc.tile_pool(name=f"bmat{b}", bufs=2) as bmat, \
                 tc.tile_pool(name=f"bps{b}", bufs=2, space="PSUM") as bps, \
                 tc.tile_pool(name=f"btmp{b}", bufs=2) as btmp:
                wree_r = Wree.rearrange("(n p) f -> p n f", p=P)
                wreo_r = Wreo.rearrange("(n p) f -> p n f", p=P)
                wime_r = Wime.rearrange("(n p) f -> p n f", p=P)
                wimo_r = Wimo.rearrange("(n p) f -> p n f", p=P)
                for fc in range(FC):
                    even = fc < 5
                    fl = fc if even else fc - 5
                    ncos, nsin = (5, 4) if even else (4, 5)
                    cm = bmat.tile([P, 5, P], dt.float32r, tag="cm")
                    nc.sync.dma_start(
                        cm[:, 0:ncos, :],
                        (wree_r if even else wreo_r)[:, :, bass.ts(fl, P)])
                    sm = bmat.tile([P, 5, P], dt.float32r, tag="sm")
                    nc.sync.dma_start(
                        sm[:, 0:nsin, :],
                        (wime_r if even else wimo_r)[:, :, bass.ts(fl, P)])
                    qcos = qTee if even else qTeo
                    qsin = qToo if even else qToe
                    kcos = kTee if even else kTeo
                    ksin = kToo if even else kToe
                    acc = {}
                    for nm, mat, sig, nchunk in (
                            ("aq", cm, qcos, ncos), ("bq", sm, qsin, nsin),
                            ("ak", cm, kcos, ncos), ("bk", sm, ksin, nsin)):
                        ps = bps.tile([P, C], dt.float32, tag=nm, name=f"ps_{nm}")
                        for i in range(nchunk):
                            nc.tensor.matmul(
                                ps[:], mat[:, i, :], sig[:, i, :],
                                start=(i == 0), stop=(i == nchunk - 1))
                        acc[nm] = ps
                    aqs = btmp.tile([P, C], dt.float32, tag="aqs")
                    nc.scalar.activation(aqs[:], acc["aq"][:], AF.Copy)
                    bqs = btmp.tile([P, C], dt.float32, tag="bqs")
                    nc.scalar.activation(bqs[:], acc["bq"][:], AF.Copy)
                    tmp = btmp.tile([P, C], dt.float32, tag="tmp")
                    nc.vector.tensor_tensor(
                        out=pre[:, fc, :], in0=aqs[:], in1=acc["ak"][:], op=OP.mult)
                    nc.vector.tensor_tensor(
                        out=tmp[:], in0=bqs[:], in1=acc["bk"][:], op=OP.mult)
                    nc.vector.tensor_tensor(
                        out=pre[:, fc, :], in0=pre[:, fc, :], in1=tmp[:], op=OP.add)
                    nc.vector.tensor_tensor(
                        out=pim[:, fc, :], in0=bqs[:], in1=acc["ak"][:], op=OP.mult)
                    tmp2 = btmp.tile([P, C], dt.float32, tag="tmp2")
                    nc.vector.tensor_tensor(
                        out=tmp2[:], in0=aqs[:], in1=acc["bk"][:], op=OP.mult)
                    nc.vector.tensor_tensor(
                        out=pim[:, fc, :], in0=pim[:, fc, :], in1=tmp2[:], op=OP.subtract)
            es_qk.close()

            # ============ P3: inverse DFT (f32r) + top-8 + denominator ======
            es_sel = ExitStack()
            sel_pool = es_sel.enter_context(
                tc.tile_pool(name=f"sel{b}", bufs=1, side="right"))
            idx8 = [sel_pool.tile([P, 8], dt.uint32, tag=f"idx8_{dc}",
                                  name=f"idx8_{b}_{dc}") for dc in range(CC)]
            negm = [sel_pool.tile([P, 1], dt.float32, tag=f"negm_{dc}",
                                  name=f"negm_{b}_{dc}") for dc in range(CC)]
            rs = [sel_pool.tile([P, 1], dt.float32, tag=f"rs_{dc}",
                                name=f"rs_{b}_{dc}") for dc in range(CC)]
            with tc.tile_pool(name=f"cr{b}", bufs=2) as crp, \
                 tc.tile_pool(name=f"ctmp{b}", bufs=2) as ctmp, \
                 tc.tile_pool(name=f"cps{b}", bufs=1, space="PSUM") as cps:
                HB = H // 2
                for cc in range(CC):
                    rcE = cps.tile([P, HB], dt.float32, tag="rcE", name="ps_rcE")
                    rcE2 = cps.tile([P, P], dt.float32, tag="rcE2", name="ps_rcE2")
                    rcO = cps.tile([P, HB], dt.float32, tag="rcO", name="ps_rcO")
                    rsE = cps.tile([P, HB], dt.float32, tag="rsE", name="ps_rsE")
                    rsO = cps.tile([P, HB], dt.float32, tag="rsO", name="ps_rsO")
                    rsO2 = cps.tile([P, P], dt.float32, tag="rsO2", name="ps_rsO2")
                    for fc in range(5):
                        st, sp = (fc == 0), (fc == 4)
                        pre_l = pre[:, fc, bass.ts(cc, P)]
                        pim_l = pim[:, fc, bass.ts(cc, P)]
                        nc.tensor.matmul(rcE[:], pre_l, cie_sb[:, fc, 0:HB],
                                         start=st, stop=sp)
                        nc.tensor.matmul(rcE2[:], pre_l, cie_sb[:, fc, HB:HB + P],
                                         start=st, stop=sp)
                        nc.tensor.matmul(rsE[:], pim_l, sie_sb[:, fc, 0:HB],
                                         start=st, stop=sp)
                    for fc in range(5, FC):
                        st, sp = (fc == 5), (fc == FC - 1)
                        pre_l = pre[:, fc, bass.ts(cc, P)]
                        pim_l = pim[:, fc, bass.ts(cc, P)]
                        nc.tensor.matmul(rcO[:], pre_l, cie_sb[:, fc, 0:HB],
                                         start=st, stop=sp)
                        nc.tensor.matmul(rsO[:], pim_l, sie_sb[:, fc, 0:HB],
                                         start=st, stop=sp)
                        nc.tensor.matmul(rsO2[:], pim_l, sie_sb[:, fc, HB:HB + P],
                                         start=st, stop=sp)
                    rcO_sb = ctmp.tile([P, HB], dt.float32, tag="rcO_sb")
                    nc.scalar.activation(rcO_sb[:], rcO[:], AF.Copy)
                    rsE_sb = ctmp.tile([P, HB], dt.float32, tag="rsE_sb")
                    nc.scalar.activation(rsE_sb[:], rsE[:], AF.Copy)
                    rsO_sb = ctmp.tile([P, HB + 1], dt.float32, tag="rsO_sb")
                    nc.scalar.activation(rsO_sb[:, 0:HB], rsO[:], AF.Copy)
                    nc.scalar.activation(rsO_sb[:, HB:HB + 1], rsO2[:, 0:1], AF.Copy)
                    s1 = ctmp.tile([P, HB], dt.float32, tag="s1")
                    nc.vector.tensor_tensor(out=s1[:], in0=rcE[:], in1=rcO_sb[:],
                                            op=OP.add)
                    s2 = ctmp.tile([P, HB], dt.float32, tag="s2")
                    nc.vector.tensor_tensor(out=s2[:], in0=rcE[:], in1=rcO_sb[:],
                                            op=OP.subtract)
                    w1 = ctmp.tile([P, HB], dt.float32, tag="w1")
                    nc.vector.tensor_tensor(out=w1[:], in0=rsE_sb[:],
                                            in1=rsO_sb[:, 0:HB], op=OP.add)
                    w2 = ctmp.tile([P, HB], dt.float32, tag="w2")
                    nc.vector.tensor_tensor(out=w2[:], in0=rsO_sb[:, 0:HB],
                                            in1=rsE_sb[:], op=OP.subtract)
                    rt = crp.tile([P, T], dt.float32, tag="rt")
                    nc.vector.tensor_tensor(out=rt[:, 0:HB], in0=s1[:], in1=w1[:],
                                            op=OP.add)
                    nc.vector.tensor_tensor(out=rt[:, 1023:HB:-1], in0=s2[:, 1:HB],
                                            in1=w2[:, 1:HB], op=OP.add)
                    nc.vector.tensor_tensor(out=rt[:, 1025:1536], in0=s2[:, 1:HB],
                                            in1=w2[:, 1:HB], op=OP.subtract)
                    nc.vector.tensor_tensor(out=rt[:, T - 1:1536:-1], in0=s1[:, 1:HB],
                                            in1=w1[:, 1:HB], op=OP.subtract)
                    nc.vector.tensor_tensor(out=rt[:, HB:HB + 1], in0=rcE2[:, 0:1],
                                            in1=rsO_sb[:, HB:HB + 1], op=OP.add)
                    nc.vector.tensor_tensor(out=rt[:, H:H + 1], in0=rcE[:, 0:1],
                                            in1=rcO_sb[:, 0:1], op=OP.subtract)
                    nc.vector.tensor_tensor(out=rt[:, 1536:1537], in0=rcE2[:, 0:1],
                                            in1=rsO_sb[:, HB:HB + 1], op=OP.subtract)
                    # top-8 + softmax denominator on approx r
                    vals = ctmp.tile([P, 8], dt.float32, tag="vals")
                    nc.vector.max(vals[:], rt[:])
                    nc.vector.max_index(idx8[cc][:], vals[:], rt[:])
                    nc.scalar.activation(negm[cc][:], vals[:, 0:1],
                                         AF.Copy, bias=0.0, scale=-1.0)
                    esc = crp.tile([P, T], dt.float32, tag="esc")
                    s_col = ctmp.tile([P, 1], dt.float32, tag="s_col")
                    nc.scalar.activation(
                        esc[:], rt[:], AF.Exp,
                        bias=negm[cc][:, 0:1], scale=1.0,
                        accum_out=s_col[:, 0:1])
                    nc.vector.reciprocal(rs[cc][:], s_col[:])
            es_p.close()

            # ============ P4: fp32 q2/k2 + exact refinement ============
            w3_t = [sel_pool.tile([P, K], dt.float32, tag=f"w3_{dc}",
                                  name=f"w3_{b}_{dc}") for dc in range(CC)]
            gov = [sel_pool.tile([P, K], dt.uint32, tag=f"gov_{dc}",
                                 name=f"gov_{b}_{dc}") for dc in range(CC)]
            with tc.tile_pool(name=f"d{b}", bufs=1) as dp, \
                 tc.tile_pool(name=f"dsc{b}", bufs=2) as dsc, \
                 tc.tile_pool(name=f"dks{b}", bufs=2) as dks, \
                 tc.tile_pool(name=f"dps{b}", bufs=2, space="PSUM") as dps:
                k2r = k2d.rearrange("(n p) w -> n p w", p=P)
                # P4a: k2 (fp32) -> DRAM doubled table
                xk_sb = dp.tile([P, CC, T], dt.float32, tag="x_p4s")
                nc.sync.dma_start(
                    xk_sb[:], key2[b].rearrange("(n p) t -> p n t", p=P))
                wk_sb = dp.tile([P, CC, C], dt.float32, tag="w_p4")
                nc.sync.dma_start(wk_sb[:], Wk.rearrange("(n p) d -> p n d", p=P))
                for dc in range(CC):
                    k2sb = dsc.tile([P, T], dt.float32, tag="k2sb")
                    for tb in range(4):
                        ps = dps.tile([P, T // 4], dt.float32, tag="p4ps")
                        for cc in range(CC):
                            nc.tensor.matmul(
                                ps[:], wk_sb[:, cc, bass.ts(dc, P)],
                                xk_sb[:, cc, bass.ts(tb, T // 4)],
                                start=(cc == 0), stop=(cc == CC - 1))
                        nc.scalar.activation(
                            k2sb[:, bass.ts(tb, T // 4)], ps[:], AF.Copy)
                    nc.sync.dma_start(k2r[b * CC + dc, :, 0:T], k2sb[:])
                    nc.sync.dma_start(k2r[b * CC + dc, :, T:2 * T], k2sb[:])
                # P4v: v projection (f32r) -> v2 DRAM table (bf16, doubled)
                xv_sb = dp.tile([P, CC, T], dt.float32r, tag="x_p4s")
                nc.sync.dma_start(
                    xv_sb[:], value2[b].rearrange("(n p) t -> p n t", p=P))
                v2r = v2.rearrange("(n p) w -> n p w", p=P)
                for dc in range(CC):
                    v_sb = dsc.tile([P, T], dt.bfloat16, tag="v_sb")
                    for tb in range(4):
                        ps = dps.tile([P, T // 4], dt.float32, tag="p4ps")
                        for cc in range(CC):
                            nc.tensor.matmul(
                                ps[:], wv_sb[:, cc, bass.ts(dc, P)],
                                xv_sb[:, cc, bass.ts(tb, T // 4)],
                                start=(cc == 0), stop=(cc == CC - 1))
                        nc.scalar.activation(
                            v_sb[:, bass.ts(tb, T // 4)], ps[:], AF.Copy)
                    nc.sync.dma_start(v2r[b * CC + dc, :, 0:T], v_sb[:])
                    nc.sync.dma_start(v2r[b * CC + dc, :, T:2 * T], v_sb[:])
                # P4b: q2 per dc + gathers + dots + selection
                xq_sb = dp.tile([P, CC, T], dt.float32, tag="x_p4s")
                nc.sync.dma_start(
                    xq_sb[:], query2[b].rearrange("(n p) t -> p n t", p=P))
                wq_sb = dp.tile([P, CC, C], dt.float32, tag="w_p4")
                nc.sync.dma_start(wq_sb[:], Wq.rearrange("(n p) d -> p n d", p=P))
                for dc in range(CC):
                    q2sb = dsc.tile([P, T], dt.float32, tag="q2sb")
                    for tb in range(4):
                        ps = dps.tile([P, T // 4], dt.float32, tag="p4ps")
                        for cc in range(CC):
                            nc.tensor.matmul(
                                ps[:], wq_sb[:, cc, bass.ts(dc, P)],
                                xq_sb[:, cc, bass.ts(tb, T // 4)],
                                start=(cc == 0), stop=(cc == CC - 1))
                        nc.scalar.activation(
                            q2sb[:, bass.ts(tb, T // 4)], ps[:], AF.Copy)

                    # gather offsets for M candidates:
                    #   (b*C+dc*128+p)*2T + T - lag_m
                    iot = dsc.tile([P, 1], dt.int32, tag="iot")
                    nc.gpsimd.iota(
                        iot[:], pattern=[[0, 1]],
                        base=(b * C + dc * P) * (2 * T) + T,
                        channel_multiplier=2 * T)
                    iot_f = dsc.tile([P, 1], dt.float32, tag="iot_f")
                    nc.vector.tensor_copy(iot_f[:], iot[:])
                    idxm_f = dsc.tile([P, M], dt.float32, tag="idxm_f")
                    nc.vector.tensor_copy(idxm_f[:], idx8[dc][:, 0:M])
                    gom = dsc.tile([P, M], dt.float32, tag="gom")
                    nc.scalar.activation(gom[:], idxm_f[:],
                                         AF.Copy, bias=0.0, scale=-1.0)
                    nc.vector.tensor_scalar_add(gom[:], gom[:], iot_f[:, 0:1])
                    gou = dsc.tile([P, M], dt.uint32, tag="gou")
                    nc.vector.tensor_copy(gou[:], gom[:])

                    refined = dsc.tile([P, 8], dt.float32, tag="refined")
                    nc.gpsimd.memset(refined[:, M:8], -3.0e38)
                    for m in range(M):
                        ksh = dks.tile([P, T], dt.float32, tag="ksh")
                        nc.gpsimd.indirect_dma_start(
                            out=ksh[:], out_offset=None,
                            in_=k2d[:, :],
                            in_offset=bass.IndirectOffsetOnAxis(
                                ap=gou[:, m:m + 1], axis=1),
                            element_offset=0)
                        scr = dks.tile([P, T], dt.float32, tag="scr")
                        nc.vector.tensor_tensor(
                            out=scr[:], in0=q2sb[:], in1=ksh[:], op=OP.mult)
                        scr2 = dks.tile([P, T], dt.float32, tag="scr2")
                        nc.scalar.activation(
                            scr2[:], scr[:], AF.Copy, scale=1.0 / T,
                            accum_out=refined[:, m:m + 1])

                    valr = dsc.tile([P, 8], dt.float32, tag="valr")
                    nc.vector.max(valr[:], refined[:])
                    pos8 = dsc.tile([P, 8], dt.uint32, tag="pos8")
                    nc.vector.max_index(pos8[:], valr[:], refined[:])
                    posf = dsc.tile([P, K], dt.float32, tag="posf")
                    nc.vector.tensor_copy(posf[:], pos8[:, 0:K])
                    # one-hot map: lag_sel[:, k] = sum_m idxm_f[:, m]*(posf==m)
                    lagf = dsc.tile([P, K], dt.float32, tag="lagf")
                    eqm = dsc.tile([P, K], dt.float32, tag="eqm")
                    contrib = dsc.tile([P, K], dt.float32, tag="contrib")
                    for m in range(M):
                        nc.vector.tensor_scalar(
                            out=eqm[:], in0=posf[:], scalar1=float(m),
                            scalar2=None, op0=OP.is_equal)
                        nc.vector.tensor_scalar_mul(
                            contrib[:], eqm[:], idxm_f[:, m:m + 1])
                        if m == 0:
                            nc.vector.tensor_copy(lagf[:], contrib[:])
                        else:
                            nc.vector.tensor_tensor(
                                out=lagf[:], in0=lagf[:], in1=contrib[:],
                                op=OP.add)
                    # weights: w3 = exp(valr[0:K] - m~) * rs
                    ew = dsc.tile([P, K], dt.float32, tag="ew")
                    nc.scalar.activation(ew[:], valr[:, 0:K],
                                         AF.Exp, bias=negm[dc][:, 0:1], scale=1.0)
                    nc.vector.tensor_scalar_mul(w3_t[dc][:], ew[:], rs[dc][:, 0:1])
                    # v2 gather offsets: rowbase + T - lag_sel
                    govf = dsc.tile([P, K], dt.float32, tag="govf")
                    nc.scalar.activation(govf[:], lagf[:],
                                         AF.Copy, bias=0.0, scale=-1.0)
                    nc.vector.tensor_scalar_add(govf[:], govf[:], iot_f[:, 0:1])
                    nc.vector.tensor_copy(gov[dc][:], govf[:])
            es_sel.close_later = None  # keep sel_pool until P5 end

            # ============ P5: agg gathers (bf16) + scale + E ============
            with tc.tile_pool(name=f"e{b}", bufs=1) as ep, \
                 tc.tile_pool(name=f"eagg{b}", bufs=1) as eagg, \
                 tc.tile_pool(name=f"eps{b}", bufs=8, space="PSUM") as eps:
                wf16 = ep.tile([P, NE, C], dt.bfloat16, tag="wf16")
                with tc.tile_pool(name=f"wfload{b}", bufs=1) as wfl:
                    wf32 = wfl.tile([P, NE, C], dt.float32, tag="wf32")
                    nc.sync.dma_start(wf32[:], Wf.rearrange("(n p) d -> p n d", p=P))
                    for j in range(NE):
                        nc.scalar.activation(wf16[:, j, :], wf32[:, j, :], AF.Copy)
                agg = [eagg.tile([P, T], dt.bfloat16, tag=f"agg{j}",
                                 name=f"agg_sb{j}") for j in range(NE)]
                for k in range(K):
                    for dc in range(CC):
                        a_t = agg[k * CC + dc]
                        for hh in range(2):
                            nc.gpsimd.indirect_dma_start(
                                out=a_t[:, bass.ts(hh, T // 2)], out_offset=None,
                                in_=v2[:, :],
                                in_offset=bass.IndirectOffsetOnAxis(
                                    ap=gov[dc][:, k:k + 1], axis=1),
                                element_offset=hh * (T // 2))
                        nc.vector.tensor_scalar_mul(
                            a_t[:], a_t[:], w3_t[dc][:, k:k + 1])
                for dco in range(CC):
                    for tb in range(4):
                        ps = eps.tile([P, T // 4], dt.float32, tag="out_ps")
                        for j in range(NE):
                            nc.tensor.matmul(
                                ps[:], wf16[:, j, bass.ts(dco, P)],
                                agg[j][:, bass.ts(tb, T // 4)],
                                start=(j == 0), stop=(j == NE - 1))
                        o_sb = ep.tile([P, T // 4], dt.float32, tag="o_sb")
                        nc.scalar.activation(o_sb[:], ps[:], AF.Copy)
                        nc.sync.dma_start(
                            out2[b, bass.ts(dco, P), bass.ts(tb, T // 4)],
                            o_sb[:])
            es_sel.close()

        es_const.close()

    nc.compile()
    return nc


def _get_nc():
    if "nc" not in _CACHE:
        _CACHE["nc"] = _build()
    return _CACHE["nc"]


def kernel(query, key, value, Wq, bq, Wk, bk, Wv, bv, Wf, bf):
    query = np.ascontiguousarray(np.asarray(query, dtype=np.float32))
    key = np.ascontiguousarray(np.asarray(key, dtype=np.float32))
    value = np.ascontiguousarray(np.asarray(value, dtype=np.float32))
    for bias in (bq, bk, bv, bf):
        assert np.all(np.asarray(bias) == 0.0), "nonzero biases unsupported"

    if "mats" not in _CACHE:
        _CACHE["mats"] = _dft_matrices()
    wree, wreo, wime, wimo, cie, sie = _CACHE["mats"]

    Wqc = np.ascontiguousarray(np.asarray(Wq, np.float32))
    Wkc = np.ascontiguousarray(np.asarray(Wk, np.float32))
    shared = {
        "Wq": Wqc, "Wk": Wkc, "Wqr": Wqc, "Wkr": Wkc,
        "Wvr": np.ascontiguousarray(np.asarray(Wv, np.float32)),
        "Wf": np.ascontiguousarray(np.asarray(Wf, np.float32)),
        "Wree": wree, "Wreo": wreo, "Wime": wime, "Wimo": wimo,
        "Cie": cie, "Sie": sie,
    }
    in_maps = []
    for c in range(NCORES):
        sl = slice(c * NB, (c + 1) * NB)
        in_maps.append({
            "query2": query[sl], "key2": key[sl], "value2": value[sl], **shared})

    from concourse.bass_utils import run_bass_kernel_spmd
    nc = _get_nc()
    res = run_bass_kernel_spmd(nc, in_maps, core_ids=list(range(NCORES)))
    _CACHE["last_results"] = res
    out = np.concatenate([res.results[c]["out2"] for c in range(NCORES)], axis=0)
    return out.astype(np.float32)


# revision 9
# speedup vs baseline: 1.0143x; 1.0143x over previous
"""AutoCorrelation Bass kernel, refinement architecture (stage 2).

Per batch: correlation pipeline (projections, fwd DFT, pointwise, inv DFT)
runs in float32r (tf32-grade, 1 cyc/row on PE) and is used ONLY to select
top-M=5 candidate lags per channel plus the softmax denominator. The top-3
selection and softmax weights then come from EXACT fp32 time-domain dots
a[tau] = (1/T) sum_t q2[t] k2[t-tau], with q2/k2 from fp32 matmuls and the
circular k-shifts gathered from a DRAM table via per-partition indirect DMA.
Value path (v-proj f32r, agg/E in bf16) only affects output values (2e-2 rel
gate; flips cost ~1e-2 each so refined selection must match fp32 reference).
"""
import numpy as np

import concourse.bass as bass
import concourse.tile as tile
from concourse import bacc, mybir

dt = mybir.dt
AF = mybir.ActivationFunctionType
OP = mybir.AluOpType

P = 128
B, C, T, K = 16, 512, 2048, 3
NB = 2
NCORES = 8
F = 1152
TC = T // P
CC = C // P
FC = F // P
NE = K * C // P
TE = 1152
TEC = TE // P
TO = 1024
TOC = TO // P
H = T // 2
M = 5                     # refinement candidates per channel

_CACHE = {}


def _dft_matrices():
    """Radix-split DFT matrices (fp64 -> fp32). Same as baseline."""
    t640 = np.arange(640.0)[:, None]
    t512 = np.arange(512.0)[:, None]
    ge = np.arange(640.0)[None, :]
    go = np.arange(512.0)[None, :]
    wree = np.where((t640 <= 512) & (ge <= 512),
                    np.cos(2 * np.pi * t640 * (2 * ge) / T), 0.0).astype(np.float32)
    wreo = np.cos(2 * np.pi * t512 * (2 * go + 1) / T).astype(np.float32)
    wime = np.where(ge <= 512,
                    -np.sin(2 * np.pi * t512 * (2 * ge) / T), 0.0).astype(np.float32)
    wimo = np.where(t640 <= 512,
                    -np.sin(2 * np.pi * t640 * (2 * go + 1) / T), 0.0).astype(np.float32)

    f64 = np.arange(F, dtype=np.float64)[None, :]
    livef = f64 <= H
    w = np.where((f64 == 0) | (f64 == H), 1.0, 2.0) * livef / (T * T)
    fc_ = f64.T
    tt = np.arange(TE, dtype=np.float64)[None, :]
    cie = np.where((fc_ <= H) & (tt <= H),
                   np.cos(2 * np.pi * fc_ * tt / T) * w.T, 0.0)
    tt2 = np.arange(TO, dtype=np.float64)[None, :]
    sie = np.where(fc_ <= H,
                   -np.sin(2 * np.pi * fc_ * tt2 / T) * w.T, 0.0)

    def permrows(m):
        out = np.zeros_like(m)
        out[0:513] = m[0:1025:2]
        out[640:1152] = m[1:1024:2]
        return out

    return (wree, wreo, wime, wimo,
            permrows(cie).astype(np.float32), permrows(sie).astype(np.float32))


def _build():
    nc = bacc.Bacc("TRN2", target_bir_lowering=False, debug=False,
                   num_devices=NCORES)

    query2 = nc.dram_tensor("query2", [NB, C, T], dt.float32, kind="ExternalInput").ap()
    key2 = nc.dram_tensor("key2", [NB, C, T], dt.float32, kind="ExternalInput").ap()
    value2 = nc.dram_tensor("value2", [NB, C, T], dt.float32r, kind="ExternalInput").ap()
    Wq = nc.dram_tensor("Wq", [C, C], dt.float32, kind="ExternalInput").ap()
    Wk = nc.dram_tensor("Wk", [C, C], dt.float32, kind="ExternalInput").ap()
    Wqr = nc.dram_tensor("Wqr", [C, C], dt.float32r, kind="ExternalInput").ap()
    Wkr = nc.dram_tensor("Wkr", [C, C], dt.float32r, kind="ExternalInput").ap()
    Wvr = nc.dram_tensor("Wvr", [C, C], dt.float32r, kind="ExternalInput").ap()
    Wf = nc.dram_tensor("Wf", [K * C, C], dt.float32, kind="ExternalInput").ap()
    Wree = nc.dram_tensor("Wree", [640, 640], dt.float32r, kind="ExternalInput").ap()
    Wreo = nc.dram_tensor("Wreo", [512, 512], dt.float32r, kind="ExternalInput").ap()
    Wime = nc.dram_tensor("Wime", [512, 640], dt.float32r, kind="ExternalInput").ap()
    Wimo = nc.dram_tensor("Wimo", [640, 512], dt.float32r, kind="ExternalInput").ap()
    Cie = nc.dram_tensor("Cie", [F, TE], dt.float32r, kind="ExternalInput").ap()
    Sie = nc.dram_tensor("Sie", [F, TO], dt.float32r, kind="ExternalInput").ap()
    out2 = nc.dram_tensor("out2", [NB, C, T], dt.float32, kind="ExternalOutput").ap()

    v2 = nc.dram_tensor("v2", [NB * C, 2 * T], dt.bfloat16).ap()     # rolled-v table
    k2d = nc.dram_tensor("k2d", [NB * C, 2 * T], dt.float32).ap()    # k2 gather table

    with tile.TileContext(nc) as tc:
        from contextlib import ExitStack

        # ---- P0: resident constants ----
        es_const = ExitStack()
        cpool = es_const.enter_context(tc.tile_pool(name="consts", bufs=1, side="left"))
        cie_sb = cpool.tile([P, FC, 640], dt.float32r, tag="cie_sb")
        nc.sync.dma_start(cie_sb[:], Cie.rearrange("(n p) t -> p n t", p=P)[:, :, 0:640])
        sie_sb = cpool.tile([P, FC, 640], dt.float32r, tag="sie_sb")
        nc.sync.dma_start(sie_sb[:], Sie.rearrange("(n p) t -> p n t", p=P)[:, :, 0:640])
        wv_sb = cpool.tile([P, CC, C], dt.float32r, tag="wv_sb")
        nc.sync.dma_start(wv_sb[:], Wvr.rearrange("(n p) d -> p n d", p=P))
        wqr_sb = cpool.tile([P, CC, C], dt.float32r, tag="wqr_sb")
        nc.sync.dma_start(wqr_sb[:], Wqr.rearrange("(n p) d -> p n d", p=P))
        wkr_sb = cpool.tile([P, CC, C], dt.float32r, tag="wkr_sb")
        nc.sync.dma_start(wkr_sb[:], Wkr.rearrange("(n p) d -> p n d", p=P))

        for b in range(NB):
            # ============ P1: radix splits + f32r split-proj + v ============
            es_qk = ExitStack()
            qk_pool = es_qk.enter_context(tc.tile_pool(name=f"qk{b}", bufs=1, side="right"))
            qTee = qk_pool.tile([P, 5, C], dt.float32r, tag="qTee")
            qTeo = qk_pool.tile([P, 4, C], dt.float32r, tag="qTeo")
            qToo = qk_pool.tile([P, 4, C], dt.float32r, tag="qToo")
            qToe = qk_pool.tile([P, 5, C], dt.float32r, tag="qToe")
            kTee = qk_pool.tile([P, 5, C], dt.float32r, tag="kTee")
            kTeo = qk_pool.tile([P, 4, C], dt.float32r, tag="kTeo")
            kToo = qk_pool.tile([P, 4, C], dt.float32r, tag="kToo")
            kToe = qk_pool.tile([P, 5, C], dt.float32r, tag="kToe")

            with tc.tile_pool(name=f"a{b}", bufs=1) as ap_, \
                 tc.tile_pool(name=f"axs{b}", bufs=2) as axs, \
                 tc.tile_pool(name=f"aps{b}", bufs=3, space="PSUM") as aps:
                for name, srcx, w_sb, dsts in (
                        ("q", query2, wqr_sb, (qTee, qTeo, qToo, qToe)),
                        ("k", key2, wkr_sb, (kTee, kTeo, kToo, kToe))):
                    dee, deo, doo, doe = dsts
                    xee = ap_.tile([P, CC, 640], dt.float32r, tag="xee")
                    xeo = ap_.tile([P, CC, 512], dt.float32r, tag="xeo")
                    xoo = ap_.tile([P, CC, 512], dt.float32r, tag="xoo")
                    xoe = ap_.tile([P, CC, 640], dt.float32r, tag="xoe")
                    for cc in range(CC):
                        x_cc = axs.tile([P, T], dt.float32, tag="x_cc")
                        nc.sync.dma_start(
                            x_cc[:],
                            srcx[b].rearrange("(n p) t -> p n t", p=P)[:, cc, :])
                        ab = axs.tile([P, 2, 511], dt.float32, tag="ab")
                        x = x_cc
                        # f32r memset hits a walrus codegen bug; zero via ACT
                        nc.scalar.activation(xee[:, cc, 513:640], x[:, 0:127],
                                             AF.Copy, bias=0.0, scale=0.0)
                        nc.scalar.activation(xoe[:, cc, 513:640], x[:, 0:127],
                                             AF.Copy, bias=0.0, scale=0.0)
                        nc.scalar.activation(xoe[:, cc, 0:1], x[:, 0:1],
                                             AF.Copy, bias=0.0, scale=0.0)
                        nc.scalar.activation(xoo[:, cc, 0:1], x[:, 0:1],
                                             AF.Copy, bias=0.0, scale=0.0)
                        nc.vector.tensor_tensor(out=ab[:, 0, :], in0=x[:, 1:512],
                                                in1=x[:, T - 1:1536:-1], op=OP.add)
                        nc.vector.tensor_tensor(out=ab[:, 1, :], in0=x[:, 1023:512:-1],
                                                in1=x[:, 1025:1536], op=OP.add)
                        nc.vector.tensor_tensor(out=xee[:, cc, 1:512], in0=ab[:, 0, :],
                                                in1=ab[:, 1, :], op=OP.add)
                        nc.vector.tensor_tensor(out=xeo[:, cc, 1:512], in0=ab[:, 0, :],
                                                in1=ab[:, 1, :], op=OP.subtract)
                        nc.vector.tensor_tensor(out=ab[:, 0, :], in0=x[:, 1:512],
                                                in1=x[:, T - 1:1536:-1], op=OP.subtract)
                        nc.vector.tensor_tensor(out=ab[:, 1, :], in0=x[:, 1023:512:-1],
                                                in1=x[:, 1025:1536], op=OP.subtract)
                        nc.vector.tensor_tensor(out=xoo[:, cc, 1:512], in0=ab[:, 0, :],
                                                in1=ab[:, 1, :], op=OP.subtract)
                        nc.vector.tensor_tensor(out=xoe[:, cc, 1:512], in0=ab[:, 0, :],
                                                in1=ab[:, 1, :], op=OP.add)
                        nc.vector.tensor_tensor(out=xee[:, cc, 0:1], in0=x[:, 0:1],
                                                in1=x[:, H:H + 1], op=OP.add)
                        nc.vector.tensor_tensor(out=xeo[:, cc, 0:1], in0=x[:, 0:1],
                                                in1=x[:, H:H + 1], op=OP.subtract)
                        nc.vector.tensor_tensor(out=xee[:, cc, 512:513], in0=x[:, 512:513],
                                                in1=x[:, 1536:1537], op=OP.add)
                        nc.vector.tensor_tensor(out=xoe[:, cc, 512:513], in0=x[:, 512:513],
                                                in1=x[:, 1536:1537], op=OP.subtract)
                    for st_, dst, nch in ((xee, dee, 5), (xeo, deo, 4),
                                          (xoo, doo, 4), (xoe, doe, 5)):
                        for i in range(nch):
                            ps = aps.tile([P, C], dt.float32, tag="proj_ps")
                            for cc in range(CC):
                                nc.tensor.matmul(
                                    ps[:], st_[:, cc, bass.ts(i, P)],
                                    w_sb[:, cc, :],
                                    start=(cc == 0), stop=(cc == CC - 1))
                            nc.scalar.activation(dst[:, i, :], ps[:], AF.Copy)

            # ============ P2: forward DFT (f32r) + pointwise ============
            es_p = ExitStack()
            p_pool = es_p.enter_context(tc.tile_pool(name=f"p{b}", bufs=1, side="left"))
            pre = p_pool.tile([P, FC, C], dt.float32r, tag="pre")
            pim = p_pool.tile([P, FC, C], dt.float32r, tag="pim")
            with tc.tile_pool(name=f"bmat{b}", bufs=2) as bmat, \
                 tc.tile_pool(name=f"bps{b}", bufs=2, space="PSUM") as bps, \
                 tc.tile_pool(name=f"btmp{b}", bufs=2) as btmp:
                wree_r = Wree.rearrange("(n p) f -> p n f", p=P)
                wreo_r = Wreo.rearrange("(n p) f -> p n f", p=P)
                wime_r = Wime.rearrange("(n p) f -> p n f", p=P)
                wimo_r = Wimo.rearrange("(n p) f -> p n f", p=P)
                for fc in range(FC):
                    even = fc < 5
                    fl = fc if even else fc - 5
                    ncos, nsin = (5, 4) if even else (4, 5)
                    cm = bmat.tile([P, 5, P], dt.float32r, tag="cm")
                    nc.sync.dma_start(
                        cm[:, 0:ncos, :],
                        (wree_r if even else wreo_r)[:, :, bass.ts(fl, P)])
                    sm = bmat.tile([P, 5, P], dt.float32r, tag="sm")
                    nc.sync.dma_start(
                        sm[:, 0:nsin, :],
                        (wime_r if even else wimo_r)[:, :, bass.ts(fl, P)])
                    qcos = qTee if even else qTeo
                    qsin = qToo if even else qToe
                    kcos = kTee if even else kTeo
                    ksin = kToo if even else kToe
                    acc = {}
                    for nm, mat, sig, nchunk in (
                            ("aq", cm, qcos, ncos), ("bq", sm, qsin, nsin),
                            ("ak", cm, kcos, ncos), ("bk", sm, ksin, nsin)):
                        ps = bps.tile([P, C], dt.float32, tag=nm, name=f"ps_{nm}")
                        for i in range(nchunk):
                            nc.tensor.matmul(
                                ps[:], mat[:, i, :], sig[:, i, :],
                                start=(i == 0), stop=(i == nchunk - 1))
                        acc[nm] = ps
                    aqs = btmp.tile([P, C], dt.float32, tag="aqs")
                    nc.scalar.activation(aqs[:], acc["aq"][:], AF.Copy)
                    bqs = btmp.tile([P, C], dt.float32, tag="bqs")
                    nc.scalar.activation(bqs[:], acc["bq"][:], AF.Copy)
                    tmp = btmp.tile([P, C], dt.float32, tag="tmp")
                    nc.vector.tensor_tensor(
                        out=pre[:, fc, :], in0=aqs[:], in1=acc["ak"][:], op=OP.mult)
                    nc.vector.tensor_tensor(
                        out=tmp[:], in0=bqs[:], in1=acc["bk"][:], op=OP.mult)
                    nc.vector.tensor_tensor(
                        out=pre[:, fc, :], in0=pre[:, fc, :], in1=tmp[:], op=OP.add)
                    nc.vector.tensor_tensor(
                        out=pim[:, fc, :], in0=bqs[:], in1=acc["ak"][:], op=OP.mult)
                    tmp2 = btmp.tile([P, C], dt.float32, tag="tmp2")
                    nc.vector.tensor_tensor(
                        out=tmp2[:], in0=aqs[:], in1=acc["bk"][:], op=OP.mult)
                    nc.vector.tensor_tensor(
                        out=pim[:, fc, :], in0=pim[:, fc, :], in1=tmp2[:], op=OP.subtract)
            es_qk.close()

            # ============ P3: inverse DFT (f32r) + top-8 + denominator ======
            es_sel = ExitStack()
            sel_pool = es_sel.enter_context(
                tc.tile_pool(name=f"sel{b}", bufs=1, side="right"))
            idx8 = [sel_pool.tile([P, 8], dt.uint32, tag=f"idx8_{dc}",
                                  name=f"idx8_{b}_{dc}") for dc in range(CC)]
            negm = [sel_pool.tile([P, 1], dt.float32, tag=f"negm_{dc}",
                                  name=f"negm_{b}_{dc}") for dc in range(CC)]
            rs = [sel_pool.tile([P, 1], dt.float32, tag=f"rs_{dc}",
                                name=f"rs_{b}_{dc}") for dc in range(CC)]
            with tc.tile_pool(name=f"cr{b}", bufs=2) as crp, \
                 tc.tile_pool(name=f"ctmp{b}", bufs=2) as ctmp, \
                 tc.tile_pool(name=f"cps{b}", bufs=1, space="PSUM") as cps:
                HB = H // 2
                for cc in range(CC):
                    rcE = cps.tile([P, HB], dt.float32, tag="rcE", name="ps_rcE")
                    rcE2 = cps.tile([P, P], dt.float32, tag="rcE2", name="ps_rcE2")
                    rcO = cps.tile([P, HB], dt.float32, tag="rcO", name="ps_rcO")
                    rsE = cps.tile([P, HB], dt.float32, tag="rsE", name="ps_rsE")
                    rsO = cps.tile([P, HB], dt.float32, tag="rsO", name="ps_rsO")
                    rsO2 = cps.tile([P, P], dt.float32, tag="rsO2", name="ps_rsO2")
                    for fc in range(5):
                        st, sp = (fc == 0), (fc == 4)
                        pre_l = pre[:, fc, bass.ts(cc, P)]
                        pim_l = pim[:, fc, bass.ts(cc, P)]
                        nc.tensor.matmul(rcE[:], pre_l, cie_sb[:, fc, 0:HB],
                                         start=st, stop=sp)
                        nc.tensor.matmul(rcE2[:], pre_l, cie_sb[:, fc, HB:HB + P],
                                         start=st, stop=sp)
                        nc.tensor.matmul(rsE[:], pim_l, sie_sb[:, fc, 0:HB],
                                         start=st, stop=sp)
                    for fc in range(5, FC):
                        st, sp = (fc == 5), (fc == FC - 1)
                        pre_l = pre[:, fc, bass.ts(cc, P)]
                        pim_l = pim[:, fc, bass.ts(cc, P)]
                        nc.tensor.matmul(rcO[:], pre_l, cie_sb[:, fc, 0:HB],
                                         start=st, stop=sp)
                        nc.tensor.matmul(rsO[:], pim_l, sie_sb[:, fc, 0:HB],
                                         start=st, stop=sp)
                        nc.tensor.matmul(rsO2[:], pim_l, sie_sb[:, fc, HB:HB + P],
                                         start=st, stop=sp)
                    rcO_sb = ctmp.tile([P, HB], dt.float32, tag="rcO_sb")
                    nc.scalar.activation(rcO_sb[:], rcO[:], AF.Copy)
                    rsE_sb = ctmp.tile([P, HB], dt.float32, tag="rsE_sb")
                    nc.scalar.activation(rsE_sb[:], rsE[:], AF.Copy)
                    rsO_sb = ctmp.tile([P, HB + 1], dt.float32, tag="rsO_sb")
                    nc.scalar.activation(rsO_sb[:, 0:HB], rsO[:], AF.Copy)
                    nc.scalar.activation(rsO_sb[:, HB:HB + 1], rsO2[:, 0:1], AF.Copy)
                    s1 = ctmp.tile([P, HB], dt.float32, tag="s1")
                    nc.vector.tensor_tensor(out=s1[:], in0=rcE[:], in1=rcO_sb[:],
                                            op=OP.add)
                    s2 = ctmp.tile([P, HB], dt.float32, tag="s2")
                    nc.vector.tensor_tensor(out=s2[:], in0=rcE[:], in1=rcO_sb[:],
                                            op=OP.subtract)
                    w1 = ctmp.tile([P, HB], dt.float32, tag="w1")
                    nc.vector.tensor_tensor(out=w1[:], in0=rsE_sb[:],
                                            in1=rsO_sb[:, 0:HB], op=OP.add)
                    w2 = ctmp.tile([P, HB], dt.float32, tag="w2")
                    nc.vector.tensor_tensor(out=w2[:], in0=rsO_sb[:, 0:HB],
                                            in1=rsE_sb[:], op=OP.subtract)
                    rt = crp.tile([P, T], dt.float32, tag="rt")
                    nc.vector.tensor_tensor(out=rt[:, 0:HB], in0=s1[:], in1=w1[:],
                                            op=OP.add)
                    nc.vector.tensor_tensor(out=rt[:, 1023:HB:-1], in0=s2[:, 1:HB],
                                            in1=w2[:, 1:HB], op=OP.add)
                    nc.vector.tensor_tensor(out=rt[:, 1025:1536], in0=s2[:, 1:HB],
                                            in1=w2[:, 1:HB], op=OP.subtract)
                    nc.vector.tensor_tensor(out=rt[:, T - 1:1536:-1], in0=s1[:, 1:HB],
                                            in1=w1[:, 1:HB], op=OP.subtract)
                    nc.vector.tensor_tensor(out=rt[:, HB:HB + 1], in0=rcE2[:, 0:1],
                                            in1=rsO_sb[:, HB:HB + 1], op=OP.add)
                    nc.vector.tensor_tensor(out=rt[:, H:H + 1], in0=rcE[:, 0:1],
                                            in1=rcO_sb[:, 0:1], op=OP.subtract)
                    nc.vector.tensor_tensor(out=rt[:, 1536:1537], in0=rcE2[:, 0:1],
                                            in1=rsO_sb[:, HB:HB + 1], op=OP.subtract)
                    # top-8 + softmax denominator on approx r
                    vals = ctmp.tile([P, 8], dt.float32, tag="vals")
                    nc.vector.max(vals[:], rt[:])
                    nc.vector.max_index(idx8[cc][:], vals[:], rt[:])
                    nc.scalar.activation(negm[cc][:], vals[:, 0:1],
                                         AF.Copy, bias=0.0, scale=-1.0)
                    esc = crp.tile([P, T], dt.float32, tag="esc")
                    s_col = ctmp.tile([P, 1], dt.float32, tag="s_col")
                    nc.scalar.activation(
                        esc[:], rt[:], AF.Exp,
                        bias=negm[cc][:, 0:1], scale=1.0,
                        accum_out=s_col[:, 0:1])
                    nc.vector.reciprocal(rs[cc][:], s_col[:])
            es_p.close()

            # ============ P4: fp32 q2/k2 + exact refinement ============
            w3_t = [sel_pool.tile([P, K], dt.float32, tag=f"w3_{dc}",
                                  name=f"w3_{b}_{dc}") for dc in range(CC)]
            gov = [sel_pool.tile([P, K], dt.uint32, tag=f"gov_{dc}",
                                 name=f"gov_{b}_{dc}") for dc in range(CC)]
            with tc.tile_pool(name=f"d{b}", bufs=1) as dp, \
                 tc.tile_pool(name=f"dsc{b}", bufs=2) as dsc, \
                 tc.tile_pool(name=f"dks{b}", bufs=2) as dks, \
                 tc.tile_pool(name=f"dps{b}", bufs=2, space="PSUM") as dps:
                k2r = k2d.rearrange("(n p) w -> n p w", p=P)
                # P4a: k2 (fp32) -> DRAM doubled table
                xk_sb = dp.tile([P, CC, T], dt.float32, tag="x_p4s")
                nc.sync.dma_start(
                    xk_sb[:], key2[b].rearrange("(n p) t -> p n t", p=P))
                wk_sb = dp.tile([P, CC, C], dt.float32, tag="w_p4")
                nc.sync.dma_start(wk_sb[:], Wk.rearrange("(n p) d -> p n d", p=P))
                for dc in range(CC):
                    k2sb = dsc.tile([P, T], dt.float32, tag="k2sb")
                    for tb in range(4):
                        ps = dps.tile([P, T // 4], dt.float32, tag="p4ps")
                        for cc in range(CC):
                            nc.tensor.matmul(
                                ps[:], wk_sb[:, cc, bass.ts(dc, P)],
                                xk_sb[:, cc, bass.ts(tb, T // 4)],
                                start=(cc == 0), stop=(cc == CC - 1))
                        nc.scalar.activation(
                            k2sb[:, bass.ts(tb, T // 4)], ps[:], AF.Copy)
                    nc.sync.dma_start(k2r[b * CC + dc, :, 0:T], k2sb[:])
                    nc.sync.dma_start(k2r[b * CC + dc, :, T:2 * T], k2sb[:])
                # P4v: v projection (f32r) -> v2 DRAM table (bf16, doubled)
                xv_sb = dp.tile([P, CC, T], dt.float32r, tag="x_p4s")
                nc.sync.dma_start(
                    xv_sb[:], value2[b].rearrange("(n p) t -> p n t", p=P))
                v2r = v2.rearrange("(n p) w -> n p w", p=P)
                for dc in range(CC):
                    v_sb = dsc.tile([P, T], dt.bfloat16, tag="v_sb")
                    for tb in range(4):
                        ps = dps.tile([P, T // 4], dt.float32, tag="p4ps")
                        for cc in range(CC):
                            nc.tensor.matmul(
                                ps[:], wv_sb[:, cc, bass.ts(dc, P)],
                                xv_sb[:, cc, bass.ts(tb, T // 4)],
                                start=(cc == 0), stop=(cc == CC - 1))
                        nc.scalar.activation(
                            v_sb[:, bass.ts(tb, T // 4)], ps[:], AF.Copy)
                    nc.sync.dma_start(v2r[b * CC + dc, :, 0:T], v_sb[:])
                    nc.sync.dma_start(v2r[b * CC + dc, :, T:2 * T], v_sb[:])
                # P4b: q2 per dc + gathers + dots + selection
                xq_sb = dp.tile([P, CC, T], dt.float32, tag="x_p4s")
                nc.sync.dma_start(
                    xq_sb[:], query2[b].rearrange("(n p) t -> p n t", p=P))
                wq_sb = dp.tile([P, CC, C], dt.float32, tag="w_p4")
                nc.sync.dma_start(wq_sb[:], Wq.rearrange("(n p) d -> p n d", p=P))
                for dc in range(CC):
                    q2sb = dsc.tile([P, T], dt.float32, tag="q2sb")
                    for tb in range(4):
                        ps = dps.tile([P, T // 4], dt.float32, tag="p4ps")
                        for cc in range(CC):
                            nc.tensor.matmul(
                                ps[:], wq_sb[:, cc, bass.ts(dc, P)],
                                xq_sb[:, cc, bass.ts(tb, T // 4)],
                                start=(cc == 0), stop=(cc == CC - 1))
                        nc.scalar.activation(
                            q2sb[:, bass.ts(tb, T // 4)], ps[:], AF.Copy)

                    # gather offsets for M candidates:
                    #   (b*C+dc*128+p)*2T + T - lag_m
                    iot = dsc.tile([P, 1], dt.int32, tag="iot")
                    nc.gpsimd.iota(
                        iot[:], pattern=[[0, 1]],
                        base=(b * C + dc * P) * (2 * T) + T,
                        channel_multiplier=2 * T)
                    iot_f = dsc.tile([P, 1], dt.float32, tag="iot_f")
                    nc.vector.tensor_copy(iot_f[:], iot[:])
                    idxm_f = dsc.tile([P, M], dt.float32, tag="idxm_f")
                    nc.vector.tensor_copy(idxm_f[:], idx8[dc][:, 0:M])
                    gom = dsc.tile([P, M], dt.float32, tag="gom")
                    nc.scalar.activation(gom[:], idxm_f[:],
                                         AF.Copy, bias=0.0, scale=-1.0)
                    nc.vector.tensor_scalar_add(gom[:], gom[:], iot_f[:, 0:1])
                    gou = dsc.tile([P, M], dt.uint32, tag="gou")
                    nc.vector.tensor_copy(gou[:], gom[:])

                    refined = dsc.tile([P, 8], dt.float32, tag="refined")
                    nc.gpsimd.memset(refined[:, M:8], -3.0e38)
                    for m in range(M):
                        ksh = dks.tile([P, T], dt.float32, tag="ksh")
                        nc.gpsimd.indirect_dma_start(
                            out=ksh[:], out_offset=None,
                            in_=k2d[:, :],
                            in_offset=bass.IndirectOffsetOnAxis(
                                ap=gou[:, m:m + 1], axis=1),
                            element_offset=0)
                        scr = dks.tile([P, T], dt.float32, tag="scr")
                        nc.vector.tensor_tensor(
                            out=scr[:], in0=q2sb[:], in1=ksh[:], op=OP.mult)
                        scr2 = dks.tile([P, T], dt.float32, tag="scr2")
                        nc.scalar.activation(
                            scr2[:], scr[:], AF.Copy, scale=1.0 / T,
                            accum_out=refined[:, m:m + 1])

                    valr = dsc.tile([P, 8], dt.float32, tag="valr")
                    nc.vector.max(valr[:], refined[:])
                    pos8 = dsc.tile([P, 8], dt.uint32, tag="pos8")
                    nc.vector.max_index(pos8[:], valr[:], refined[:])
                    posf = dsc.tile([P, K], dt.float32, tag="posf")
                    nc.vector.tensor_copy(posf[:], pos8[:, 0:K])
                    # one-hot map: lag_sel[:, k] = sum_m idxm_f[:, m]*(posf==m)
                    lagf = dsc.tile([P, K], dt.float32, tag="lagf")
                    eqm = dsc.tile([P, K], dt.float32, tag="eqm")
                    contrib = dsc.tile([P, K], dt.float32, tag="contrib")
                    for m in range(M):
                        nc.vector.tensor_scalar(
                            out=eqm[:], in0=posf[:], scalar1=float(m),
                            scalar2=None, op0=OP.is_equal)
                        nc.vector.tensor_scalar_mul(
                            contrib[:], eqm[:], idxm_f[:, m:m + 1])
                        if m == 0:
                            nc.vector.tensor_copy(lagf[:], contrib[:])
                        else:
                            nc.vector.tensor_tensor(
                                out=lagf[:], in0=lagf[:], in1=contrib[:],
                                op=OP.add)
                    # weights: w3 = exp(valr[0:K] - m~) * rs
                    ew = dsc.tile([P, K], dt.float32, tag="ew")
                    nc.scalar.activation(ew[:], valr[:, 0:K],
                                         AF.Exp, bias=negm[dc][:, 0:1], scale=1.0)
                    nc.vector.tensor_scalar_mul(w3_t[dc][:], ew[:], rs[dc][:, 0:1])
                    # v2 gather offsets: rowbase + T - lag_sel
                    govf = dsc.tile([P, K], dt.float32, tag="govf")
                    nc.scalar.activation(govf[:], lagf[:],
                                         AF.Copy, bias=0.0, scale=-1.0)
                    nc.vector.tensor_scalar_add(govf[:], govf[:], iot_f[:, 0:1])
                    nc.vector.tensor_copy(gov[dc][:], govf[:])
            es_sel.close_later = None  # keep sel_pool until P5 end

            # ============ P5: agg gathers (bf16) + scale + E ============
            with tc.tile_pool(name=f"e{b}", bufs=1) as ep, \
                 tc.tile_pool(name=f"eagg{b}", bufs=1) as eagg, \
                 tc.tile_pool(name=f"eps{b}", bufs=8, space="PSUM") as eps:
                wf16 = ep.tile([P, NE, C], dt.bfloat16, tag="wf16")
                with tc.tile_pool(name=f"wfload{b}", bufs=1) as wfl:
                    wf32 = wfl.tile([P, NE, C], dt.float32, tag="wf32")
                    nc.sync.dma_start(wf32[:], Wf.rearrange("(n p) d -> p n d", p=P))
                    for j in range(NE):
                        nc.scalar.activation(wf16[:, j, :], wf32[:, j, :], AF.Copy)
                TQ = T // 4
                for tb in range(4):
                    agg = eagg.tile([P, NE, TQ], dt.bfloat16, tag="aggs",
                                    name="aggs", bufs=2)
                    for k in range(K):
                        for dc in range(CC):
                            j = k * CC + dc
                            nc.gpsimd.indirect_dma_start(
                                out=agg[:, j, :], out_offset=None,
                                in_=v2[:, :],
                                in_offset=bass.IndirectOffsetOnAxis(
                                    ap=gov[dc][:, k:k + 1], axis=1),
                                element_offset=tb * TQ)
                            nc.vector.tensor_scalar_mul(
                                agg[:, j, :], agg[:, j, :],
                                w3_t[dc][:, k:k + 1])
                    for dco in range(CC):
                        ps = eps.tile([P, TQ], dt.float32, tag="out_ps")
                        for j in range(NE):
                            nc.tensor.matmul(
                                ps[:], wf16[:, j, bass.ts(dco, P)],
                                agg[:, j, :],
                                start=(j == 0), stop=(j == NE - 1))
                        o_sb = ep.tile([P, TQ], dt.float32, tag="o_sb")
                        nc.scalar.activation(o_sb[:], ps[:], AF.Copy)
                        nc.sync.dma_start(
                            out2[b, bass.ts(dco, P), bass.ts(tb, TQ)], o_sb[:])
            es_sel.close()

        es_const.close()

    nc.compile()
    return nc


def _get_nc():
    if "nc" not in _CACHE:
        _CACHE["nc"] = _build()
    return _CACHE["nc"]


def kernel(query, key, value, Wq, bq, Wk, bk, Wv, bv, Wf, bf):
    query = np.ascontiguousarray(np.asarray(query, dtype=np.float32))
    key = np.ascontiguousarray(np.asarray(key, dtype=np.float32))
    value = np.ascontiguousarray(np.asarray(value, dtype=np.float32))
    for bias in (bq, bk, bv, bf):
        assert np.all(np.asarray(bias) == 0.0), "nonzero biases unsupported"

    if "mats" not in _CACHE:
        _CACHE["mats"] = _dft_matrices()
    wree, wreo, wime, wimo, cie, sie = _CACHE["mats"]

    Wqc = np.ascontiguousarray(np.asarray(Wq, np.float32))
    Wkc = np.ascontiguousarray(np.asarray(Wk, np.float32))
    shared = {
        "Wq": Wqc, "Wk": Wkc, "Wqr": Wqc, "Wkr": Wkc,
        "Wvr": np.ascontiguousarray(np.asarray(Wv, np.float32)),
        "Wf": np.ascontiguousarray(np.asarray(Wf, np.float32)),
        "Wree": wree, "Wreo": wreo, "Wime": wime, "Wimo": wimo,
        "Cie": cie, "Sie": sie,
    }
    in_maps = []
    for c in range(NCORES):
        sl = slice(c * NB, (c + 1) * NB)
        in_maps.append({
            "query2": query[sl], "key2": key[sl], "value2": value[sl], **shared})

    from concourse.bass_utils import run_bass_kernel_spmd
    nc = _get_nc()
    res = run_bass_kernel_spmd(nc, in_maps, core_ids=list(range(NCORES)))
    _CACHE["last_results"] = res
    out = np.concatenate([res.results[c]["out2"] for c in range(NCORES)], axis=0)
    return out.astype(np.float32)


# revision 13
# speedup vs baseline: 1.0176x; 1.0033x over previous
"""AutoCorrelation Bass kernel, refinement architecture (stage 2).

Per batch: correlation pipeline (projections, fwd DFT, pointwise, inv DFT)
runs in float32r (tf32-grade, 1 cyc/row on PE) and is used ONLY to select
top-M=5 candidate lags per channel plus the softmax denominator. The top-3
selection and softmax weights then come from EXACT fp32 time-domain dots
a[tau] = (1/T) sum_t q2[t] k2[t-tau], with q2/k2 from fp32 matmuls and the
circular k-shifts gathered from a DRAM table via per-partition indirect DMA.
Value path (v-proj f32r, agg/E in bf16) only affects output values (2e-2 rel
gate; flips cost ~1e-2 each so refined selection must match fp32 reference).
"""
import numpy as np

import concourse.bass as bass
import concourse.tile as tile
from concourse import bacc, mybir

dt = mybir.dt
AF = mybir.ActivationFunctionType
OP = mybir.AluOpType

P = 128
B, C, T, K = 16, 512, 2048, 3
NB = 2
NCORES = 8
F = 1152
TC = T // P
CC = C // P
FC = F // P
NE = K * C // P
TE = 1152
TEC = TE // P
TO = 1024
TOC = TO // P
H = T // 2
M = 5                     # refinement candidates per channel

_CACHE = {}


def _dft_matrices():
    """Radix-split DFT matrices (fp64 -> fp32). Same as baseline."""
    t640 = np.arange(640.0)[:, None]
    t512 = np.arange(512.0)[:, None]
    ge = np.arange(640.0)[None, :]
    go = np.arange(512.0)[None, :]
    wree = np.where((t640 <= 512) & (ge <= 512),
                    np.cos(2 * np.pi * t640 * (2 * ge) / T), 0.0).astype(np.float32)
    wreo = np.cos(2 * np.pi * t512 * (2 * go + 1) / T).astype(np.float32)
    wime = np.where(ge <= 512,
                    -np.sin(2 * np.pi * t512 * (2 * ge) / T), 0.0).astype(np.float32)
    wimo = np.where(t640 <= 512,
                    -np.sin(2 * np.pi * t640 * (2 * go + 1) / T), 0.0).astype(np.float32)

    f64 = np.arange(F, dtype=np.float64)[None, :]
    livef = f64 <= H
    w = np.where((f64 == 0) | (f64 == H), 1.0, 2.0) * livef / (T * T)
    fc_ = f64.T
    tt = np.arange(TE, dtype=np.float64)[None, :]
    cie = np.where((fc_ <= H) & (tt <= H),
                   np.cos(2 * np.pi * fc_ * tt / T) * w.T, 0.0)
    tt2 = np.arange(TO, dtype=np.float64)[None, :]
    sie = np.where(fc_ <= H,
                   -np.sin(2 * np.pi * fc_ * tt2 / T) * w.T, 0.0)

    def permrows(m):
        out = np.zeros_like(m)
        out[0:513] = m[0:1025:2]
        out[640:1152] = m[1:1024:2]
        return out

    return (wree, wreo, wime, wimo,
            permrows(cie).astype(np.float32), permrows(sie).astype(np.float32))


def _build():
    nc = bacc.Bacc("TRN2", target_bir_lowering=False, debug=False,
                   num_devices=NCORES)

    query2 = nc.dram_tensor("query2", [NB, C, T], dt.float32, kind="ExternalInput").ap()
    key2 = nc.dram_tensor("key2", [NB, C, T], dt.float32, kind="ExternalInput").ap()
    value2 = nc.dram_tensor("value2", [NB, C, T], dt.float32r, kind="ExternalInput").ap()
    Wq = nc.dram_tensor("Wq", [C, C], dt.float32, kind="ExternalInput").ap()
    Wk = nc.dram_tensor("Wk", [C, C], dt.float32, kind="ExternalInput").ap()
    Wqr = nc.dram_tensor("Wqr", [C, C], dt.float32r, kind="ExternalInput").ap()
    Wkr = nc.dram_tensor("Wkr", [C, C], dt.float32r, kind="ExternalInput").ap()
    Wvr = nc.dram_tensor("Wvr", [C, C], dt.float32r, kind="ExternalInput").ap()
    Wf = nc.dram_tensor("Wf", [K * C, C], dt.float32, kind="ExternalInput").ap()
    Wree = nc.dram_tensor("Wree", [640, 640], dt.float32r, kind="ExternalInput").ap()
    Wreo = nc.dram_tensor("Wreo", [512, 512], dt.float32r, kind="ExternalInput").ap()
    Wime = nc.dram_tensor("Wime", [512, 640], dt.float32r, kind="ExternalInput").ap()
    Wimo = nc.dram_tensor("Wimo", [640, 512], dt.float32r, kind="ExternalInput").ap()
    Cie = nc.dram_tensor("Cie", [F, TE], dt.float32r, kind="ExternalInput").ap()
    Sie = nc.dram_tensor("Sie", [F, TO], dt.float32r, kind="ExternalInput").ap()
    out2 = nc.dram_tensor("out2", [NB, C, T], dt.float32, kind="ExternalOutput").ap()

    v2 = nc.dram_tensor("v2", [NB * C, 2 * T], dt.bfloat16).ap()     # rolled-v table
    k2d = nc.dram_tensor("k2d", [NB * C, 2 * T], dt.float32).ap()    # k2 gather table

    with tile.TileContext(nc) as tc:
        from contextlib import ExitStack

        # ---- P0: resident constants ----
        es_const = ExitStack()
        cpool = es_const.enter_context(tc.tile_pool(name="consts", bufs=1, side="left"))
        cie_sb = cpool.tile([P, FC, 640], dt.float32r, tag="cie_sb")
        nc.sync.dma_start(cie_sb[:], Cie.rearrange("(n p) t -> p n t", p=P)[:, :, 0:640])
        sie_sb = cpool.tile([P, FC, 640], dt.float32r, tag="sie_sb")
        nc.sync.dma_start(sie_sb[:], Sie.rearrange("(n p) t -> p n t", p=P)[:, :, 0:640])
        wv_sb = cpool.tile([P, CC, C], dt.float32r, tag="wv_sb")
        nc.sync.dma_start(wv_sb[:], Wvr.rearrange("(n p) d -> p n d", p=P))
        wqr_sb = cpool.tile([P, CC, C], dt.float32r, tag="wqr_sb")
        nc.sync.dma_start(wqr_sb[:], Wqr.rearrange("(n p) d -> p n d", p=P))
        wkr_sb = cpool.tile([P, CC, C], dt.float32r, tag="wkr_sb")
        nc.sync.dma_start(wkr_sb[:], Wkr.rearrange("(n p) d -> p n d", p=P))

        for b in range(NB):
            # ============ P1: radix splits + f32r split-proj + v ============
            es_qk = ExitStack()
            qk_pool = es_qk.enter_context(tc.tile_pool(name=f"qk{b}", bufs=1, side="right"))
            qTee = qk_pool.tile([P, 5, C], dt.float32r, tag="qTee")
            qTeo = qk_pool.tile([P, 4, C], dt.float32r, tag="qTeo")
            qToo = qk_pool.tile([P, 4, C], dt.float32r, tag="qToo")
            qToe = qk_pool.tile([P, 5, C], dt.float32r, tag="qToe")
            kTee = qk_pool.tile([P, 5, C], dt.float32r, tag="kTee")
            kTeo = qk_pool.tile([P, 4, C], dt.float32r, tag="kTeo")
            kToo = qk_pool.tile([P, 4, C], dt.float32r, tag="kToo")
            kToe = qk_pool.tile([P, 5, C], dt.float32r, tag="kToe")

            with tc.tile_pool(name=f"a{b}", bufs=1) as ap_, \
                 tc.tile_pool(name=f"axs{b}", bufs=2) as axs, \
                 tc.tile_pool(name=f"aps{b}", bufs=3, space="PSUM") as aps:
                for name, srcx, w_sb, dsts in (
                        ("q", query2, wqr_sb, (qTee, qTeo, qToo, qToe)),
                        ("k", key2, wkr_sb, (kTee, kTeo, kToo, kToe))):
                    dee, deo, doo, doe = dsts
                    xee = ap_.tile([P, CC, 640], dt.float32r, tag="xee")
                    xeo = ap_.tile([P, CC, 512], dt.float32r, tag="xeo")
                    xoo = ap_.tile([P, CC, 512], dt.float32r, tag="xoo")
                    xoe = ap_.tile([P, CC, 640], dt.float32r, tag="xoe")
                    for cc in range(CC):
                        x_cc = axs.tile([P, T], dt.float32, tag="x_cc")
                        nc.sync.dma_start(
                            x_cc[:],
                            srcx[b].rearrange("(n p) t -> p n t", p=P)[:, cc, :])
                        ab = axs.tile([P, 2, 511], dt.float32, tag="ab")
                        x = x_cc
                        # f32r memset hits a walrus codegen bug; zero via ACT
                        nc.scalar.activation(xee[:, cc, 513:640], x[:, 0:127],
                                             AF.Copy, bias=0.0, scale=0.0)
                        nc.scalar.activation(xoe[:, cc, 513:640], x[:, 0:127],
                                             AF.Copy, bias=0.0, scale=0.0)
                        nc.scalar.activation(xoe[:, cc, 0:1], x[:, 0:1],
                                             AF.Copy, bias=0.0, scale=0.0)
                        nc.scalar.activation(xoo[:, cc, 0:1], x[:, 0:1],
                                             AF.Copy, bias=0.0, scale=0.0)
                        nc.vector.tensor_tensor(out=ab[:, 0, :], in0=x[:, 1:512],
                                                in1=x[:, T - 1:1536:-1], op=OP.add)
                        nc.vector.tensor_tensor(out=ab[:, 1, :], in0=x[:, 1023:512:-1],
                                                in1=x[:, 1025:1536], op=OP.add)
                        nc.vector.tensor_tensor(out=xee[:, cc, 1:512], in0=ab[:, 0, :],
                                                in1=ab[:, 1, :], op=OP.add)
                        nc.vector.tensor_tensor(out=xeo[:, cc, 1:512], in0=ab[:, 0, :],
                                                in1=ab[:, 1, :], op=OP.subtract)
                        nc.vector.tensor_tensor(out=ab[:, 0, :], in0=x[:, 1:512],
                                                in1=x[:, T - 1:1536:-1], op=OP.subtract)
                        nc.vector.tensor_tensor(out=ab[:, 1, :], in0=x[:, 1023:512:-1],
                                                in1=x[:, 1025:1536], op=OP.subtract)
                        nc.vector.tensor_tensor(out=xoo[:, cc, 1:512], in0=ab[:, 0, :],
                                                in1=ab[:, 1, :], op=OP.subtract)
                        nc.vector.tensor_tensor(out=xoe[:, cc, 1:512], in0=ab[:, 0, :],
                                                in1=ab[:, 1, :], op=OP.add)
                        nc.vector.tensor_tensor(out=xee[:, cc, 0:1], in0=x[:, 0:1],
                                                in1=x[:, H:H + 1], op=OP.add)
                        nc.vector.tensor_tensor(out=xeo[:, cc, 0:1], in0=x[:, 0:1],
                                                in1=x[:, H:H + 1], op=OP.subtract)
                        nc.vector.tensor_tensor(out=xee[:, cc, 512:513], in0=x[:, 512:513],
                                                in1=x[:, 1536:1537], op=OP.add)
                        nc.vector.tensor_tensor(out=xoe[:, cc, 512:513], in0=x[:, 512:513],
                                                in1=x[:, 1536:1537], op=OP.subtract)
                    for st_, dst, nch in ((xee, dee, 5), (xeo, deo, 4),
                                          (xoo, doo, 4), (xoe, doe, 5)):
                        for i in range(nch):
                            ps = aps.tile([P, C], dt.float32, tag="proj_ps")
                            for cc in range(CC):
                                nc.tensor.matmul(
                                    ps[:], st_[:, cc, bass.ts(i, P)],
                                    w_sb[:, cc, :],
                                    start=(cc == 0), stop=(cc == CC - 1))
                            nc.scalar.activation(dst[:, i, :], ps[:], AF.Copy)

            # ============ P2: forward DFT (f32r) + pointwise ============
            es_p = ExitStack()
            p_pool = es_p.enter_context(tc.tile_pool(name=f"p{b}", bufs=1, side="left"))
            pre = p_pool.tile([P, FC, C], dt.float32r, tag="pre")
            pim = p_pool.tile([P, FC, C], dt.float32r, tag="pim")
            with tc.tile_pool(name=f"bmat{b}", bufs=2) as bmat, \
                 tc.tile_pool(name=f"bps{b}", bufs=2, space="PSUM") as bps, \
                 tc.tile_pool(name=f"btmp{b}", bufs=2) as btmp:
                wree_r = Wree.rearrange("(n p) f -> p n f", p=P)
                wreo_r = Wreo.rearrange("(n p) f -> p n f", p=P)
                wime_r = Wime.rearrange("(n p) f -> p n f", p=P)
                wimo_r = Wimo.rearrange("(n p) f -> p n f", p=P)
                for fc in range(FC):
                    even = fc < 5
                    fl = fc if even else fc - 5
                    ncos, nsin = (5, 4) if even else (4, 5)
                    cm = bmat.tile([P, 5, P], dt.float32r, tag="cm")
                    nc.sync.dma_start(
                        cm[:, 0:ncos, :],
                        (wree_r if even else wreo_r)[:, :, bass.ts(fl, P)])
                    sm = bmat.tile([P, 5, P], dt.float32r, tag="sm")
                    nc.sync.dma_start(
                        sm[:, 0:nsin, :],
                        (wime_r if even else wimo_r)[:, :, bass.ts(fl, P)])
                    qcos = qTee if even else qTeo
                    qsin = qToo if even else qToe
                    kcos = kTee if even else kTeo
                    ksin = kToo if even else kToe
                    acc = {}
                    for nm, mat, sig, nchunk in (
                            ("aq", cm, qcos, ncos), ("bq", sm, qsin, nsin),
                            ("ak", cm, kcos, ncos), ("bk", sm, ksin, nsin)):
                        ps = bps.tile([P, C], dt.float32, tag=nm, name=f"ps_{nm}")
                        for i in range(nchunk):
                            nc.tensor.matmul(
                                ps[:], mat[:, i, :], sig[:, i, :],
                                start=(i == 0), stop=(i == nchunk - 1))
                        acc[nm] = ps
                    aqs = btmp.tile([P, C], dt.float32, tag="aqs")
                    nc.scalar.activation(aqs[:], acc["aq"][:], AF.Copy)
                    bqs = btmp.tile([P, C], dt.float32, tag="bqs")
                    nc.scalar.activation(bqs[:], acc["bq"][:], AF.Copy)
                    tmp = btmp.tile([P, C], dt.float32, tag="tmp")
                    nc.vector.tensor_tensor(
                        out=pre[:, fc, :], in0=aqs[:], in1=acc["ak"][:], op=OP.mult)
                    nc.vector.tensor_tensor(
                        out=tmp[:], in0=bqs[:], in1=acc["bk"][:], op=OP.mult)
                    nc.vector.tensor_tensor(
                        out=pre[:, fc, :], in0=pre[:, fc, :], in1=tmp[:], op=OP.add)
                    nc.vector.tensor_tensor(
                        out=pim[:, fc, :], in0=bqs[:], in1=acc["ak"][:], op=OP.mult)
                    tmp2 = btmp.tile([P, C], dt.float32, tag="tmp2")
                    nc.vector.tensor_tensor(
                        out=tmp2[:], in0=aqs[:], in1=acc["bk"][:], op=OP.mult)
                    nc.vector.tensor_tensor(
                        out=pim[:, fc, :], in0=pim[:, fc, :], in1=tmp2[:], op=OP.subtract)
            es_qk.close()

            # ============ P3: inverse DFT (f32r) + top-8 + denominator ======
            es_sel = ExitStack()
            sel_pool = es_sel.enter_context(
                tc.tile_pool(name=f"sel{b}", bufs=1, side="right"))
            idx8 = [sel_pool.tile([P, 8], dt.uint32, tag=f"idx8_{dc}",
                                  name=f"idx8_{b}_{dc}") for dc in range(CC)]
            negm = [sel_pool.tile([P, 1], dt.float32, tag=f"negm_{dc}",
                                  name=f"negm_{b}_{dc}") for dc in range(CC)]
            rs = [sel_pool.tile([P, 1], dt.float32, tag=f"rs_{dc}",
                                name=f"rs_{b}_{dc}") for dc in range(CC)]
            with tc.tile_pool(name=f"cr{b}", bufs=2) as crp, \
                 tc.tile_pool(name=f"ctmp{b}", bufs=2) as ctmp, \
                 tc.tile_pool(name=f"cps{b}", bufs=1, space="PSUM") as cps:
                HB = H // 2
                for cc in range(CC):
                    rcE = cps.tile([P, HB], dt.float32, tag="rcE", name="ps_rcE")
                    rcE2 = cps.tile([P, P], dt.float32, tag="rcE2", name="ps_rcE2")
                    rcO = cps.tile([P, HB], dt.float32, tag="rcO", name="ps_rcO")
                    rsE = cps.tile([P, HB], dt.float32, tag="rsE", name="ps_rsE")
                    rsO = cps.tile([P, HB], dt.float32, tag="rsO", name="ps_rsO")
                    rsO2 = cps.tile([P, P], dt.float32, tag="rsO2", name="ps_rsO2")
                    for fc in range(5):
                        st, sp = (fc == 0), (fc == 4)
                        pre_l = pre[:, fc, bass.ts(cc, P)]
                        pim_l = pim[:, fc, bass.ts(cc, P)]
                        nc.tensor.matmul(rcE[:], pre_l, cie_sb[:, fc, 0:HB],
                                         start=st, stop=sp)
                        nc.tensor.matmul(rcE2[:], pre_l, cie_sb[:, fc, HB:HB + P],
                                         start=st, stop=sp)
                        nc.tensor.matmul(rsE[:], pim_l, sie_sb[:, fc, 0:HB],
                                         start=st, stop=sp)
                    for fc in range(5, FC):
                        st, sp = (fc == 5), (fc == FC - 1)
                        pre_l = pre[:, fc, bass.ts(cc, P)]
                        pim_l = pim[:, fc, bass.ts(cc, P)]
                        nc.tensor.matmul(rcO[:], pre_l, cie_sb[:, fc, 0:HB],
                                         start=st, stop=sp)
                        nc.tensor.matmul(rsO[:], pim_l, sie_sb[:, fc, 0:HB],
                                         start=st, stop=sp)
                        nc.tensor.matmul(rsO2[:], pim_l, sie_sb[:, fc, HB:HB + P],
                                         start=st, stop=sp)
                    rcO_sb = ctmp.tile([P, HB], dt.float32, tag="rcO_sb")
                    nc.scalar.activation(rcO_sb[:], rcO[:], AF.Copy)
                    rsE_sb = ctmp.tile([P, HB], dt.float32, tag="rsE_sb")
                    nc.scalar.activation(rsE_sb[:], rsE[:], AF.Copy)
                    rsO_sb = ctmp.tile([P, HB + 1], dt.float32, tag="rsO_sb")
                    nc.scalar.activation(rsO_sb[:, 0:HB], rsO[:], AF.Copy)
                    nc.scalar.activation(rsO_sb[:, HB:HB + 1], rsO2[:, 0:1], AF.Copy)
                    s1 = ctmp.tile([P, HB], dt.float32, tag="s1")
                    nc.vector.tensor_tensor(out=s1[:], in0=rcE[:], in1=rcO_sb[:],
                                            op=OP.add)
                    s2 = ctmp.tile([P, HB], dt.float32, tag="s2")
                    nc.vector.tensor_tensor(out=s2[:], in0=rcE[:], in1=rcO_sb[:],
                                            op=OP.subtract)
                    w1 = ctmp.tile([P, HB], dt.float32, tag="w1")
                    nc.vector.tensor_tensor(out=w1[:], in0=rsE_sb[:],
                                            in1=rsO_sb[:, 0:HB], op=OP.add)
                    w2 = ctmp.tile([P, HB], dt.float32, tag="w2")
                    nc.vector.tensor_tensor(out=w2[:], in0=rsO_sb[:, 0:HB],
                                            in1=rsE_sb[:], op=OP.subtract)
                    rt = crp.tile([P, T], dt.float32, tag="rt")
                    nc.vector.tensor_tensor(out=rt[:, 0:HB], in0=s1[:], in1=w1[:],
                                            op=OP.add)
                    nc.vector.tensor_tensor(out=rt[:, 1023:HB:-1], in0=s2[:, 1:HB],
                                            in1=w2[:, 1:HB], op=OP.add)
                    nc.vector.tensor_tensor(out=rt[:, 1025:1536], in0=s2[:, 1:HB],
                                            in1=w2[:, 1:HB], op=OP.subtract)
                    nc.vector.tensor_tensor(out=rt[:, T - 1:1536:-1], in0=s1[:, 1:HB],
                                            in1=w1[:, 1:HB], op=OP.subtract)
                    nc.vector.tensor_tensor(out=rt[:, HB:HB + 1], in0=rcE2[:, 0:1],
                                            in1=rsO_sb[:, HB:HB + 1], op=OP.add)
                    nc.vector.tensor_tensor(out=rt[:, H:H + 1], in0=rcE[:, 0:1],
                                            in1=rcO_sb[:, 0:1], op=OP.subtract)
                    nc.vector.tensor_tensor(out=rt[:, 1536:1537], in0=rcE2[:, 0:1],
                                            in1=rsO_sb[:, HB:HB + 1], op=OP.subtract)
                    # top-8 + softmax denominator on approx r
                    vals = ctmp.tile([P, 8], dt.float32, tag="vals")
                    nc.vector.max(vals[:], rt[:])
                    nc.vector.max_index(idx8[cc][:], vals[:], rt[:])
                    nc.scalar.activation(negm[cc][:], vals[:, 0:1],
                                         AF.Copy, bias=0.0, scale=-1.0)
                    esc = crp.tile([P, T], dt.float32, tag="esc")
                    s_col = ctmp.tile([P, 1], dt.float32, tag="s_col")
                    nc.scalar.activation(
                        esc[:], rt[:], AF.Exp,
                        bias=negm[cc][:, 0:1], scale=1.0,
                        accum_out=s_col[:, 0:1])
                    nc.vector.reciprocal(rs[cc][:], s_col[:])
            es_p.close()

            # ============ P4: fp32 q2/k2 + exact refinement ============
            w3_t = [sel_pool.tile([P, K], dt.float32, tag=f"w3_{dc}",
                                  name=f"w3_{b}_{dc}") for dc in range(CC)]
            gov = [sel_pool.tile([P, K], dt.uint32, tag=f"gov_{dc}",
                                 name=f"gov_{b}_{dc}") for dc in range(CC)]
            with tc.tile_pool(name=f"d{b}", bufs=1) as dp, \
                 tc.tile_pool(name=f"dsc{b}", bufs=2) as dsc, \
                 tc.tile_pool(name=f"dks{b}", bufs=2) as dks, \
                 tc.tile_pool(name=f"dps{b}", bufs=2, space="PSUM") as dps:
                k2r = k2d.rearrange("(n p) w -> n p w", p=P)
                # P4a: k2 (fp32) -> DRAM doubled table
                xk_sb = dp.tile([P, CC, T], dt.float32, tag="x_p4s")
                nc.sync.dma_start(
                    xk_sb[:], key2[b].rearrange("(n p) t -> p n t", p=P))
                wk_sb = dp.tile([P, CC, C], dt.float32, tag="w_p4")
                nc.sync.dma_start(wk_sb[:], Wk.rearrange("(n p) d -> p n d", p=P))
                for dc in range(CC):
                    k2sb = dsc.tile([P, T], dt.float32, tag="k2sb")
                    for tb in range(4):
                        ps = dps.tile([P, T // 4], dt.float32, tag="p4ps")
                        for cc in range(CC):
                            nc.tensor.matmul(
                                ps[:], wk_sb[:, cc, bass.ts(dc, P)],
                                xk_sb[:, cc, bass.ts(tb, T // 4)],
                                start=(cc == 0), stop=(cc == CC - 1))
                        nc.scalar.activation(
                            k2sb[:, bass.ts(tb, T // 4)], ps[:], AF.Copy)
                    nc.sync.dma_start(k2r[b * CC + dc, :, 0:T], k2sb[:])
                    nc.sync.dma_start(k2r[b * CC + dc, :, T:2 * T], k2sb[:])
                # P4v: v projection (f32r) -> v2 DRAM table (bf16, doubled)
                xv_sb = dp.tile([P, CC, T], dt.float32r, tag="x_p4s")
                nc.sync.dma_start(
                    xv_sb[:], value2[b].rearrange("(n p) t -> p n t", p=P))
                v2r = v2.rearrange("(n p) w -> n p w", p=P)
                for dc in range(CC):
                    v_sb = dsc.tile([P, T], dt.bfloat16, tag="v_sb")
                    for tb in range(4):
                        ps = dps.tile([P, T // 4], dt.float32, tag="p4ps")
                        for cc in range(CC):
                            nc.tensor.matmul(
                                ps[:], wv_sb[:, cc, bass.ts(dc, P)],
                                xv_sb[:, cc, bass.ts(tb, T // 4)],
                                start=(cc == 0), stop=(cc == CC - 1))
                        nc.scalar.activation(
                            v_sb[:, bass.ts(tb, T // 4)], ps[:], AF.Copy)
                    nc.sync.dma_start(v2r[b * CC + dc, :, 0:T], v_sb[:])
                    nc.sync.dma_start(v2r[b * CC + dc, :, T:2 * T], v_sb[:])
                # P4b: q2 per dc + gathers + dots + selection
                xq_sb = dp.tile([P, CC, T], dt.float32, tag="x_p4s")
                nc.sync.dma_start(
                    xq_sb[:], query2[b].rearrange("(n p) t -> p n t", p=P))
                wq_sb = dp.tile([P, CC, C], dt.float32, tag="w_p4")
                nc.sync.dma_start(wq_sb[:], Wq.rearrange("(n p) d -> p n d", p=P))
                for dc in range(CC):
                    q2sb = dsc.tile([P, T], dt.float32, tag="q2sb")
                    for tb in range(4):
                        ps = dps.tile([P, T // 4], dt.float32, tag="p4ps")
                        for cc in range(CC):
                            nc.tensor.matmul(
                                ps[:], wq_sb[:, cc, bass.ts(dc, P)],
                                xq_sb[:, cc, bass.ts(tb, T // 4)],
                                start=(cc == 0), stop=(cc == CC - 1))
                        nc.scalar.activation(
                            q2sb[:, bass.ts(tb, T // 4)], ps[:], AF.Copy)

                    # gather offsets for M candidates:
                    #   (b*C+dc*128+p)*2T + T - lag_m
                    iot = dsc.tile([P, 1], dt.int32, tag="iot")
                    nc.gpsimd.iota(
                        iot[:], pattern=[[0, 1]],
                        base=(b * C + dc * P) * (2 * T) + T,
                        channel_multiplier=2 * T)
                    iot_f = dsc.tile([P, 1], dt.float32, tag="iot_f")
                    nc.vector.tensor_copy(iot_f[:], iot[:])
                    idxm_f = dsc.tile([P, M], dt.float32, tag="idxm_f")
                    nc.vector.tensor_copy(idxm_f[:], idx8[dc][:, 0:M])
                    gom = dsc.tile([P, M], dt.float32, tag="gom")
                    nc.scalar.activation(gom[:], idxm_f[:],
                                         AF.Copy, bias=0.0, scale=-1.0)
                    nc.vector.tensor_scalar_add(gom[:], gom[:], iot_f[:, 0:1])
                    gou = dsc.tile([P, M], dt.uint32, tag="gou")
                    nc.vector.tensor_copy(gou[:], gom[:])

                    refined = dsc.tile([P, 8], dt.float32, tag="refined")
                    nc.gpsimd.memset(refined[:, M:8], -3.0e38)
                    for m in range(M):
                        ksh = dks.tile([P, T], dt.float32, tag="ksh")
                        nc.gpsimd.indirect_dma_start(
                            out=ksh[:], out_offset=None,
                            in_=k2d[:, :],
                            in_offset=bass.IndirectOffsetOnAxis(
                                ap=gou[:, m:m + 1], axis=1),
                            element_offset=0)
                        scr = dks.tile([P, T], dt.float32, tag="scr")
                        nc.vector.tensor_tensor(
                            out=scr[:], in0=q2sb[:], in1=ksh[:], op=OP.mult)
                        scr2 = dks.tile([P, T], dt.float32, tag="scr2")
                        nc.scalar.activation(
                            scr2[:], scr[:], AF.Copy, scale=1.0 / T,
                            accum_out=refined[:, m:m + 1])

                    valr = dsc.tile([P, 8], dt.float32, tag="valr")
                    nc.vector.max(valr[:], refined[:])
                    pos8 = dsc.tile([P, 8], dt.uint32, tag="pos8")
                    nc.vector.max_index(pos8[:], valr[:], refined[:])
                    posf = dsc.tile([P, K], dt.float32, tag="posf")
                    nc.vector.tensor_copy(posf[:], pos8[:, 0:K])
                    # one-hot map: lag_sel[:, k] = sum_m idxm_f[:, m]*(posf==m)
                    lagf = dsc.tile([P, K], dt.float32, tag="lagf")
                    eqm = dsc.tile([P, K], dt.float32, tag="eqm")
                    contrib = dsc.tile([P, K], dt.float32, tag="contrib")
                    for m in range(M):
                        nc.vector.tensor_scalar(
                            out=eqm[:], in0=posf[:], scalar1=float(m),
                            scalar2=None, op0=OP.is_equal)
                        nc.vector.tensor_scalar_mul(
                            contrib[:], eqm[:], idxm_f[:, m:m + 1])
                        if m == 0:
                            nc.vector.tensor_copy(lagf[:], contrib[:])
                        else:
                            nc.vector.tensor_tensor(
                                out=lagf[:], in0=lagf[:], in1=contrib[:],
                                op=OP.add)
                    # weights: w3 = exp(valr[0:K] - m~) * rs
                    ew = dsc.tile([P, K], dt.float32, tag="ew")
                    nc.scalar.activation(ew[:], valr[:, 0:K],
                                         AF.Exp, bias=negm[dc][:, 0:1], scale=1.0)
                    nc.vector.tensor_scalar_mul(w3_t[dc][:], ew[:], rs[dc][:, 0:1])
                    # v2 gather offsets: rowbase + T - lag_sel
                    govf = dsc.tile([P, K], dt.float32, tag="govf")
                    nc.scalar.activation(govf[:], lagf[:],
                                         AF.Copy, bias=0.0, scale=-1.0)
                    nc.vector.tensor_scalar_add(govf[:], govf[:], iot_f[:, 0:1])
                    nc.vector.tensor_copy(gov[dc][:], govf[:])
            es_sel.close_later = None  # keep sel_pool until P5 end

            # ============ P5: agg gathers (bf16) + scale + E ============
            with tc.tile_pool(name=f"e{b}", bufs=1) as ep, \
                 tc.tile_pool(name=f"eagg{b}", bufs=3) as eagg, \
                 tc.tile_pool(name=f"eps{b}", bufs=8, space="PSUM") as eps:
                wf16 = ep.tile([P, NE, C], dt.bfloat16, tag="wf16")
                with tc.tile_pool(name=f"wfload{b}", bufs=1) as wfl:
                    wf32 = wfl.tile([P, NE, C], dt.float32, tag="wf32")
                    nc.sync.dma_start(wf32[:], Wf.rearrange("(n p) d -> p n d", p=P))
                    for j in range(NE):
                        nc.scalar.activation(wf16[:, j, :], wf32[:, j, :], AF.Copy)
                TQ = T // 4
                for tb in range(4):
                    agg = eagg.tile([P, NE, TQ], dt.bfloat16, tag="aggs",
                                    name="aggs", bufs=3)
                    for k in range(K):
                        for dc in range(CC):
                            j = k * CC + dc
                            nc.gpsimd.indirect_dma_start(
                                out=agg[:, j, :], out_offset=None,
                                in_=v2[:, :],
                                in_offset=bass.IndirectOffsetOnAxis(
                                    ap=gov[dc][:, k:k + 1], axis=1),
                                element_offset=tb * TQ)
                            nc.vector.tensor_scalar_mul(
                                agg[:, j, :], agg[:, j, :],
                                w3_t[dc][:, k:k + 1])
                    for dco in range(CC):
                        ps = eps.tile([P, TQ], dt.float32, tag="out_ps")
                        for j in range(NE):
                            nc.tensor.matmul(
                                ps[:], wf16[:, j, bass.ts(dco, P)],
                                agg[:, j, :],
                                start=(j == 0), stop=(j == NE - 1))
                        o_sb = ep.tile([P, TQ], dt.float32, tag="o_sb")
                        nc.scalar.activation(o_sb[:], ps[:], AF.Copy)
                        nc.sync.dma_start(
                            out2[b, bass.ts(dco, P), bass.ts(tb, TQ)], o_sb[:])
            es_sel.close()

        es_const.close()

    nc.compile()
    return nc


def _get_nc():
    if "nc" not in _CACHE:
        _CACHE["nc"] = _build()
    return _CACHE["nc"]


def kernel(query, key, value, Wq, bq, Wk, bk, Wv, bv, Wf, bf):
    query = np.ascontiguousarray(np.asarray(query, dtype=np.float32))
    key = np.ascontiguousarray(np.asarray(key, dtype=np.float32))
    value = np.ascontiguousarray(np.asarray(value, dtype=np.float32))
    for bias in (bq, bk, bv, bf):
        assert np.all(np.asarray(bias) == 0.0), "nonzero biases unsupported"

    if "mats" not in _CACHE:
        _CACHE["mats"] = _dft_matrices()
    wree, wreo, wime, wimo, cie, sie = _CACHE["mats"]

    Wqc = np.ascontiguousarray(np.asarray(Wq, np.float32))
    Wkc = np.ascontiguousarray(np.asarray(Wk, np.float32))
    shared = {
        "Wq": Wqc, "Wk": Wkc, "Wqr": Wqc, "Wkr": Wkc,
        "Wvr": np.ascontiguousarray(np.asarray(Wv, np.float32)),
        "Wf": np.ascontiguousarray(np.asarray(Wf, np.float32)),
        "Wree": wree, "Wreo": wreo, "Wime": wime, "Wimo": wimo,
        "Cie": cie, "Sie": sie,
    }
    in_maps = []
    for c in range(NCORES):
        sl = slice(c * NB, (c + 1) * NB)
        in_maps.append({
            "query2": query[sl], "key2": key[sl], "value2": value[sl], **shared})

    from concourse.bass_utils import run_bass_kernel_spmd
    nc = _get_nc()
    res = run_bass_kernel_spmd(nc, in_maps, core_ids=list(range(NCORES)))
    _CACHE["last_results"] = res
    out = np.concatenate([res.results[c]["out2"] for c in range(NCORES)], axis=0)
    return out.astype(np.float32)


# revision 17
# speedup vs baseline: 1.0436x; 1.0255x over previous
"""AutoCorrelation Bass kernel, refinement architecture (stage 2).

Per batch: correlation pipeline (projections, fwd DFT, pointwise, inv DFT)
runs in float32r (tf32-grade, 1 cyc/row on PE) and is used ONLY to select
top-M=5 candidate lags per channel plus the softmax denominator. The top-3
selection and softmax weights then come from EXACT fp32 time-domain dots
a[tau] = (1/T) sum_t q2[t] k2[t-tau], with q2/k2 from fp32 matmuls and the
circular k-shifts gathered from a DRAM table via per-partition indirect DMA.
Value path (v-proj f32r, agg/E in bf16) only affects output values (2e-2 rel
gate; flips cost ~1e-2 each so refined selection must match fp32 reference).
"""
import numpy as np

import concourse.bass as bass
import concourse.tile as tile
from concourse import bacc, mybir

dt = mybir.dt
AF = mybir.ActivationFunctionType
OP = mybir.AluOpType

P = 128
B, C, T, K = 16, 512, 2048, 3
NB = 2
NCORES = 8
F = 1152
TC = T // P
CC = C // P
FC = F // P
NE = K * C // P
TE = 1152
TEC = TE // P
TO = 1024
TOC = TO // P
H = T // 2
M = 5                     # refinement candidates per channel

_CACHE = {}


def _dft_matrices():
    """Radix-split DFT matrices (fp64 -> fp32). Same as baseline."""
    t640 = np.arange(640.0)[:, None]
    t512 = np.arange(512.0)[:, None]
    ge = np.arange(640.0)[None, :]
    go = np.arange(512.0)[None, :]
    wree = np.where((t640 <= 512) & (ge <= 512),
                    np.cos(2 * np.pi * t640 * (2 * ge) / T), 0.0).astype(np.float32)
    wreo = np.cos(2 * np.pi * t512 * (2 * go + 1) / T).astype(np.float32)
    wime = np.where(ge <= 512,
                    -np.sin(2 * np.pi * t512 * (2 * ge) / T), 0.0).astype(np.float32)
    wimo = np.where(t640 <= 512,
                    -np.sin(2 * np.pi * t640 * (2 * go + 1) / T), 0.0).astype(np.float32)

    f64 = np.arange(F, dtype=np.float64)[None, :]
    livef = f64 <= H
    w = np.where((f64 == 0) | (f64 == H), 1.0, 2.0) * livef / (T * T)
    fc_ = f64.T
    tt = np.arange(TE, dtype=np.float64)[None, :]
    cie = np.where((fc_ <= H) & (tt <= H),
                   np.cos(2 * np.pi * fc_ * tt / T) * w.T, 0.0)
    tt2 = np.arange(TO, dtype=np.float64)[None, :]
    sie = np.where(fc_ <= H,
                   -np.sin(2 * np.pi * fc_ * tt2 / T) * w.T, 0.0)

    def permrows(m):
        out = np.zeros_like(m)
        out[0:513] = m[0:1025:2]
        out[640:1152] = m[1:1024:2]
        return out

    return (wree, wreo, wime, wimo,
            permrows(cie).astype(np.float32), permrows(sie).astype(np.float32))


def _build():
    nc = bacc.Bacc("TRN2", target_bir_lowering=False, debug=False,
                   num_devices=NCORES)

    query2 = nc.dram_tensor("query2", [NB, C, T], dt.float32, kind="ExternalInput").ap()
    key2 = nc.dram_tensor("key2", [NB, C, T], dt.float32, kind="ExternalInput").ap()
    value2 = nc.dram_tensor("value2", [NB, C, T], dt.float32r, kind="ExternalInput").ap()
    Wq = nc.dram_tensor("Wq", [C, C], dt.float32, kind="ExternalInput").ap()
    Wk = nc.dram_tensor("Wk", [C, C], dt.float32, kind="ExternalInput").ap()
    Wqr = nc.dram_tensor("Wqr", [C, C], dt.float32r, kind="ExternalInput").ap()
    Wkr = nc.dram_tensor("Wkr", [C, C], dt.float32r, kind="ExternalInput").ap()
    Wvr = nc.dram_tensor("Wvr", [C, C], dt.float32r, kind="ExternalInput").ap()
    Wf = nc.dram_tensor("Wf", [K * C, C], dt.float32, kind="ExternalInput").ap()
    Wree = nc.dram_tensor("Wree", [640, 640], dt.float32r, kind="ExternalInput").ap()
    Wreo = nc.dram_tensor("Wreo", [512, 512], dt.float32r, kind="ExternalInput").ap()
    Wime = nc.dram_tensor("Wime", [512, 640], dt.float32r, kind="ExternalInput").ap()
    Wimo = nc.dram_tensor("Wimo", [640, 512], dt.float32r, kind="ExternalInput").ap()
    Cie = nc.dram_tensor("Cie", [F, TE], dt.float32r, kind="ExternalInput").ap()
    Sie = nc.dram_tensor("Sie", [F, TO], dt.float32r, kind="ExternalInput").ap()
    out2 = nc.dram_tensor("out2", [NB, C, T], dt.float32, kind="ExternalOutput").ap()

    v2 = nc.dram_tensor("v2", [NB * C, 2 * T], dt.bfloat16).ap()     # rolled-v table
    k2d = nc.dram_tensor("k2d", [NB * C, 2 * T], dt.float32).ap()    # k2 gather table

    with tile.TileContext(nc) as tc:
        from contextlib import ExitStack

        # ---- P0: resident constants ----
        es_const = ExitStack()
        cpool = es_const.enter_context(tc.tile_pool(name="consts", bufs=1, side="left"))
        cie_sb = cpool.tile([P, FC, 640], dt.float32r, tag="cie_sb")
        sie_sb = cpool.tile([P, FC, 640], dt.float32r, tag="sie_sb")
        wv_sb = cpool.tile([P, CC, C], dt.float32r, tag="wv_sb")
        wqr_sb = cpool.tile([P, CC, C], dt.float32r, tag="wqr_sb")
        nc.sync.dma_start(wqr_sb[:], Wqr.rearrange("(n p) d -> p n d", p=P))
        wkr_sb = cpool.tile([P, CC, C], dt.float32r, tag="wkr_sb")
        nc.sync.dma_start(wkr_sb[:], Wkr.rearrange("(n p) d -> p n d", p=P))

        for b in range(NB):
            # ============ P1: radix splits + f32r split-proj + v ============
            es_qk = ExitStack()
            qk_pool = es_qk.enter_context(tc.tile_pool(name=f"qk{b}", bufs=1, side="right"))
            qTee = qk_pool.tile([P, 5, C], dt.float32r, tag="qTee")
            qTeo = qk_pool.tile([P, 4, C], dt.float32r, tag="qTeo")
            qToo = qk_pool.tile([P, 4, C], dt.float32r, tag="qToo")
            qToe = qk_pool.tile([P, 5, C], dt.float32r, tag="qToe")
            kTee = qk_pool.tile([P, 5, C], dt.float32r, tag="kTee")
            kTeo = qk_pool.tile([P, 4, C], dt.float32r, tag="kTeo")
            kToo = qk_pool.tile([P, 4, C], dt.float32r, tag="kToo")
            kToe = qk_pool.tile([P, 5, C], dt.float32r, tag="kToe")

            with tc.tile_pool(name=f"a{b}", bufs=1) as ap_, \
                 tc.tile_pool(name=f"axs{b}", bufs=2) as axs, \
                 tc.tile_pool(name=f"aps{b}", bufs=3, space="PSUM") as aps:
                for name, srcx, w_sb, dsts in (
                        ("q", query2, wqr_sb, (qTee, qTeo, qToo, qToe)),
                        ("k", key2, wkr_sb, (kTee, kTeo, kToo, kToe))):
                    dee, deo, doo, doe = dsts
                    xee = ap_.tile([P, CC, 640], dt.float32r, tag="xee")
                    xeo = ap_.tile([P, CC, 512], dt.float32r, tag="xeo")
                    xoo = ap_.tile([P, CC, 512], dt.float32r, tag="xoo")
                    xoe = ap_.tile([P, CC, 640], dt.float32r, tag="xoe")
                    for cc in range(CC):
                        x_cc = axs.tile([P, T], dt.float32, tag="x_cc")
                        nc.sync.dma_start(
                            x_cc[:],
                            srcx[b].rearrange("(n p) t -> p n t", p=P)[:, cc, :])
                        ab = axs.tile([P, 2, 511], dt.float32, tag="ab")
                        x = x_cc
                        # f32r memset hits a walrus codegen bug; zero via ACT
                        nc.scalar.activation(xee[:, cc, 513:640], x[:, 0:127],
                                             AF.Copy, bias=0.0, scale=0.0)
                        nc.scalar.activation(xoe[:, cc, 513:640], x[:, 0:127],
                                             AF.Copy, bias=0.0, scale=0.0)
                        nc.scalar.activation(xoe[:, cc, 0:1], x[:, 0:1],
                                             AF.Copy, bias=0.0, scale=0.0)
                        nc.scalar.activation(xoo[:, cc, 0:1], x[:, 0:1],
                                             AF.Copy, bias=0.0, scale=0.0)
                        nc.vector.tensor_tensor(out=ab[:, 0, :], in0=x[:, 1:512],
                                                in1=x[:, T - 1:1536:-1], op=OP.add)
                        nc.vector.tensor_tensor(out=ab[:, 1, :], in0=x[:, 1023:512:-1],
                                                in1=x[:, 1025:1536], op=OP.add)
                        nc.vector.tensor_tensor(out=xee[:, cc, 1:512], in0=ab[:, 0, :],
                                                in1=ab[:, 1, :], op=OP.add)
                        nc.vector.tensor_tensor(out=xeo[:, cc, 1:512], in0=ab[:, 0, :],
                                                in1=ab[:, 1, :], op=OP.subtract)
                        nc.vector.tensor_tensor(out=ab[:, 0, :], in0=x[:, 1:512],
                                                in1=x[:, T - 1:1536:-1], op=OP.subtract)
                        nc.vector.tensor_tensor(out=ab[:, 1, :], in0=x[:, 1023:512:-1],
                                                in1=x[:, 1025:1536], op=OP.subtract)
                        nc.vector.tensor_tensor(out=xoo[:, cc, 1:512], in0=ab[:, 0, :],
                                                in1=ab[:, 1, :], op=OP.subtract)
                        nc.vector.tensor_tensor(out=xoe[:, cc, 1:512], in0=ab[:, 0, :],
                                                in1=ab[:, 1, :], op=OP.add)
                        nc.vector.tensor_tensor(out=xee[:, cc, 0:1], in0=x[:, 0:1],
                                                in1=x[:, H:H + 1], op=OP.add)
                        nc.vector.tensor_tensor(out=xeo[:, cc, 0:1], in0=x[:, 0:1],
                                                in1=x[:, H:H + 1], op=OP.subtract)
                        nc.vector.tensor_tensor(out=xee[:, cc, 512:513], in0=x[:, 512:513],
                                                in1=x[:, 1536:1537], op=OP.add)
                        nc.vector.tensor_tensor(out=xoe[:, cc, 512:513], in0=x[:, 512:513],
                                                in1=x[:, 1536:1537], op=OP.subtract)
                    for st_, dst, nch in ((xee, dee, 5), (xeo, deo, 4),
                                          (xoo, doo, 4), (xoe, doe, 5)):
                        for i in range(nch):
                            ps = aps.tile([P, C], dt.float32, tag="proj_ps")
                            for cc in range(CC):
                                nc.tensor.matmul(
                                    ps[:], st_[:, cc, bass.ts(i, P)],
                                    w_sb[:, cc, :],
                                    start=(cc == 0), stop=(cc == CC - 1))
                            nc.scalar.activation(dst[:, i, :], ps[:], AF.Copy)

            if b == 0:
                # deferred const loads: issued after P1's input DMAs so the
                # head of the in-order DMA queue feeds the splits first
                nc.sync.dma_start(
                    cie_sb[:], Cie.rearrange("(n p) t -> p n t", p=P)[:, :, 0:640])
                nc.sync.dma_start(
                    sie_sb[:], Sie.rearrange("(n p) t -> p n t", p=P)[:, :, 0:640])
                nc.sync.dma_start(wv_sb[:], Wvr.rearrange("(n p) d -> p n d", p=P))
            # ============ P2: forward DFT (f32r) + pointwise ============
            es_p = ExitStack()
            p_pool = es_p.enter_context(tc.tile_pool(name=f"p{b}", bufs=1, side="left"))
            pre = p_pool.tile([P, FC, C], dt.float32r, tag="pre")
            pim = p_pool.tile([P, FC, C], dt.float32r, tag="pim")
            with tc.tile_pool(name=f"bmat{b}", bufs=2) as bmat, \
                 tc.tile_pool(name=f"bps{b}", bufs=2, space="PSUM") as bps, \
                 tc.tile_pool(name=f"btmp{b}", bufs=2) as btmp:
                wree_r = Wree.rearrange("(n p) f -> p n f", p=P)
                wreo_r = Wreo.rearrange("(n p) f -> p n f", p=P)
                wime_r = Wime.rearrange("(n p) f -> p n f", p=P)
                wimo_r = Wimo.rearrange("(n p) f -> p n f", p=P)
                for fc in range(FC):
                    even = fc < 5
                    fl = fc if even else fc - 5
                    ncos, nsin = (5, 4) if even else (4, 5)
                    cm = bmat.tile([P, 5, P], dt.float32r, tag="cm")
                    nc.sync.dma_start(
                        cm[:, 0:ncos, :],
                        (wree_r if even else wreo_r)[:, :, bass.ts(fl, P)])
                    sm = bmat.tile([P, 5, P], dt.float32r, tag="sm")
                    nc.sync.dma_start(
                        sm[:, 0:nsin, :],
                        (wime_r if even else wimo_r)[:, :, bass.ts(fl, P)])
                    qcos = qTee if even else qTeo
                    qsin = qToo if even else qToe
                    kcos = kTee if even else kTeo
                    ksin = kToo if even else kToe
                    acc = {}
                    for nm, mat, sig, nchunk in (
                            ("aq", cm, qcos, ncos), ("bq", sm, qsin, nsin),
                            ("ak", cm, kcos, ncos), ("bk", sm, ksin, nsin)):
                        ps = bps.tile([P, C], dt.float32, tag=nm, name=f"ps_{nm}")
                        for i in range(nchunk):
                            nc.tensor.matmul(
                                ps[:], mat[:, i, :], sig[:, i, :],
                                start=(i == 0), stop=(i == nchunk - 1))
                        acc[nm] = ps
                    aqs = btmp.tile([P, C], dt.float32, tag="aqs")
                    nc.scalar.activation(aqs[:], acc["aq"][:], AF.Copy)
                    bqs = btmp.tile([P, C], dt.float32, tag="bqs")
                    nc.scalar.activation(bqs[:], acc["bq"][:], AF.Copy)
                    tmp = btmp.tile([P, C], dt.float32, tag="tmp")
                    nc.vector.tensor_tensor(
                        out=pre[:, fc, :], in0=aqs[:], in1=acc["ak"][:], op=OP.mult)
                    nc.vector.tensor_tensor(
                        out=tmp[:], in0=bqs[:], in1=acc["bk"][:], op=OP.mult)
                    nc.vector.tensor_tensor(
                        out=pre[:, fc, :], in0=pre[:, fc, :], in1=tmp[:], op=OP.add)
                    nc.vector.tensor_tensor(
                        out=pim[:, fc, :], in0=bqs[:], in1=acc["ak"][:], op=OP.mult)
                    tmp2 = btmp.tile([P, C], dt.float32, tag="tmp2")
                    nc.vector.tensor_tensor(
                        out=tmp2[:], in0=aqs[:], in1=acc["bk"][:], op=OP.mult)
                    nc.vector.tensor_tensor(
                        out=pim[:, fc, :], in0=pim[:, fc, :], in1=tmp2[:], op=OP.subtract)
            es_qk.close()

            # ============ P3: inverse DFT (f32r) + top-8 + denominator ======
            es_sel = ExitStack()
            sel_pool = es_sel.enter_context(
                tc.tile_pool(name=f"sel{b}", bufs=1, side="right"))
            idx8 = [sel_pool.tile([P, 8], dt.uint32, tag=f"idx8_{dc}",
                                  name=f"idx8_{b}_{dc}") for dc in range(CC)]
            negm = [sel_pool.tile([P, 1], dt.float32, tag=f"negm_{dc}",
                                  name=f"negm_{b}_{dc}") for dc in range(CC)]
            rs = [sel_pool.tile([P, 1], dt.float32, tag=f"rs_{dc}",
                                name=f"rs_{b}_{dc}") for dc in range(CC)]
            with tc.tile_pool(name=f"cr{b}", bufs=2) as crp, \
                 tc.tile_pool(name=f"ctmp{b}", bufs=2) as ctmp, \
                 tc.tile_pool(name=f"cps{b}", bufs=1, space="PSUM") as cps:
                HB = H // 2
                for cc in range(CC):
                    rcE = cps.tile([P, HB], dt.float32, tag="rcE", name="ps_rcE")
                    rcE2 = cps.tile([P, P], dt.float32, tag="rcE2", name="ps_rcE2")
                    rcO = cps.tile([P, HB], dt.float32, tag="rcO", name="ps_rcO")
                    rsE = cps.tile([P, HB], dt.float32, tag="rsE", name="ps_rsE")
                    rsO = cps.tile([P, HB], dt.float32, tag="rsO", name="ps_rsO")
                    rsO2 = cps.tile([P, P], dt.float32, tag="rsO2", name="ps_rsO2")
                    for fc in range(5):
                        st, sp = (fc == 0), (fc == 4)
                        pre_l = pre[:, fc, bass.ts(cc, P)]
                        pim_l = pim[:, fc, bass.ts(cc, P)]
                        nc.tensor.matmul(rcE[:], pre_l, cie_sb[:, fc, 0:HB],
                                         start=st, stop=sp)
                        nc.tensor.matmul(rcE2[:], pre_l, cie_sb[:, fc, HB:HB + P],
                                         start=st, stop=sp)
                        nc.tensor.matmul(rsE[:], pim_l, sie_sb[:, fc, 0:HB],
                                         start=st, stop=sp)
                    for fc in range(5, FC):
                        st, sp = (fc == 5), (fc == FC - 1)
                        pre_l = pre[:, fc, bass.ts(cc, P)]
                        pim_l = pim[:, fc, bass.ts(cc, P)]
                        nc.tensor.matmul(rcO[:], pre_l, cie_sb[:, fc, 0:HB],
                                         start=st, stop=sp)
                        nc.tensor.matmul(rsO[:], pim_l, sie_sb[:, fc, 0:HB],
                                         start=st, stop=sp)
                        nc.tensor.matmul(rsO2[:], pim_l, sie_sb[:, fc, HB:HB + P],
                                         start=st, stop=sp)
                    rcO_sb = ctmp.tile([P, HB], dt.float32, tag="rcO_sb")
                    nc.scalar.activation(rcO_sb[:], rcO[:], AF.Copy)
                    rsE_sb = ctmp.tile([P, HB], dt.float32, tag="rsE_sb")
                    nc.scalar.activation(rsE_sb[:], rsE[:], AF.Copy)
                    rsO_sb = ctmp.tile([P, HB + 1], dt.float32, tag="rsO_sb")
                    nc.scalar.activation(rsO_sb[:, 0:HB], rsO[:], AF.Copy)
                    nc.scalar.activation(rsO_sb[:, HB:HB + 1], rsO2[:, 0:1], AF.Copy)
                    s1 = ctmp.tile([P, HB], dt.float32, tag="s1")
                    nc.vector.tensor_tensor(out=s1[:], in0=rcE[:], in1=rcO_sb[:],
                                            op=OP.add)
                    s2 = ctmp.tile([P, HB], dt.float32, tag="s2")
                    nc.vector.tensor_tensor(out=s2[:], in0=rcE[:], in1=rcO_sb[:],
                                            op=OP.subtract)
                    w1 = ctmp.tile([P, HB], dt.float32, tag="w1")
                    nc.vector.tensor_tensor(out=w1[:], in0=rsE_sb[:],
                                            in1=rsO_sb[:, 0:HB], op=OP.add)
                    w2 = ctmp.tile([P, HB], dt.float32, tag="w2")
                    nc.vector.tensor_tensor(out=w2[:], in0=rsO_sb[:, 0:HB],
                                            in1=rsE_sb[:], op=OP.subtract)
                    rt = crp.tile([P, T], dt.float32, tag="rt")
                    nc.vector.tensor_tensor(out=rt[:, 0:HB], in0=s1[:], in1=w1[:],
                                            op=OP.add)
                    nc.vector.tensor_tensor(out=rt[:, 1023:HB:-1], in0=s2[:, 1:HB],
                                            in1=w2[:, 1:HB], op=OP.add)
                    nc.vector.tensor_tensor(out=rt[:, 1025:1536], in0=s2[:, 1:HB],
                                            in1=w2[:, 1:HB], op=OP.subtract)
                    nc.vector.tensor_tensor(out=rt[:, T - 1:1536:-1], in0=s1[:, 1:HB],
                                            in1=w1[:, 1:HB], op=OP.subtract)
                    nc.vector.tensor_tensor(out=rt[:, HB:HB + 1], in0=rcE2[:, 0:1],
                                            in1=rsO_sb[:, HB:HB + 1], op=OP.add)
                    nc.vector.tensor_tensor(out=rt[:, H:H + 1], in0=rcE[:, 0:1],
                                            in1=rcO_sb[:, 0:1], op=OP.subtract)
                    nc.vector.tensor_tensor(out=rt[:, 1536:1537], in0=rcE2[:, 0:1],
                                            in1=rsO_sb[:, HB:HB + 1], op=OP.subtract)
                    # top-8 + softmax denominator on approx r
                    vals = ctmp.tile([P, 8], dt.float32, tag="vals")
                    nc.vector.max(vals[:], rt[:])
                    nc.vector.max_index(idx8[cc][:], vals[:], rt[:])
                    nc.scalar.activation(negm[cc][:], vals[:, 0:1],
                                         AF.Copy, bias=0.0, scale=-1.0)
                    esc = crp.tile([P, T], dt.float32, tag="esc")
                    s_col = ctmp.tile([P, 1], dt.float32, tag="s_col")
                    nc.scalar.activation(
                        esc[:], rt[:], AF.Exp,
                        bias=negm[cc][:, 0:1], scale=1.0,
                        accum_out=s_col[:, 0:1])
                    nc.vector.reciprocal(rs[cc][:], s_col[:])
            es_p.close()

            # ============ P4: fp32 q2/k2 + exact refinement ============
            w3_t = [sel_pool.tile([P, K], dt.float32, tag=f"w3_{dc}",
                                  name=f"w3_{b}_{dc}") for dc in range(CC)]
            gov = [sel_pool.tile([P, K], dt.uint32, tag=f"gov_{dc}",
                                 name=f"gov_{b}_{dc}") for dc in range(CC)]
            with tc.tile_pool(name=f"d{b}", bufs=1) as dp, \
                 tc.tile_pool(name=f"dsc{b}", bufs=2) as dsc, \
                 tc.tile_pool(name=f"dks{b}", bufs=3) as dks, \
                 tc.tile_pool(name=f"dk2{b}", bufs=1) as dk2, \
                 tc.tile_pool(name=f"dps{b}", bufs=2, space="PSUM") as dps:
                k2r = k2d.rearrange("(n p) w -> n p w", p=P)
                # P4a: k2 (fp32) -> DRAM doubled table
                xk_sb = dp.tile([P, CC, T], dt.float32, tag="x_p4s")
                nc.sync.dma_start(
                    xk_sb[:], key2[b].rearrange("(n p) t -> p n t", p=P))
                wk_sb = dp.tile([P, CC, C], dt.float32, tag="w_p4")
                nc.sync.dma_start(wk_sb[:], Wk.rearrange("(n p) d -> p n d", p=P))
                for dc in range(CC):
                    k2sb = dsc.tile([P, T], dt.float32, tag="k2sb")
                    for tb in range(4):
                        ps = dps.tile([P, T // 4], dt.float32, tag="p4ps")
                        for cc in range(CC):
                            nc.tensor.matmul(
                                ps[:], wk_sb[:, cc, bass.ts(dc, P)],
                                xk_sb[:, cc, bass.ts(tb, T // 4)],
                                start=(cc == 0), stop=(cc == CC - 1))
                        nc.scalar.activation(
                            k2sb[:, bass.ts(tb, T // 4)], ps[:], AF.Copy)
                    nc.sync.dma_start(k2r[b * CC + dc, :, 0:T], k2sb[:])
                    nc.sync.dma_start(k2r[b * CC + dc, :, T:2 * T], k2sb[:])
                # P4v: v projection (f32r) -> v2 DRAM table (bf16, doubled)
                xv_sb = dp.tile([P, CC, T], dt.float32r, tag="x_p4s")
                nc.sync.dma_start(
                    xv_sb[:], value2[b].rearrange("(n p) t -> p n t", p=P))
                v2r = v2.rearrange("(n p) w -> n p w", p=P)
                for dc in range(CC):
                    v_sb = dsc.tile([P, T], dt.bfloat16, tag="v_sb")
                    for tb in range(4):
                        ps = dps.tile([P, T // 4], dt.float32, tag="p4ps")
                        for cc in range(CC):
                            nc.tensor.matmul(
                                ps[:], wv_sb[:, cc, bass.ts(dc, P)],
                                xv_sb[:, cc, bass.ts(tb, T // 4)],
                                start=(cc == 0), stop=(cc == CC - 1))
                        nc.scalar.activation(
                            v_sb[:, bass.ts(tb, T // 4)], ps[:], AF.Copy)
                    nc.sync.dma_start(v2r[b * CC + dc, :, 0:T], v_sb[:])
                    nc.sync.dma_start(v2r[b * CC + dc, :, T:2 * T], v_sb[:])
                # P4b: q2 per dc + gathers + dots + selection
                xq_sb = dp.tile([P, CC, T], dt.float32, tag="x_p4s")
                nc.sync.dma_start(
                    xq_sb[:], query2[b].rearrange("(n p) t -> p n t", p=P))
                wq_sb = dp.tile([P, CC, C], dt.float32, tag="w_p4")
                nc.sync.dma_start(wq_sb[:], Wq.rearrange("(n p) d -> p n d", p=P))
                for dc in range(CC):
                    q2sb = dsc.tile([P, T], dt.float32, tag="q2sb")
                    for tb in range(4):
                        ps = dps.tile([P, T // 4], dt.float32, tag="p4ps")
                        for cc in range(CC):
                            nc.tensor.matmul(
                                ps[:], wq_sb[:, cc, bass.ts(dc, P)],
                                xq_sb[:, cc, bass.ts(tb, T // 4)],
                                start=(cc == 0), stop=(cc == CC - 1))
                        nc.scalar.activation(
                            q2sb[:, bass.ts(tb, T // 4)], ps[:], AF.Copy)

                    # gather offsets for M candidates:
                    #   (b*C+dc*128+p)*2T + T - lag_m
                    iot = dsc.tile([P, 1], dt.int32, tag="iot")
                    nc.gpsimd.iota(
                        iot[:], pattern=[[0, 1]],
                        base=(b * C + dc * P) * (2 * T) + T,
                        channel_multiplier=2 * T)
                    iot_f = dsc.tile([P, 1], dt.float32, tag="iot_f")
                    nc.vector.tensor_copy(iot_f[:], iot[:])
                    idxm_f = dsc.tile([P, M], dt.float32, tag="idxm_f")
                    nc.vector.tensor_copy(idxm_f[:], idx8[dc][:, 0:M])
                    gom = dsc.tile([P, M], dt.float32, tag="gom")
                    nc.scalar.activation(gom[:], idxm_f[:],
                                         AF.Copy, bias=0.0, scale=-1.0)
                    nc.vector.tensor_scalar_add(gom[:], gom[:], iot_f[:, 0:1])
                    gou = dsc.tile([P, M], dt.uint32, tag="gou")
                    nc.vector.tensor_copy(gou[:], gom[:])

                    refined = dsc.tile([P, 8], dt.float32, tag="refined")
                    nc.gpsimd.memset(refined[:, M:8], -3.0e38)
                    for m in range(M):
                        ksh = dks.tile([P, T], dt.float32, tag="ksh")
                        nc.gpsimd.indirect_dma_start(
                            out=ksh[:], out_offset=None,
                            in_=k2d[:, :],
                            in_offset=bass.IndirectOffsetOnAxis(
                                ap=gou[:, m:m + 1], axis=1),
                            element_offset=0)
                        scr = dks.tile([P, T], dt.float32, tag="scr")
                        nc.vector.tensor_tensor(
                            out=scr[:], in0=q2sb[:], in1=ksh[:], op=OP.mult)
                        scr2 = dk2.tile([P, T], dt.float32, tag="scr2")
                        nc.scalar.activation(
                            scr2[:], scr[:], AF.Copy, scale=1.0 / T,
                            accum_out=refined[:, m:m + 1])

                    valr = dsc.tile([P, 8], dt.float32, tag="valr")
                    nc.vector.max(valr[:], refined[:])
                    pos8 = dsc.tile([P, 8], dt.uint32, tag="pos8")
                    nc.vector.max_index(pos8[:], valr[:], refined[:])
                    posf = dsc.tile([P, K], dt.float32, tag="posf")
                    nc.vector.tensor_copy(posf[:], pos8[:, 0:K])
                    # one-hot map: lag_sel[:, k] = sum_m idxm_f[:, m]*(posf==m)
                    lagf = dsc.tile([P, K], dt.float32, tag="lagf")
                    eqm = dsc.tile([P, K], dt.float32, tag="eqm")
                    contrib = dsc.tile([P, K], dt.float32, tag="contrib")
                    for m in range(M):
                        nc.vector.tensor_scalar(
                            out=eqm[:], in0=posf[:], scalar1=float(m),
                            scalar2=None, op0=OP.is_equal)
                        nc.vector.tensor_scalar_mul(
                            contrib[:], eqm[:], idxm_f[:, m:m + 1])
                        if m == 0:
                            nc.vector.tensor_copy(lagf[:], contrib[:])
                        else:
                            nc.vector.tensor_tensor(
                                out=lagf[:], in0=lagf[:], in1=contrib[:],
                                op=OP.add)
                    # weights: w3 = exp(valr[0:K] - m~) * rs
                    ew = dsc.tile([P, K], dt.float32, tag="ew")
                    nc.scalar.activation(ew[:], valr[:, 0:K],
                                         AF.Exp, bias=negm[dc][:, 0:1], scale=1.0)
                    nc.vector.tensor_scalar_mul(w3_t[dc][:], ew[:], rs[dc][:, 0:1])
                    # v2 gather offsets: rowbase + T - lag_sel
                    govf = dsc.tile([P, K], dt.float32, tag="govf")
                    nc.scalar.activation(govf[:], lagf[:],
                                         AF.Copy, bias=0.0, scale=-1.0)
                    nc.vector.tensor_scalar_add(govf[:], govf[:], iot_f[:, 0:1])
                    nc.vector.tensor_copy(gov[dc][:], govf[:])
            es_sel.close_later = None  # keep sel_pool until P5 end

            # ============ P5: agg gathers (bf16) + scale + E ============
            with tc.tile_pool(name=f"e{b}", bufs=1) as ep, \
                 tc.tile_pool(name=f"eagg{b}", bufs=3) as eagg, \
                 tc.tile_pool(name=f"eps{b}", bufs=8, space="PSUM") as eps:
                wf16 = ep.tile([P, NE, C], dt.bfloat16, tag="wf16")
                with tc.tile_pool(name=f"wfload{b}", bufs=1) as wfl:
                    wf32 = wfl.tile([P, NE, C], dt.float32, tag="wf32")
                    nc.sync.dma_start(wf32[:], Wf.rearrange("(n p) d -> p n d", p=P))
                    for j in range(NE):
                        nc.scalar.activation(wf16[:, j, :], wf32[:, j, :], AF.Copy)
                TQ = T // 4
                for tb in range(4):
                    agg = eagg.tile([P, NE, TQ], dt.bfloat16, tag="aggs",
                                    name="aggs", bufs=3)
                    for k in range(K):
                        for dc in range(CC):
                            j = k * CC + dc
                            nc.gpsimd.indirect_dma_start(
                                out=agg[:, j, :], out_offset=None,
                                in_=v2[:, :],
                                in_offset=bass.IndirectOffsetOnAxis(
                                    ap=gov[dc][:, k:k + 1], axis=1),
                                element_offset=tb * TQ)
                            nc.vector.tensor_scalar_mul(
                                agg[:, j, :], agg[:, j, :],
                                w3_t[dc][:, k:k + 1])
                    for dco in range(CC):
                        ps = eps.tile([P, TQ], dt.float32, tag="out_ps")
                        for j in range(NE):
                            nc.tensor.matmul(
                                ps[:], wf16[:, j, bass.ts(dco, P)],
                                agg[:, j, :],
                                start=(j == 0), stop=(j == NE - 1))
                        o_sb = ep.tile([P, TQ], dt.float32, tag="o_sb")
                        nc.scalar.activation(o_sb[:], ps[:], AF.Copy)
                        nc.sync.dma_start(
                            out2[b, bass.ts(dco, P), bass.ts(tb, TQ)], o_sb[:])
            es_sel.close()

        es_const.close()

    nc.compile()
    return nc


def _get_nc():
    if "nc" not in _CACHE:
        _CACHE["nc"] = _build()
    return _CACHE["nc"]


def kernel(query, key, value, Wq, bq, Wk, bk, Wv, bv, Wf, bf):
    query = np.ascontiguousarray(np.asarray(query, dtype=np.float32))
    key = np.ascontiguousarray(np.asarray(key, dtype=np.float32))
    value = np.ascontiguousarray(np.asarray(value, dtype=np.float32))
    for bias in (bq, bk, bv, bf):
        assert np.all(np.asarray(bias) == 0.0), "nonzero biases unsupported"

    if "mats" not in _CACHE:
        _CACHE["mats"] = _dft_matrices()
    wree, wreo, wime, wimo, cie, sie = _CACHE["mats"]

    Wqc = np.ascontiguousarray(np.asarray(Wq, np.float32))
    Wkc = np.ascontiguousarray(np.asarray(Wk, np.float32))
    shared = {
        "Wq": Wqc, "Wk": Wkc, "Wqr": Wqc, "Wkr": Wkc,
        "Wvr": np.ascontiguousarray(np.asarray(Wv, np.float32)),
        "Wf": np.ascontiguousarray(np.asarray(Wf, np.float32)),
        "Wree": wree, "Wreo": wreo, "Wime": wime, "Wimo": wimo,
        "Cie": cie, "Sie": sie,
    }
    in_maps = []
    for c in range(NCORES):
        sl = slice(c * NB, (c + 1) * NB)
        in_maps.append({
            "query2": query[sl], "key2": key[sl], "value2": value[sl], **shared})

    from concourse.bass_utils import run_bass_kernel_spmd
    nc = _get_nc()
    res = run_bass_kernel_spmd(nc, in_maps, core_ids=list(range(NCORES)))
    _CACHE["last_results"] = res
    out = np.concatenate([res.results[c]["out2"] for c in range(NCORES)], axis=0)
    return out.astype(np.float32)


# revision 25
# speedup vs baseline: 1.0709x; 1.0262x over previous
"""AutoCorrelation Bass kernel, refinement architecture (stage 2).

Per batch: correlation pipeline (projections, fwd DFT, pointwise, inv DFT)
runs in float32r (tf32-grade, 1 cyc/row on PE) and is used ONLY to select
top-M=5 candidate lags per channel plus the softmax denominator. The top-3
selection and softmax weights then come from EXACT fp32 time-domain dots
a[tau] = (1/T) sum_t q2[t] k2[t-tau], with q2/k2 from fp32 matmuls and the
circular k-shifts gathered from a DRAM table via per-partition indirect DMA.
Value path (v-proj f32r, agg/E in bf16) only affects output values (2e-2 rel
gate; flips cost ~1e-2 each so refined selection must match fp32 reference).
"""
import numpy as np

import concourse.bass as bass
import concourse.tile as tile
from concourse import bacc, mybir

dt = mybir.dt
AF = mybir.ActivationFunctionType
OP = mybir.AluOpType

P = 128
B, C, T, K = 16, 512, 2048, 3
NB = 2
NCORES = 8
F = 1152
TC = T // P
CC = C // P
FC = F // P
NE = K * C // P
TE = 1152
TEC = TE // P
TO = 1024
TOC = TO // P
H = T // 2
M = 5                     # refinement candidates per channel

_CACHE = {}


def _dft_matrices():
    """Radix-split DFT matrices (fp64 -> fp32). Same as baseline."""
    t640 = np.arange(640.0)[:, None]
    t512 = np.arange(512.0)[:, None]
    ge = np.arange(640.0)[None, :]
    go = np.arange(512.0)[None, :]
    wree = np.where((t640 <= 512) & (ge <= 512),
                    np.cos(2 * np.pi * t640 * (2 * ge) / T), 0.0).astype(np.float32)
    wreo = np.cos(2 * np.pi * t512 * (2 * go + 1) / T).astype(np.float32)
    wime = np.where(ge <= 512,
                    -np.sin(2 * np.pi * t512 * (2 * ge) / T), 0.0).astype(np.float32)
    wimo = np.where(t640 <= 512,
                    -np.sin(2 * np.pi * t640 * (2 * go + 1) / T), 0.0).astype(np.float32)

    f64 = np.arange(F, dtype=np.float64)[None, :]
    livef = f64 <= H
    w = np.where((f64 == 0) | (f64 == H), 1.0, 2.0) * livef / (T * T)
    fc_ = f64.T
    tt = np.arange(TE, dtype=np.float64)[None, :]
    cie = np.where((fc_ <= H) & (tt <= H),
                   np.cos(2 * np.pi * fc_ * tt / T) * w.T, 0.0)
    tt2 = np.arange(TO, dtype=np.float64)[None, :]
    sie = np.where(fc_ <= H,
                   -np.sin(2 * np.pi * fc_ * tt2 / T) * w.T, 0.0)

    def permrows(m):
        out = np.zeros_like(m)
        out[0:513] = m[0:1025:2]
        out[640:1152] = m[1:1024:2]
        return out

    return (wree, wreo, wime, wimo,
            permrows(cie).astype(np.float32), permrows(sie).astype(np.float32))


def _build():
    nc = bacc.Bacc("TRN2", target_bir_lowering=False, debug=False,
                   num_devices=NCORES)

    query2 = nc.dram_tensor("query2", [NB, C, T], dt.float32, kind="ExternalInput").ap()
    key2 = nc.dram_tensor("key2", [NB, C, T], dt.float32, kind="ExternalInput").ap()
    value2 = nc.dram_tensor("value2", [NB, C, T], dt.float32r, kind="ExternalInput").ap()
    Wq = nc.dram_tensor("Wq", [C, C], dt.float32, kind="ExternalInput").ap()
    Wk = nc.dram_tensor("Wk", [C, C], dt.float32, kind="ExternalInput").ap()
    Wqr = nc.dram_tensor("Wqr", [C, C], dt.float32r, kind="ExternalInput").ap()
    Wkr = nc.dram_tensor("Wkr", [C, C], dt.float32r, kind="ExternalInput").ap()
    Wvr = nc.dram_tensor("Wvr", [C, C], dt.float32r, kind="ExternalInput").ap()
    Wf = nc.dram_tensor("Wf", [K * C, C], dt.float32, kind="ExternalInput").ap()
    Wree = nc.dram_tensor("Wree", [640, 640], dt.float32r, kind="ExternalInput").ap()
    Wreo = nc.dram_tensor("Wreo", [512, 512], dt.float32r, kind="ExternalInput").ap()
    Wime = nc.dram_tensor("Wime", [512, 640], dt.float32r, kind="ExternalInput").ap()
    Wimo = nc.dram_tensor("Wimo", [640, 512], dt.float32r, kind="ExternalInput").ap()
    Cie = nc.dram_tensor("Cie", [F, TE], dt.float32r, kind="ExternalInput").ap()
    Sie = nc.dram_tensor("Sie", [F, TO], dt.float32r, kind="ExternalInput").ap()
    out2 = nc.dram_tensor("out2", [NB, C, T], dt.float32, kind="ExternalOutput").ap()

    v2 = nc.dram_tensor("v2", [NB * C, 2 * T], dt.bfloat16).ap()     # rolled-v table
    k2d = nc.dram_tensor("k2d", [NB * C, 2 * T], dt.float32).ap()    # k2 gather table

    with tile.TileContext(nc) as tc:
        from contextlib import ExitStack

        # ---- P0: resident constants ----
        es_const = ExitStack()
        cpool = es_const.enter_context(tc.tile_pool(name="consts", bufs=1, side="left"))
        cie_sb = cpool.tile([P, FC, 640], dt.float32r, tag="cie_sb")
        sie_sb = cpool.tile([P, FC, 640], dt.float32r, tag="sie_sb")
        wv_sb = cpool.tile([P, CC, C], dt.float32r, tag="wv_sb")
        wqr_sb = cpool.tile([P, CC, C], dt.float32r, tag="wqr_sb")
        nc.sync.dma_start(wqr_sb[:], Wqr.rearrange("(n p) d -> p n d", p=P))
        wkr_sb = cpool.tile([P, CC, C], dt.float32r, tag="wkr_sb")
        nc.sync.dma_start(wkr_sb[:], Wkr.rearrange("(n p) d -> p n d", p=P))

        for b in range(NB):
            # ============ P1: radix splits + f32r split-proj + v ============
            es_qk = ExitStack()
            qk_pool = es_qk.enter_context(tc.tile_pool(name=f"qk{b}", bufs=1, side="right"))
            qTee = qk_pool.tile([P, 5, C], dt.float32r, tag="qTee")
            qTeo = qk_pool.tile([P, 4, C], dt.float32r, tag="qTeo")
            qToo = qk_pool.tile([P, 4, C], dt.float32r, tag="qToo")
            qToe = qk_pool.tile([P, 5, C], dt.float32r, tag="qToe")
            kTee = qk_pool.tile([P, 5, C], dt.float32r, tag="kTee")
            kTeo = qk_pool.tile([P, 4, C], dt.float32r, tag="kTeo")
            kToo = qk_pool.tile([P, 4, C], dt.float32r, tag="kToo")
            kToe = qk_pool.tile([P, 5, C], dt.float32r, tag="kToe")

            with tc.tile_pool(name=f"a{b}", bufs=1) as ap_, \
                 tc.tile_pool(name=f"axs{b}", bufs=2) as axs, \
                 tc.tile_pool(name=f"aps{b}", bufs=3, space="PSUM") as aps:
                for name, srcx, w_sb, dsts in (
                        ("q", query2, wqr_sb, (qTee, qTeo, qToo, qToe)),
                        ("k", key2, wkr_sb, (kTee, kTeo, kToo, kToe))):
                    dee, deo, doo, doe = dsts
                    xee = ap_.tile([P, CC, 640], dt.float32r, tag="xee")
                    xeo = ap_.tile([P, CC, 512], dt.float32r, tag="xeo")
                    xoo = ap_.tile([P, CC, 512], dt.float32r, tag="xoo")
                    xoe = ap_.tile([P, CC, 640], dt.float32r, tag="xoe")
                    for cc in range(CC):
                        x_cc = axs.tile([P, T], dt.float32, tag="x_cc")
                        nc.sync.dma_start(
                            x_cc[:],
                            srcx[b].rearrange("(n p) t -> p n t", p=P)[:, cc, :])
                        ab = axs.tile([P, 2, 511], dt.float32, tag="ab")
                        x = x_cc
                        # f32r memset hits a walrus codegen bug; zero via ACT
                        nc.scalar.activation(xee[:, cc, 513:640], x[:, 0:127],
                                             AF.Copy, bias=0.0, scale=0.0)
                        nc.scalar.activation(xoe[:, cc, 513:640], x[:, 0:127],
                                             AF.Copy, bias=0.0, scale=0.0)
                        nc.scalar.activation(xoe[:, cc, 0:1], x[:, 0:1],
                                             AF.Copy, bias=0.0, scale=0.0)
                        nc.scalar.activation(xoo[:, cc, 0:1], x[:, 0:1],
                                             AF.Copy, bias=0.0, scale=0.0)
                        nc.vector.tensor_tensor(out=ab[:, 0, :], in0=x[:, 1:512],
                                                in1=x[:, T - 1:1536:-1], op=OP.add)
                        nc.vector.tensor_tensor(out=ab[:, 1, :], in0=x[:, 1023:512:-1],
                                                in1=x[:, 1025:1536], op=OP.add)
                        nc.vector.tensor_tensor(out=xee[:, cc, 1:512], in0=ab[:, 0, :],
                                                in1=ab[:, 1, :], op=OP.add)
                        nc.vector.tensor_tensor(out=xeo[:, cc, 1:512], in0=ab[:, 0, :],
                                                in1=ab[:, 1, :], op=OP.subtract)
                        nc.vector.tensor_tensor(out=ab[:, 0, :], in0=x[:, 1:512],
                                                in1=x[:, T - 1:1536:-1], op=OP.subtract)
                        nc.vector.tensor_tensor(out=ab[:, 1, :], in0=x[:, 1023:512:-1],
                                                in1=x[:, 1025:1536], op=OP.subtract)
                        nc.vector.tensor_tensor(out=xoo[:, cc, 1:512], in0=ab[:, 0, :],
                                                in1=ab[:, 1, :], op=OP.subtract)
                        nc.vector.tensor_tensor(out=xoe[:, cc, 1:512], in0=ab[:, 0, :],
                                                in1=ab[:, 1, :], op=OP.add)
                        nc.vector.tensor_tensor(out=xee[:, cc, 0:1], in0=x[:, 0:1],
                                                in1=x[:, H:H + 1], op=OP.add)
                        nc.vector.tensor_tensor(out=xeo[:, cc, 0:1], in0=x[:, 0:1],
                                                in1=x[:, H:H + 1], op=OP.subtract)
                        nc.vector.tensor_tensor(out=xee[:, cc, 512:513], in0=x[:, 512:513],
                                                in1=x[:, 1536:1537], op=OP.add)
                        nc.vector.tensor_tensor(out=xoe[:, cc, 512:513], in0=x[:, 512:513],
                                                in1=x[:, 1536:1537], op=OP.subtract)
                    for st_, dst, nch in ((xee, dee, 5), (xeo, deo, 4),
                                          (xoo, doo, 4), (xoe, doe, 5)):
                        for i in range(nch):
                            ps = aps.tile([P, C], dt.float32, tag="proj_ps")
                            for cc in range(CC):
                                nc.tensor.matmul(
                                    ps[:], st_[:, cc, bass.ts(i, P)],
                                    w_sb[:, cc, :],
                                    start=(cc == 0), stop=(cc == CC - 1))
                            nc.scalar.activation(dst[:, i, :], ps[:], AF.Copy)

            if b == 0:
                # deferred const loads: issued after P1's input DMAs so the
                # head of the in-order DMA queue feeds the splits first
                nc.sync.dma_start(
                    cie_sb[:], Cie.rearrange("(n p) t -> p n t", p=P)[:, :, 0:640])
                nc.sync.dma_start(
                    sie_sb[:], Sie.rearrange("(n p) t -> p n t", p=P)[:, :, 0:640])
                nc.sync.dma_start(wv_sb[:], Wvr.rearrange("(n p) d -> p n d", p=P))
            # ============ P2: forward DFT (f32r) + pointwise ============
            es_p = ExitStack()
            p_pool = es_p.enter_context(tc.tile_pool(name=f"p{b}", bufs=1, side="left"))
            pre = p_pool.tile([P, FC, C], dt.float32r, tag="pre")
            pim = p_pool.tile([P, FC, C], dt.float32r, tag="pim")
            with tc.tile_pool(name=f"bmat{b}", bufs=2) as bmat, \
                 tc.tile_pool(name=f"bps{b}", bufs=2, space="PSUM") as bps, \
                 tc.tile_pool(name=f"btmp{b}", bufs=2) as btmp:
                wree_r = Wree.rearrange("(n p) f -> p n f", p=P)
                wreo_r = Wreo.rearrange("(n p) f -> p n f", p=P)
                wime_r = Wime.rearrange("(n p) f -> p n f", p=P)
                wimo_r = Wimo.rearrange("(n p) f -> p n f", p=P)
                for fc in range(FC):
                    even = fc < 5
                    fl = fc if even else fc - 5
                    ncos, nsin = (5, 4) if even else (4, 5)
                    cm = bmat.tile([P, 5, P], dt.float32r, tag="cm")
                    nc.sync.dma_start(
                        cm[:, 0:ncos, :],
                        (wree_r if even else wreo_r)[:, :, bass.ts(fl, P)])
                    sm = bmat.tile([P, 5, P], dt.float32r, tag="sm")
                    nc.sync.dma_start(
                        sm[:, 0:nsin, :],
                        (wime_r if even else wimo_r)[:, :, bass.ts(fl, P)])
                    qcos = qTee if even else qTeo
                    qsin = qToo if even else qToe
                    kcos = kTee if even else kTeo
                    ksin = kToo if even else kToe
                    acc = {}
                    for nm, mat, sig, nchunk in (
                            ("aq", cm, qcos, ncos), ("bq", sm, qsin, nsin),
                            ("ak", cm, kcos, ncos), ("bk", sm, ksin, nsin)):
                        ps = bps.tile([P, C], dt.float32, tag=nm, name=f"ps_{nm}",
                                      bufs=(1 if nm in ("ak", "bk") else 2))
                        for i in range(nchunk):
                            nc.tensor.matmul(
                                ps[:], mat[:, i, :], sig[:, i, :],
                                start=(i == 0), stop=(i == nchunk - 1))
                        acc[nm] = ps
                    aqs = btmp.tile([P, C], dt.float32, tag="aqs")
                    nc.scalar.activation(aqs[:], acc["aq"][:], AF.Copy)
                    bqs = btmp.tile([P, C], dt.float32, tag="bqs")
                    nc.scalar.activation(bqs[:], acc["bq"][:], AF.Copy)
                    tmp = btmp.tile([P, C], dt.float32, tag="tmp")
                    nc.vector.tensor_tensor(
                        out=pre[:, fc, :], in0=aqs[:], in1=acc["ak"][:], op=OP.mult)
                    nc.vector.tensor_tensor(
                        out=tmp[:], in0=bqs[:], in1=acc["bk"][:], op=OP.mult)
                    nc.vector.tensor_tensor(
                        out=pre[:, fc, :], in0=pre[:, fc, :], in1=tmp[:], op=OP.add)
                    nc.vector.tensor_tensor(
                        out=pim[:, fc, :], in0=bqs[:], in1=acc["ak"][:], op=OP.mult)
                    tmp2 = btmp.tile([P, C], dt.float32, tag="tmp2")
                    nc.vector.tensor_tensor(
                        out=tmp2[:], in0=aqs[:], in1=acc["bk"][:], op=OP.mult)
                    nc.vector.tensor_tensor(
                        out=pim[:, fc, :], in0=pim[:, fc, :], in1=tmp2[:], op=OP.subtract)
            es_qk.close()

            # ============ P3: inverse DFT (f32r) + top-8 + denominator ======
            es_sel = ExitStack()
            sel_pool = es_sel.enter_context(
                tc.tile_pool(name=f"sel{b}", bufs=1, side="right"))
            idx8 = [sel_pool.tile([P, 8], dt.uint32, tag=f"idx8_{dc}",
                                  name=f"idx8_{b}_{dc}") for dc in range(CC)]
            negm = [sel_pool.tile([P, 1], dt.float32, tag=f"negm_{dc}",
                                  name=f"negm_{b}_{dc}") for dc in range(CC)]
            rs = [sel_pool.tile([P, 1], dt.float32, tag=f"rs_{dc}",
                                name=f"rs_{b}_{dc}") for dc in range(CC)]
            with tc.tile_pool(name=f"cr{b}", bufs=2) as crp, \
                 tc.tile_pool(name=f"ctmp{b}", bufs=2) as ctmp, \
                 tc.tile_pool(name=f"cps{b}", bufs=1, space="PSUM") as cps:
                HB = H // 2
                for cc in range(CC):
                    rcE = cps.tile([P, HB], dt.float32, tag="rcE", name="ps_rcE", bufs=2)
                    rcE2 = cps.tile([P, P], dt.float32, tag="rcE2", name="ps_rcE2")
                    rcO = cps.tile([P, HB], dt.float32, tag="rcO", name="ps_rcO")
                    rsE = cps.tile([P, HB], dt.float32, tag="rsE", name="ps_rsE")
                    rsO = cps.tile([P, HB], dt.float32, tag="rsO", name="ps_rsO", bufs=2)
                    rsO2 = cps.tile([P, P], dt.float32, tag="rsO2", name="ps_rsO2")
                    for fc in range(5):
                        st, sp = (fc == 0), (fc == 4)
                        pre_l = pre[:, fc, bass.ts(cc, P)]
                        pim_l = pim[:, fc, bass.ts(cc, P)]
                        nc.tensor.matmul(rcE[:], pre_l, cie_sb[:, fc, 0:HB],
                                         start=st, stop=sp)
                        nc.tensor.matmul(rcE2[:], pre_l, cie_sb[:, fc, HB:HB + P],
                                         start=st, stop=sp)
                        nc.tensor.matmul(rsE[:], pim_l, sie_sb[:, fc, 0:HB],
                                         start=st, stop=sp)
                    for fc in range(5, FC):
                        st, sp = (fc == 5), (fc == FC - 1)
                        pre_l = pre[:, fc, bass.ts(cc, P)]
                        pim_l = pim[:, fc, bass.ts(cc, P)]
                        nc.tensor.matmul(rcO[:], pre_l, cie_sb[:, fc, 0:HB],
                                         start=st, stop=sp)
                        nc.tensor.matmul(rsO[:], pim_l, sie_sb[:, fc, 0:HB],
                                         start=st, stop=sp)
                        nc.tensor.matmul(rsO2[:], pim_l, sie_sb[:, fc, HB:HB + P],
                                         start=st, stop=sp)
                    rcO_sb = ctmp.tile([P, HB], dt.float32, tag="rcO_sb")
                    nc.scalar.activation(rcO_sb[:], rcO[:], AF.Copy)
                    rsE_sb = ctmp.tile([P, HB], dt.float32, tag="rsE_sb")
                    nc.scalar.activation(rsE_sb[:], rsE[:], AF.Copy)
                    rsO_sb = ctmp.tile([P, HB + 1], dt.float32, tag="rsO_sb")
                    nc.scalar.activation(rsO_sb[:, 0:HB], rsO[:], AF.Copy)
                    nc.scalar.activation(rsO_sb[:, HB:HB + 1], rsO2[:, 0:1], AF.Copy)
                    s1 = ctmp.tile([P, HB], dt.float32, tag="s1")
                    nc.vector.tensor_tensor(out=s1[:], in0=rcE[:], in1=rcO_sb[:],
                                            op=OP.add)
                    s2 = ctmp.tile([P, HB], dt.float32, tag="s2")
                    nc.vector.tensor_tensor(out=s2[:], in0=rcE[:], in1=rcO_sb[:],
                                            op=OP.subtract)
                    w1 = ctmp.tile([P, HB], dt.float32, tag="w1")
                    nc.vector.tensor_tensor(out=w1[:], in0=rsE_sb[:],
                                            in1=rsO_sb[:, 0:HB], op=OP.add)
                    w2 = ctmp.tile([P, HB], dt.float32, tag="w2")
                    nc.vector.tensor_tensor(out=w2[:], in0=rsO_sb[:, 0:HB],
                                            in1=rsE_sb[:], op=OP.subtract)
                    rt = crp.tile([P, T], dt.float32, tag="rt")
                    nc.vector.tensor_tensor(out=rt[:, 0:HB], in0=s1[:], in1=w1[:],
                                            op=OP.add)
                    nc.vector.tensor_tensor(out=rt[:, 1023:HB:-1], in0=s2[:, 1:HB],
                                            in1=w2[:, 1:HB], op=OP.add)
                    nc.vector.tensor_tensor(out=rt[:, 1025:1536], in0=s2[:, 1:HB],
                                            in1=w2[:, 1:HB], op=OP.subtract)
                    nc.vector.tensor_tensor(out=rt[:, T - 1:1536:-1], in0=s1[:, 1:HB],
                                            in1=w1[:, 1:HB], op=OP.subtract)
                    nc.vector.tensor_tensor(out=rt[:, HB:HB + 1], in0=rcE2[:, 0:1],
                                            in1=rsO_sb[:, HB:HB + 1], op=OP.add)
                    nc.vector.tensor_tensor(out=rt[:, H:H + 1], in0=rcE[:, 0:1],
                                            in1=rcO_sb[:, 0:1], op=OP.subtract)
                    nc.vector.tensor_tensor(out=rt[:, 1536:1537], in0=rcE2[:, 0:1],
                                            in1=rsO_sb[:, HB:HB + 1], op=OP.subtract)
                    # top-8 + softmax denominator on approx r
                    vals = ctmp.tile([P, 8], dt.float32, tag="vals")
                    nc.vector.max(vals[:], rt[:])
                    nc.vector.max_index(idx8[cc][:], vals[:], rt[:])
                    nc.scalar.activation(negm[cc][:], vals[:, 0:1],
                                         AF.Copy, bias=0.0, scale=-1.0)
                    esc = crp.tile([P, T], dt.float32, tag="esc")
                    s_col = ctmp.tile([P, 1], dt.float32, tag="s_col")
                    nc.scalar.activation(
                        esc[:], rt[:], AF.Exp,
                        bias=negm[cc][:, 0:1], scale=1.0,
                        accum_out=s_col[:, 0:1])
                    nc.vector.reciprocal(rs[cc][:], s_col[:])
            es_p.close()

            # ============ P4: fp32 q2/k2 + exact refinement ============
            w3_t = [sel_pool.tile([P, K], dt.float32, tag=f"w3_{dc}",
                                  name=f"w3_{b}_{dc}") for dc in range(CC)]
            gov = [sel_pool.tile([P, K], dt.uint32, tag=f"gov_{dc}",
                                 name=f"gov_{b}_{dc}") for dc in range(CC)]
            with tc.tile_pool(name=f"d{b}", bufs=1) as dp, \
                 tc.tile_pool(name=f"dsc{b}", bufs=2) as dsc, \
                 tc.tile_pool(name=f"dks{b}", bufs=3) as dks, \
                 tc.tile_pool(name=f"dk2{b}", bufs=1) as dk2, \
                 tc.tile_pool(name=f"dps{b}", bufs=2, space="PSUM") as dps:
                k2r = k2d.rearrange("(n p) w -> n p w", p=P)
                # P4a: k2 (fp32) -> DRAM doubled table
                xk_sb = dp.tile([P, CC, T], dt.float32, tag="x_p4s")
                nc.sync.dma_start(
                    xk_sb[:], key2[b].rearrange("(n p) t -> p n t", p=P))
                wk_sb = dp.tile([P, CC, C], dt.float32, tag="w_p4")
                nc.sync.dma_start(wk_sb[:], Wk.rearrange("(n p) d -> p n d", p=P))
                for dc in range(CC):
                    k2sb = dsc.tile([P, T], dt.float32, tag="k2sb")
                    for tb in range(4):
                        ps = dps.tile([P, T // 4], dt.float32, tag="p4ps")
                        for cc in range(CC):
                            nc.tensor.matmul(
                                ps[:], wk_sb[:, cc, bass.ts(dc, P)],
                                xk_sb[:, cc, bass.ts(tb, T // 4)],
                                start=(cc == 0), stop=(cc == CC - 1))
                        nc.scalar.activation(
                            k2sb[:, bass.ts(tb, T // 4)], ps[:], AF.Copy)
                    nc.sync.dma_start(k2r[b * CC + dc, :, 0:T], k2sb[:])
                    nc.sync.dma_start(k2r[b * CC + dc, :, T:2 * T], k2sb[:])
                # P4v: v projection (f32r) -> v2 DRAM table (bf16, doubled)
                xv_sb = dp.tile([P, CC, T], dt.float32r, tag="x_p4s")
                nc.sync.dma_start(
                    xv_sb[:], value2[b].rearrange("(n p) t -> p n t", p=P))
                v2r = v2.rearrange("(n p) w -> n p w", p=P)
                for dc in range(CC):
                    v_sb = dsc.tile([P, T], dt.bfloat16, tag="v_sb")
                    for tb in range(4):
                        ps = dps.tile([P, T // 4], dt.float32, tag="p4ps")
                        for cc in range(CC):
                            nc.tensor.matmul(
                                ps[:], wv_sb[:, cc, bass.ts(dc, P)],
                                xv_sb[:, cc, bass.ts(tb, T // 4)],
                                start=(cc == 0), stop=(cc == CC - 1))
                        nc.scalar.activation(
                            v_sb[:, bass.ts(tb, T // 4)], ps[:], AF.Copy)
                    nc.sync.dma_start(v2r[b * CC + dc, :, 0:T], v_sb[:])
                    nc.sync.dma_start(v2r[b * CC + dc, :, T:2 * T], v_sb[:])
                # P4b: q2 per dc + gathers + dots + selection
                xq_sb = dp.tile([P, CC, T], dt.float32, tag="x_p4s")
                nc.sync.dma_start(
                    xq_sb[:], query2[b].rearrange("(n p) t -> p n t", p=P))
                wq_sb = dp.tile([P, CC, C], dt.float32, tag="w_p4")
                nc.sync.dma_start(wq_sb[:], Wq.rearrange("(n p) d -> p n d", p=P))
                for dc in range(CC):
                    q2sb = dsc.tile([P, T], dt.float32, tag="q2sb")
                    for tb in range(4):
                        ps = dps.tile([P, T // 4], dt.float32, tag="p4ps")
                        for cc in range(CC):
                            nc.tensor.matmul(
                                ps[:], wq_sb[:, cc, bass.ts(dc, P)],
                                xq_sb[:, cc, bass.ts(tb, T // 4)],
                                start=(cc == 0), stop=(cc == CC - 1))
                        nc.scalar.activation(
                            q2sb[:, bass.ts(tb, T // 4)], ps[:], AF.Copy)

                    # gather offsets for M candidates:
                    #   (b*C+dc*128+p)*2T + T - lag_m
                    iot = dsc.tile([P, 1], dt.int32, tag="iot")
                    nc.gpsimd.iota(
                        iot[:], pattern=[[0, 1]],
                        base=(b * C + dc * P) * (2 * T) + T,
                        channel_multiplier=2 * T)
                    iot_f = dsc.tile([P, 1], dt.float32, tag="iot_f")
                    nc.vector.tensor_copy(iot_f[:], iot[:])
                    idxm_f = dsc.tile([P, M], dt.float32, tag="idxm_f")
                    nc.vector.tensor_copy(idxm_f[:], idx8[dc][:, 0:M])
                    gom = dsc.tile([P, M], dt.float32, tag="gom")
                    nc.scalar.activation(gom[:], idxm_f[:],
                                         AF.Copy, bias=0.0, scale=-1.0)
                    nc.vector.tensor_scalar_add(gom[:], gom[:], iot_f[:, 0:1])
                    gou = dsc.tile([P, M], dt.uint32, tag="gou")
                    nc.vector.tensor_copy(gou[:], gom[:])

                    refined = dsc.tile([P, 8], dt.float32, tag="refined")
                    nc.gpsimd.memset(refined[:, M:8], -3.0e38)
                    for m in range(M):
                        ksh = dks.tile([P, T], dt.float32, tag="ksh")
                        nc.gpsimd.indirect_dma_start(
                            out=ksh[:], out_offset=None,
                            in_=k2d[:, :],
                            in_offset=bass.IndirectOffsetOnAxis(
                                ap=gou[:, m:m + 1], axis=1),
                            element_offset=0)
                        scr = dks.tile([P, T], dt.float32, tag="scr")
                        nc.vector.tensor_tensor(
                            out=scr[:], in0=q2sb[:], in1=ksh[:], op=OP.mult)
                        scr2 = dk2.tile([P, T], dt.float32, tag="scr2")
                        nc.scalar.activation(
                            scr2[:], scr[:], AF.Copy, scale=1.0 / T,
                            accum_out=refined[:, m:m + 1])

                    valr = dsc.tile([P, 8], dt.float32, tag="valr")
                    nc.vector.max(valr[:], refined[:])
                    pos8 = dsc.tile([P, 8], dt.uint32, tag="pos8")
                    nc.vector.max_index(pos8[:], valr[:], refined[:])
                    posf = dsc.tile([P, K], dt.float32, tag="posf")
                    nc.vector.tensor_copy(posf[:], pos8[:, 0:K])
                    # one-hot map: lag_sel[:, k] = sum_m idxm_f[:, m]*(posf==m)
                    lagf = dsc.tile([P, K], dt.float32, tag="lagf")
                    eqm = dsc.tile([P, K], dt.float32, tag="eqm")
                    contrib = dsc.tile([P, K], dt.float32, tag="contrib")
                    for m in range(M):
                        nc.vector.tensor_scalar(
                            out=eqm[:], in0=posf[:], scalar1=float(m),
                            scalar2=None, op0=OP.is_equal)
                        nc.vector.tensor_scalar_mul(
                            contrib[:], eqm[:], idxm_f[:, m:m + 1])
                        if m == 0:
                            nc.vector.tensor_copy(lagf[:], contrib[:])
                        else:
                            nc.vector.tensor_tensor(
                                out=lagf[:], in0=lagf[:], in1=contrib[:],
                                op=OP.add)
                    # weights: w3 = exp(valr[0:K] - m~) * rs
                    ew = dsc.tile([P, K], dt.float32, tag="ew")
                    nc.scalar.activation(ew[:], valr[:, 0:K],
                                         AF.Exp, bias=negm[dc][:, 0:1], scale=1.0)
                    nc.vector.tensor_scalar_mul(w3_t[dc][:], ew[:], rs[dc][:, 0:1])
                    # v2 gather offsets: rowbase + T - lag_sel
                    govf = dsc.tile([P, K], dt.float32, tag="govf")
                    nc.scalar.activation(govf[:], lagf[:],
                                         AF.Copy, bias=0.0, scale=-1.0)
                    nc.vector.tensor_scalar_add(govf[:], govf[:], iot_f[:, 0:1])
                    nc.vector.tensor_copy(gov[dc][:], govf[:])
            es_sel.close_later = None  # keep sel_pool until P5 end

            # ============ P5: agg gathers (bf16) + scale + E ============
            with tc.tile_pool(name=f"e{b}", bufs=1) as ep, \
                 tc.tile_pool(name=f"eagg{b}", bufs=3) as eagg, \
                 tc.tile_pool(name=f"eps{b}", bufs=6, space="PSUM") as eps:
                wf16 = ep.tile([P, NE, C], dt.bfloat16, tag="wf16")
                with tc.tile_pool(name=f"wfload{b}", bufs=1) as wfl:
                    wf32 = wfl.tile([P, NE, C], dt.float32, tag="wf32")
                    nc.sync.dma_start(wf32[:], Wf.rearrange("(n p) d -> p n d", p=P))
                    for j in range(NE):
                        nc.scalar.activation(wf16[:, j, :], wf32[:, j, :], AF.Copy)
                TQ = T // 4
                for tb in range(4):
                    agg = eagg.tile([P, NE, TQ], dt.bfloat16, tag="aggs",
                                    name="aggs", bufs=3)
                    for k in range(K):
                        for dc in range(CC):
                            j = k * CC + dc
                            nc.gpsimd.indirect_dma_start(
                                out=agg[:, j, :], out_offset=None,
                                in_=v2[:, :],
                                in_offset=bass.IndirectOffsetOnAxis(
                                    ap=gov[dc][:, k:k + 1], axis=1),
                                element_offset=tb * TQ)
                            nc.vector.tensor_scalar_mul(
                                agg[:, j, :], agg[:, j, :],
                                w3_t[dc][:, k:k + 1])
                    for dco in range(CC):
                        ps = eps.tile([P, TQ], dt.float32, tag="out_ps")
                        for j in range(NE):
                            nc.tensor.matmul(
                                ps[:], wf16[:, j, bass.ts(dco, P)],
                                agg[:, j, :],
                                start=(j == 0), stop=(j == NE - 1))
                        o_sb = ep.tile([P, TQ], dt.float32, tag="o_sb")
                        nc.scalar.activation(o_sb[:], ps[:], AF.Copy)
                        nc.sync.dma_start(
                            out2[b, bass.ts(dco, P), bass.ts(tb, TQ)], o_sb[:])
            es_sel.close()

        es_const.close()

    nc.compile()
    return nc


def _get_nc():
    if "nc" not in _CACHE:
        _CACHE["nc"] = _build()
    return _CACHE["nc"]


def kernel(query, key, value, Wq, bq, Wk, bk, Wv, bv, Wf, bf):
    query = np.ascontiguousarray(np.asarray(query, dtype=np.float32))
    key = np.ascontiguousarray(np.asarray(key, dtype=np.float32))
    value = np.ascontiguousarray(np.asarray(value, dtype=np.float32))
    for bias in (bq, bk, bv, bf):
        assert np.all(np.asarray(bias) == 0.0), "nonzero biases unsupported"

    if "mats" not in _CACHE:
        _CACHE["mats"] = _dft_matrices()
    wree, wreo, wime, wimo, cie, sie = _CACHE["mats"]

    Wqc = np.ascontiguousarray(np.asarray(Wq, np.float32))
    Wkc = np.ascontiguousarray(np.asarray(Wk, np.float32))
    shared = {
        "Wq": Wqc, "Wk": Wkc, "Wqr": Wqc, "Wkr": Wkc,
        "Wvr": np.ascontiguousarray(np.asarray(Wv, np.float32)),
        "Wf": np.ascontiguousarray(np.asarray(Wf, np.float32)),
        "Wree": wree, "Wreo": wreo, "Wime": wime, "Wimo": wimo,
        "Cie": cie, "Sie": sie,
    }
    in_maps = []
    for c in range(NCORES):
        sl = slice(c * NB, (c + 1) * NB)
        in_maps.append({
            "query2": query[sl], "key2": key[sl], "value2": value[sl], **shared})

    from concourse.bass_utils import run_bass_kernel_spmd
    nc = _get_nc()
    res = run_bass_kernel_spmd(nc, in_maps, core_ids=list(range(NCORES)))
    _CACHE["last_results"] = res
    out = np.concatenate([res.results[c]["out2"] for c in range(NCORES)], axis=0)
    return out.astype(np.float32)


# revision 26
# speedup vs baseline: 1.0875x; 1.0155x over previous
"""AutoCorrelation Bass kernel, refinement architecture (stage 2).

Per batch: correlation pipeline (projections, fwd DFT, pointwise, inv DFT)
runs in float32r (tf32-grade, 1 cyc/row on PE) and is used ONLY to select
top-M=5 candidate lags per channel plus the softmax denominator. The top-3
selection and softmax weights then come from EXACT fp32 time-domain dots
a[tau] = (1/T) sum_t q2[t] k2[t-tau], with q2/k2 from fp32 matmuls and the
circular k-shifts gathered from a DRAM table via per-partition indirect DMA.
Value path (v-proj f32r, agg/E in bf16) only affects output values (2e-2 rel
gate; flips cost ~1e-2 each so refined selection must match fp32 reference).
"""
import numpy as np

import concourse.bass as bass
import concourse.tile as tile
from concourse import bacc, mybir

dt = mybir.dt
AF = mybir.ActivationFunctionType
OP = mybir.AluOpType

P = 128
B, C, T, K = 16, 512, 2048, 3
NB = 2
NCORES = 8
F = 1152
TC = T // P
CC = C // P
FC = F // P
NE = K * C // P
TE = 1152
TEC = TE // P
TO = 1024
TOC = TO // P
H = T // 2
M = 5                     # refinement candidates per channel

_CACHE = {}


def _dft_matrices():
    """Radix-split DFT matrices (fp64 -> fp32). Same as baseline."""
    t640 = np.arange(640.0)[:, None]
    t512 = np.arange(512.0)[:, None]
    ge = np.arange(640.0)[None, :]
    go = np.arange(512.0)[None, :]
    wree = np.where((t640 <= 512) & (ge <= 512),
                    np.cos(2 * np.pi * t640 * (2 * ge) / T), 0.0).astype(np.float32)
    wreo = np.cos(2 * np.pi * t512 * (2 * go + 1) / T).astype(np.float32)
    wime = np.where(ge <= 512,
                    -np.sin(2 * np.pi * t512 * (2 * ge) / T), 0.0).astype(np.float32)
    wimo = np.where(t640 <= 512,
                    -np.sin(2 * np.pi * t640 * (2 * go + 1) / T), 0.0).astype(np.float32)

    f64 = np.arange(F, dtype=np.float64)[None, :]
    livef = f64 <= H
    w = np.where((f64 == 0) | (f64 == H), 1.0, 2.0) * livef / (T * T)
    fc_ = f64.T
    tt = np.arange(TE, dtype=np.float64)[None, :]
    cie = np.where((fc_ <= H) & (tt <= H),
                   np.cos(2 * np.pi * fc_ * tt / T) * w.T, 0.0)
    tt2 = np.arange(TO, dtype=np.float64)[None, :]
    sie = np.where(fc_ <= H,
                   -np.sin(2 * np.pi * fc_ * tt2 / T) * w.T, 0.0)

    def permrows(m):
        out = np.zeros_like(m)
        out[0:513] = m[0:1025:2]
        out[640:1152] = m[1:1024:2]
        return out

    return (wree, wreo, wime, wimo,
            permrows(cie).astype(np.float32), permrows(sie).astype(np.float32))


def _build():
    nc = bacc.Bacc("TRN2", target_bir_lowering=False, debug=False,
                   num_devices=NCORES)

    query2 = nc.dram_tensor("query2", [NB, C, T], dt.float32, kind="ExternalInput").ap()
    key2 = nc.dram_tensor("key2", [NB, C, T], dt.float32, kind="ExternalInput").ap()
    value2 = nc.dram_tensor("value2", [NB, C, T], dt.float32r, kind="ExternalInput").ap()
    Wq = nc.dram_tensor("Wq", [C, C], dt.float32, kind="ExternalInput").ap()
    Wk = nc.dram_tensor("Wk", [C, C], dt.float32, kind="ExternalInput").ap()
    Wqr = nc.dram_tensor("Wqr", [C, C], dt.float32r, kind="ExternalInput").ap()
    Wkr = nc.dram_tensor("Wkr", [C, C], dt.float32r, kind="ExternalInput").ap()
    Wvr = nc.dram_tensor("Wvr", [C, C], dt.float32r, kind="ExternalInput").ap()
    Wf = nc.dram_tensor("Wf", [K * C, C], dt.float32, kind="ExternalInput").ap()
    Wree = nc.dram_tensor("Wree", [640, 640], dt.float32r, kind="ExternalInput").ap()
    Wreo = nc.dram_tensor("Wreo", [512, 512], dt.float32r, kind="ExternalInput").ap()
    Wime = nc.dram_tensor("Wime", [512, 640], dt.float32r, kind="ExternalInput").ap()
    Wimo = nc.dram_tensor("Wimo", [640, 512], dt.float32r, kind="ExternalInput").ap()
    Cie = nc.dram_tensor("Cie", [F, TE], dt.float32r, kind="ExternalInput").ap()
    Sie = nc.dram_tensor("Sie", [F, TO], dt.float32r, kind="ExternalInput").ap()
    out2 = nc.dram_tensor("out2", [NB, C, T], dt.float32, kind="ExternalOutput").ap()

    v2 = nc.dram_tensor("v2", [NB * C, 2 * T], dt.bfloat16).ap()     # rolled-v table
    k2d = nc.dram_tensor("k2d", [NB * C, 2 * T], dt.float32).ap()    # k2 gather table

    with tile.TileContext(nc) as tc:
        from contextlib import ExitStack

        # ---- P0: resident constants ----
        es_const = ExitStack()
        cpool = es_const.enter_context(tc.tile_pool(name="consts", bufs=1, side="left"))
        cie_sb = cpool.tile([P, FC, 640], dt.float32r, tag="cie_sb")
        sie_sb = cpool.tile([P, FC, 640], dt.float32r, tag="sie_sb")
        wv_sb = cpool.tile([P, CC, C], dt.float32r, tag="wv_sb")
        wqr_sb = cpool.tile([P, CC, C], dt.float32r, tag="wqr_sb")
        nc.sync.dma_start(wqr_sb[:], Wqr.rearrange("(n p) d -> p n d", p=P))
        wkr_sb = cpool.tile([P, CC, C], dt.float32r, tag="wkr_sb")
        nc.sync.dma_start(wkr_sb[:], Wkr.rearrange("(n p) d -> p n d", p=P))

        for b in range(NB):
            # ============ P1: radix splits + f32r split-proj + v ============
            es_qk = ExitStack()
            qk_pool = es_qk.enter_context(tc.tile_pool(name=f"qk{b}", bufs=1, side="right"))
            qTee = qk_pool.tile([P, 5, C], dt.float32r, tag="qTee")
            qTeo = qk_pool.tile([P, 4, C], dt.float32r, tag="qTeo")
            qToo = qk_pool.tile([P, 4, C], dt.float32r, tag="qToo")
            qToe = qk_pool.tile([P, 5, C], dt.float32r, tag="qToe")
            kTee = qk_pool.tile([P, 5, C], dt.float32r, tag="kTee")
            kTeo = qk_pool.tile([P, 4, C], dt.float32r, tag="kTeo")
            kToo = qk_pool.tile([P, 4, C], dt.float32r, tag="kToo")
            kToe = qk_pool.tile([P, 5, C], dt.float32r, tag="kToe")

            with tc.tile_pool(name=f"a{b}", bufs=1) as ap_, \
                 tc.tile_pool(name=f"axs{b}", bufs=2) as axs, \
                 tc.tile_pool(name=f"aps{b}", bufs=3, space="PSUM") as aps:
                for name, srcx, w_sb, dsts in (
                        ("q", query2, wqr_sb, (qTee, qTeo, qToo, qToe)),
                        ("k", key2, wkr_sb, (kTee, kTeo, kToo, kToe))):
                    dee, deo, doo, doe = dsts
                    xee = ap_.tile([P, CC, 640], dt.float32r, tag="xee")
                    xeo = ap_.tile([P, CC, 512], dt.float32r, tag="xeo")
                    xoo = ap_.tile([P, CC, 512], dt.float32r, tag="xoo")
                    xoe = ap_.tile([P, CC, 640], dt.float32r, tag="xoe")
                    for cc in range(CC):
                        x_cc = axs.tile([P, T], dt.float32, tag="x_cc")
                        nc.sync.dma_start(
                            x_cc[:],
                            srcx[b].rearrange("(n p) t -> p n t", p=P)[:, cc, :])
                        ab = axs.tile([P, 2, 511], dt.float32, tag="ab")
                        x = x_cc
                        # f32r memset hits a walrus codegen bug; zero via ACT
                        nc.scalar.activation(xee[:, cc, 513:640], x[:, 0:127],
                                             AF.Copy, bias=0.0, scale=0.0)
                        nc.scalar.activation(xoe[:, cc, 513:640], x[:, 0:127],
                                             AF.Copy, bias=0.0, scale=0.0)
                        nc.scalar.activation(xoe[:, cc, 0:1], x[:, 0:1],
                                             AF.Copy, bias=0.0, scale=0.0)
                        nc.scalar.activation(xoo[:, cc, 0:1], x[:, 0:1],
                                             AF.Copy, bias=0.0, scale=0.0)
                        nc.vector.tensor_tensor(out=ab[:, 0, :], in0=x[:, 1:512],
                                                in1=x[:, T - 1:1536:-1], op=OP.add)
                        nc.vector.tensor_tensor(out=ab[:, 1, :], in0=x[:, 1023:512:-1],
                                                in1=x[:, 1025:1536], op=OP.add)
                        nc.vector.tensor_tensor(out=xee[:, cc, 1:512], in0=ab[:, 0, :],
                                                in1=ab[:, 1, :], op=OP.add)
                        nc.vector.tensor_tensor(out=xeo[:, cc, 1:512], in0=ab[:, 0, :],
                                                in1=ab[:, 1, :], op=OP.subtract)
                        nc.vector.tensor_tensor(out=ab[:, 0, :], in0=x[:, 1:512],
                                                in1=x[:, T - 1:1536:-1], op=OP.subtract)
                        nc.vector.tensor_tensor(out=ab[:, 1, :], in0=x[:, 1023:512:-1],
                                                in1=x[:, 1025:1536], op=OP.subtract)
                        nc.vector.tensor_tensor(out=xoo[:, cc, 1:512], in0=ab[:, 0, :],
                                                in1=ab[:, 1, :], op=OP.subtract)
                        nc.vector.tensor_tensor(out=xoe[:, cc, 1:512], in0=ab[:, 0, :],
                                                in1=ab[:, 1, :], op=OP.add)
                        nc.vector.tensor_tensor(out=xee[:, cc, 0:1], in0=x[:, 0:1],
                                                in1=x[:, H:H + 1], op=OP.add)
                        nc.vector.tensor_tensor(out=xeo[:, cc, 0:1], in0=x[:, 0:1],
                                                in1=x[:, H:H + 1], op=OP.subtract)
                        nc.vector.tensor_tensor(out=xee[:, cc, 512:513], in0=x[:, 512:513],
                                                in1=x[:, 1536:1537], op=OP.add)
                        nc.vector.tensor_tensor(out=xoe[:, cc, 512:513], in0=x[:, 512:513],
                                                in1=x[:, 1536:1537], op=OP.subtract)
                    for st_, dst, nch in ((xee, dee, 5), (xeo, deo, 4),
                                          (xoo, doo, 4), (xoe, doe, 5)):
                        for i in range(nch):
                            ps = aps.tile([P, C], dt.float32, tag="proj_ps")
                            for cc in range(CC):
                                nc.tensor.matmul(
                                    ps[:], st_[:, cc, bass.ts(i, P)],
                                    w_sb[:, cc, :],
                                    start=(cc == 0), stop=(cc == CC - 1))
                            nc.scalar.activation(dst[:, i, :], ps[:], AF.Copy)

            if b == 0:
                # deferred const loads: issued after P1's input DMAs so the
                # head of the in-order DMA queue feeds the splits first
                nc.sync.dma_start(
                    cie_sb[:], Cie.rearrange("(n p) t -> p n t", p=P)[:, :, 0:640])
                nc.sync.dma_start(
                    sie_sb[:], Sie.rearrange("(n p) t -> p n t", p=P)[:, :, 0:640])
                nc.sync.dma_start(wv_sb[:], Wvr.rearrange("(n p) d -> p n d", p=P))
            # ============ P2: forward DFT (f32r) + pointwise ============
            es_p = ExitStack()
            p_pool = es_p.enter_context(tc.tile_pool(name=f"p{b}", bufs=1, side="left"))
            pre = p_pool.tile([P, FC, C], dt.float32r, tag="pre")
            pim = p_pool.tile([P, FC, C], dt.float32r, tag="pim")
            with tc.tile_pool(name=f"bmat{b}", bufs=2) as bmat, \
                 tc.tile_pool(name=f"bps{b}", bufs=2, space="PSUM") as bps, \
                 tc.tile_pool(name=f"btmp{b}", bufs=2) as btmp:
                wree_r = Wree.rearrange("(n p) f -> p n f", p=P)
                wreo_r = Wreo.rearrange("(n p) f -> p n f", p=P)
                wime_r = Wime.rearrange("(n p) f -> p n f", p=P)
                wimo_r = Wimo.rearrange("(n p) f -> p n f", p=P)
                for fc in range(FC):
                    even = fc < 5
                    fl = fc if even else fc - 5
                    ncos, nsin = (5, 4) if even else (4, 5)
                    cm = bmat.tile([P, 5, P], dt.float32r, tag="cm")
                    nc.sync.dma_start(
                        cm[:, 0:ncos, :],
                        (wree_r if even else wreo_r)[:, :, bass.ts(fl, P)])
                    sm = bmat.tile([P, 5, P], dt.float32r, tag="sm")
                    nc.sync.dma_start(
                        sm[:, 0:nsin, :],
                        (wime_r if even else wimo_r)[:, :, bass.ts(fl, P)])
                    qcos = qTee if even else qTeo
                    qsin = qToo if even else qToe
                    kcos = kTee if even else kTeo
                    ksin = kToo if even else kToe
                    acc = {}
                    for nm, mat, sig, nchunk in (
                            ("aq", cm, qcos, ncos), ("bq", sm, qsin, nsin),
                            ("ak", cm, kcos, ncos), ("bk", sm, ksin, nsin)):
                        ps = bps.tile([P, C], dt.float32, tag=nm, name=f"ps_{nm}",
                                      bufs=(1 if nm in ("ak", "bk") else 2))
                        for i in range(nchunk):
                            nc.tensor.matmul(
                                ps[:], mat[:, i, :], sig[:, i, :],
                                start=(i == 0), stop=(i == nchunk - 1))
                        acc[nm] = ps
                    aqs = btmp.tile([P, C], dt.float32, tag="aqs")
                    nc.scalar.activation(aqs[:], acc["aq"][:], AF.Copy)
                    bqs = btmp.tile([P, C], dt.float32, tag="bqs")
                    nc.scalar.activation(bqs[:], acc["bq"][:], AF.Copy)
                    tmp = btmp.tile([P, C], dt.float32, tag="tmp")
                    nc.vector.tensor_tensor(
                        out=pre[:, fc, :], in0=aqs[:], in1=acc["ak"][:], op=OP.mult)
                    nc.vector.tensor_tensor(
                        out=tmp[:], in0=bqs[:], in1=acc["bk"][:], op=OP.mult)
                    nc.vector.tensor_tensor(
                        out=pre[:, fc, :], in0=pre[:, fc, :], in1=tmp[:], op=OP.add)
                    nc.vector.tensor_tensor(
                        out=pim[:, fc, :], in0=bqs[:], in1=acc["ak"][:], op=OP.mult)
                    tmp2 = btmp.tile([P, C], dt.float32, tag="tmp2")
                    nc.vector.tensor_tensor(
                        out=tmp2[:], in0=aqs[:], in1=acc["bk"][:], op=OP.mult)
                    nc.vector.tensor_tensor(
                        out=pim[:, fc, :], in0=pim[:, fc, :], in1=tmp2[:], op=OP.subtract)
            es_qk.close()

            # ============ P3: inverse DFT (f32r) + top-8 + denominator ======
            es_sel = ExitStack()
            sel_pool = es_sel.enter_context(
                tc.tile_pool(name=f"sel{b}", bufs=1, side="right"))
            idx8 = [sel_pool.tile([P, 8], dt.uint32, tag=f"idx8_{dc}",
                                  name=f"idx8_{b}_{dc}") for dc in range(CC)]
            negm = [sel_pool.tile([P, 1], dt.float32, tag=f"negm_{dc}",
                                  name=f"negm_{b}_{dc}") for dc in range(CC)]
            rs = [sel_pool.tile([P, 1], dt.float32, tag=f"rs_{dc}",
                                name=f"rs_{b}_{dc}") for dc in range(CC)]
            with tc.tile_pool(name=f"cr{b}", bufs=2) as crp, \
                 tc.tile_pool(name=f"ctmp{b}", bufs=2) as ctmp, \
                 tc.tile_pool(name=f"cps{b}", bufs=1, space="PSUM") as cps:
                HB = H // 2
                for cc in range(CC):
                    rcE = cps.tile([P, HB], dt.float32, tag="rcE", name="ps_rcE", bufs=2)
                    rcE2 = cps.tile([P, P], dt.float32, tag="rcE2", name="ps_rcE2")
                    rcO = cps.tile([P, HB], dt.float32, tag="rcO", name="ps_rcO")
                    rsE = cps.tile([P, HB], dt.float32, tag="rsE", name="ps_rsE")
                    rsO = cps.tile([P, HB], dt.float32, tag="rsO", name="ps_rsO", bufs=2)
                    rsO2 = cps.tile([P, P], dt.float32, tag="rsO2", name="ps_rsO2")
                    for fc in range(5):
                        st, sp = (fc == 0), (fc == 4)
                        pre_l = pre[:, fc, bass.ts(cc, P)]
                        pim_l = pim[:, fc, bass.ts(cc, P)]
                        nc.tensor.matmul(rcE[:], pre_l, cie_sb[:, fc, 0:HB],
                                         start=st, stop=sp)
                        nc.tensor.matmul(rcE2[:], pre_l, cie_sb[:, fc, HB:HB + P],
                                         start=st, stop=sp)
                        nc.tensor.matmul(rsE[:], pim_l, sie_sb[:, fc, 0:HB],
                                         start=st, stop=sp)
                    for fc in range(5, FC):
                        st, sp = (fc == 5), (fc == FC - 1)
                        pre_l = pre[:, fc, bass.ts(cc, P)]
                        pim_l = pim[:, fc, bass.ts(cc, P)]
                        nc.tensor.matmul(rcO[:], pre_l, cie_sb[:, fc, 0:HB],
                                         start=st, stop=sp)
                        nc.tensor.matmul(rsO[:], pim_l, sie_sb[:, fc, 0:HB],
                                         start=st, stop=sp)
                        nc.tensor.matmul(rsO2[:], pim_l, sie_sb[:, fc, HB:HB + P],
                                         start=st, stop=sp)
                    rcO_sb = ctmp.tile([P, HB], dt.float32, tag="rcO_sb")
                    nc.scalar.activation(rcO_sb[:], rcO[:], AF.Copy)
                    rsE_sb = ctmp.tile([P, HB], dt.float32, tag="rsE_sb")
                    nc.scalar.activation(rsE_sb[:], rsE[:], AF.Copy)
                    rsO_sb = ctmp.tile([P, HB + 1], dt.float32, tag="rsO_sb")
                    nc.scalar.activation(rsO_sb[:, 0:HB], rsO[:], AF.Copy)
                    nc.scalar.activation(rsO_sb[:, HB:HB + 1], rsO2[:, 0:1], AF.Copy)
                    s1 = ctmp.tile([P, HB], dt.float32, tag="s1")
                    nc.vector.tensor_tensor(out=s1[:], in0=rcE[:], in1=rcO_sb[:],
                                            op=OP.add)
                    s2 = ctmp.tile([P, HB], dt.float32, tag="s2")
                    nc.vector.tensor_tensor(out=s2[:], in0=rcE[:], in1=rcO_sb[:],
                                            op=OP.subtract)
                    w1 = ctmp.tile([P, HB], dt.float32, tag="w1")
                    nc.vector.tensor_tensor(out=w1[:], in0=rsE_sb[:],
                                            in1=rsO_sb[:, 0:HB], op=OP.add)
                    w2 = ctmp.tile([P, HB], dt.float32, tag="w2")
                    nc.vector.tensor_tensor(out=w2[:], in0=rsO_sb[:, 0:HB],
                                            in1=rsE_sb[:], op=OP.subtract)
                    rt = crp.tile([P, T], dt.float32, tag="rt")
                    nc.vector.tensor_tensor(out=rt[:, 0:HB], in0=s1[:], in1=w1[:],
                                            op=OP.add)
                    nc.vector.tensor_tensor(out=rt[:, 1023:HB:-1], in0=s2[:, 1:HB],
                                            in1=w2[:, 1:HB], op=OP.add)
                    nc.vector.tensor_tensor(out=rt[:, 1025:1536], in0=s2[:, 1:HB],
                                            in1=w2[:, 1:HB], op=OP.subtract)
                    nc.vector.tensor_tensor(out=rt[:, T - 1:1536:-1], in0=s1[:, 1:HB],
                                            in1=w1[:, 1:HB], op=OP.subtract)
                    nc.vector.tensor_tensor(out=rt[:, HB:HB + 1], in0=rcE2[:, 0:1],
                                            in1=rsO_sb[:, HB:HB + 1], op=OP.add)
                    nc.vector.tensor_tensor(out=rt[:, H:H + 1], in0=rcE[:, 0:1],
                                            in1=rcO_sb[:, 0:1], op=OP.subtract)
                    nc.vector.tensor_tensor(out=rt[:, 1536:1537], in0=rcE2[:, 0:1],
                                            in1=rsO_sb[:, HB:HB + 1], op=OP.subtract)
                    # top-8 + softmax denominator on approx r
                    vals = ctmp.tile([P, 8], dt.float32, tag="vals")
                    nc.vector.max(vals[:], rt[:])
                    nc.vector.max_index(idx8[cc][:], vals[:], rt[:])
                    nc.scalar.activation(negm[cc][:], vals[:, 0:1],
                                         AF.Copy, bias=0.0, scale=-1.0)
                    esc = crp.tile([P, T], dt.float32, tag="esc")
                    s_col = ctmp.tile([P, 1], dt.float32, tag="s_col")
                    nc.scalar.activation(
                        esc[:], rt[:], AF.Exp,
                        bias=negm[cc][:, 0:1], scale=1.0,
                        accum_out=s_col[:, 0:1])
                    nc.vector.reciprocal(rs[cc][:], s_col[:])
            es_p.close()

            # ============ P4: fp32 q2/k2 + exact refinement ============
            w3_t = [sel_pool.tile([P, K], dt.float32, tag=f"w3_{dc}",
                                  name=f"w3_{b}_{dc}") for dc in range(CC)]
            gov = [sel_pool.tile([P, K], dt.uint32, tag=f"gov_{dc}",
                                 name=f"gov_{b}_{dc}") for dc in range(CC)]
            with tc.tile_pool(name=f"d{b}", bufs=1) as dp, \
                 tc.tile_pool(name=f"dsc{b}", bufs=2) as dsc, \
                 tc.tile_pool(name=f"dks{b}", bufs=3) as dks, \
                 tc.tile_pool(name=f"dk2{b}", bufs=1) as dk2, \
                 tc.tile_pool(name=f"dps{b}", bufs=2, space="PSUM") as dps:
                k2r = k2d.rearrange("(n p) w -> n p w", p=P)
                # P4a: k2 (fp32) -> DRAM doubled table
                xk_sb = dp.tile([P, CC, T], dt.float32, tag="x_p4s")
                nc.sync.dma_start(
                    xk_sb[:], key2[b].rearrange("(n p) t -> p n t", p=P))
                wk_sb = dp.tile([P, CC, C], dt.float32, tag="w_p4")
                nc.sync.dma_start(wk_sb[:], Wk.rearrange("(n p) d -> p n d", p=P))
                for dc in range(CC):
                    k2sb = dsc.tile([P, T], dt.float32, tag="k2sb")
                    for tb in range(4):
                        ps = dps.tile([P, T // 4], dt.float32, tag="p4ps")
                        for cc in range(CC):
                            nc.tensor.matmul(
                                ps[:], wk_sb[:, cc, bass.ts(dc, P)],
                                xk_sb[:, cc, bass.ts(tb, T // 4)],
                                start=(cc == 0), stop=(cc == CC - 1))
                        nc.scalar.activation(
                            k2sb[:, bass.ts(tb, T // 4)], ps[:], AF.Copy)
                    nc.sync.dma_start(k2r[b * CC + dc, :, 0:T], k2sb[:])
                    nc.sync.dma_start(k2r[b * CC + dc, :, T:2 * T], k2sb[:])
                # P4b: q2 per dc + gathers + dots + selection
                xq_sb = dp.tile([P, CC, T], dt.float32, tag="x_p4s")
                nc.sync.dma_start(
                    xq_sb[:], query2[b].rearrange("(n p) t -> p n t", p=P))
                wq_sb = dp.tile([P, CC, C], dt.float32, tag="w_p4")
                nc.sync.dma_start(wq_sb[:], Wq.rearrange("(n p) d -> p n d", p=P))
                for dc in range(CC):
                    q2sb = dsc.tile([P, T], dt.float32, tag="q2sb")
                    for tb in range(4):
                        ps = dps.tile([P, T // 4], dt.float32, tag="p4ps")
                        for cc in range(CC):
                            nc.tensor.matmul(
                                ps[:], wq_sb[:, cc, bass.ts(dc, P)],
                                xq_sb[:, cc, bass.ts(tb, T // 4)],
                                start=(cc == 0), stop=(cc == CC - 1))
                        nc.scalar.activation(
                            q2sb[:, bass.ts(tb, T // 4)], ps[:], AF.Copy)

                    # gather offsets for M candidates:
                    #   (b*C+dc*128+p)*2T + T - lag_m
                    iot = dsc.tile([P, 1], dt.int32, tag="iot")
                    nc.gpsimd.iota(
                        iot[:], pattern=[[0, 1]],
                        base=(b * C + dc * P) * (2 * T) + T,
                        channel_multiplier=2 * T)
                    iot_f = dsc.tile([P, 1], dt.float32, tag="iot_f")
                    nc.vector.tensor_copy(iot_f[:], iot[:])
                    idxm_f = dsc.tile([P, M], dt.float32, tag="idxm_f")
                    nc.vector.tensor_copy(idxm_f[:], idx8[dc][:, 0:M])
                    gom = dsc.tile([P, M], dt.float32, tag="gom")
                    nc.scalar.activation(gom[:], idxm_f[:],
                                         AF.Copy, bias=0.0, scale=-1.0)
                    nc.vector.tensor_scalar_add(gom[:], gom[:], iot_f[:, 0:1])
                    gou = dsc.tile([P, M], dt.uint32, tag="gou")
                    nc.vector.tensor_copy(gou[:], gom[:])

                    refined = dsc.tile([P, 8], dt.float32, tag="refined")
                    nc.gpsimd.memset(refined[:, M:8], -3.0e38)
                    for m in range(M):
                        ksh = dks.tile([P, T], dt.float32, tag="ksh")
                        nc.gpsimd.indirect_dma_start(
                            out=ksh[:], out_offset=None,
                            in_=k2d[:, :],
                            in_offset=bass.IndirectOffsetOnAxis(
                                ap=gou[:, m:m + 1], axis=1),
                            element_offset=0)
                        scr = dks.tile([P, T], dt.float32, tag="scr")
                        nc.vector.tensor_tensor(
                            out=scr[:], in0=q2sb[:], in1=ksh[:], op=OP.mult)
                        scr2 = dk2.tile([P, T], dt.float32, tag="scr2")
                        nc.scalar.activation(
                            scr2[:], scr[:], AF.Copy, scale=1.0 / T,
                            accum_out=refined[:, m:m + 1])

                    valr = dsc.tile([P, 8], dt.float32, tag="valr")
                    nc.vector.max(valr[:], refined[:])
                    pos8 = dsc.tile([P, 8], dt.uint32, tag="pos8")
                    nc.vector.max_index(pos8[:], valr[:], refined[:])
                    posf = dsc.tile([P, K], dt.float32, tag="posf")
                    nc.vector.tensor_copy(posf[:], pos8[:, 0:K])
                    # one-hot map: lag_sel[:, k] = sum_m idxm_f[:, m]*(posf==m)
                    lagf = dsc.tile([P, K], dt.float32, tag="lagf")
                    eqm = dsc.tile([P, K], dt.float32, tag="eqm")
                    contrib = dsc.tile([P, K], dt.float32, tag="contrib")
                    for m in range(M):
                        nc.vector.tensor_scalar(
                            out=eqm[:], in0=posf[:], scalar1=float(m),
                            scalar2=None, op0=OP.is_equal)
                        nc.vector.tensor_scalar_mul(
                            contrib[:], eqm[:], idxm_f[:, m:m + 1])
                        if m == 0:
                            nc.vector.tensor_copy(lagf[:], contrib[:])
                        else:
                            nc.vector.tensor_tensor(
                                out=lagf[:], in0=lagf[:], in1=contrib[:],
                                op=OP.add)
                    # weights: w3 = exp(valr[0:K] - m~) * rs
                    ew = dsc.tile([P, K], dt.float32, tag="ew")
                    nc.scalar.activation(ew[:], valr[:, 0:K],
                                         AF.Exp, bias=negm[dc][:, 0:1], scale=1.0)
                    nc.vector.tensor_scalar_mul(w3_t[dc][:], ew[:], rs[dc][:, 0:1])
                    # v2 gather offsets: rowbase + T - lag_sel
                    govf = dsc.tile([P, K], dt.float32, tag="govf")
                    nc.scalar.activation(govf[:], lagf[:],
                                         AF.Copy, bias=0.0, scale=-1.0)
                    nc.vector.tensor_scalar_add(govf[:], govf[:], iot_f[:, 0:1])
                    nc.vector.tensor_copy(gov[dc][:], govf[:])
                # P4v: v projection (f32r) -> v2 DRAM table (bf16, doubled)
                xv_sb = dp.tile([P, CC, T], dt.float32r, tag="x_p4s")
                nc.sync.dma_start(
                    xv_sb[:], value2[b].rearrange("(n p) t -> p n t", p=P))
                v2r = v2.rearrange("(n p) w -> n p w", p=P)
                for dc in range(CC):
                    v_sb = dsc.tile([P, T], dt.bfloat16, tag="v_sb")
                    for tb in range(4):
                        ps = dps.tile([P, T // 4], dt.float32, tag="p4ps")
                        for cc in range(CC):
                            nc.tensor.matmul(
                                ps[:], wv_sb[:, cc, bass.ts(dc, P)],
                                xv_sb[:, cc, bass.ts(tb, T // 4)],
                                start=(cc == 0), stop=(cc == CC - 1))
                        nc.scalar.activation(
                            v_sb[:, bass.ts(tb, T // 4)], ps[:], AF.Copy)
                    nc.sync.dma_start(v2r[b * CC + dc, :, 0:T], v_sb[:])
                    nc.sync.dma_start(v2r[b * CC + dc, :, T:2 * T], v_sb[:])
            es_sel.close_later = None  # keep sel_pool until P5 end

            # ============ P5: agg gathers (bf16) + scale + E ============
            with tc.tile_pool(name=f"e{b}", bufs=1) as ep, \
                 tc.tile_pool(name=f"eagg{b}", bufs=3) as eagg, \
                 tc.tile_pool(name=f"eps{b}", bufs=6, space="PSUM") as eps:
                wf16 = ep.tile([P, NE, C], dt.bfloat16, tag="wf16")
                with tc.tile_pool(name=f"wfload{b}", bufs=1) as wfl:
                    wf32 = wfl.tile([P, NE, C], dt.float32, tag="wf32")
                    nc.sync.dma_start(wf32[:], Wf.rearrange("(n p) d -> p n d", p=P))
                    for j in range(NE):
                        nc.scalar.activation(wf16[:, j, :], wf32[:, j, :], AF.Copy)
                TQ = T // 4
                for tb in range(4):
                    agg = eagg.tile([P, NE, TQ], dt.bfloat16, tag="aggs",
                                    name="aggs", bufs=3)
                    for k in range(K):
                        for dc in range(CC):
                            j = k * CC + dc
                            nc.gpsimd.indirect_dma_start(
                                out=agg[:, j, :], out_offset=None,
                                in_=v2[:, :],
                                in_offset=bass.IndirectOffsetOnAxis(
                                    ap=gov[dc][:, k:k + 1], axis=1),
                                element_offset=tb * TQ)
                            nc.vector.tensor_scalar_mul(
                                agg[:, j, :], agg[:, j, :],
                                w3_t[dc][:, k:k + 1])
                    for dco in range(CC):
                        ps = eps.tile([P, TQ], dt.float32, tag="out_ps")
                        for j in range(NE):
                            nc.tensor.matmul(
                                ps[:], wf16[:, j, bass.ts(dco, P)],
                                agg[:, j, :],
                                start=(j == 0), stop=(j == NE - 1))
                        o_sb = ep.tile([P, TQ], dt.float32, tag="o_sb")
                        nc.scalar.activation(o_sb[:], ps[:], AF.Copy)
                        nc.sync.dma_start(
                            out2[b, bass.ts(dco, P), bass.ts(tb, TQ)], o_sb[:])
            es_sel.close()

        es_const.close()

    nc.compile()
    return nc


def _get_nc():
    if "nc" not in _CACHE:
        _CACHE["nc"] = _build()
    return _CACHE["nc"]


def kernel(query, key, value, Wq, bq, Wk, bk, Wv, bv, Wf, bf):
    query = np.ascontiguousarray(np.asarray(query, dtype=np.float32))
    key = np.ascontiguousarray(np.asarray(key, dtype=np.float32))
    value = np.ascontiguousarray(np.asarray(value, dtype=np.float32))
    for bias in (bq, bk, bv, bf):
        assert np.all(np.asarray(bias) == 0.0), "nonzero biases unsupported"

    if "mats" not in _CACHE:
        _CACHE["mats"] = _dft_matrices()
    wree, wreo, wime, wimo, cie, sie = _CACHE["mats"]

    Wqc = np.ascontiguousarray(np.asarray(Wq, np.float32))
    Wkc = np.ascontiguousarray(np.asarray(Wk, np.float32))
    shared = {
        "Wq": Wqc, "Wk": Wkc, "Wqr": Wqc, "Wkr": Wkc,
        "Wvr": np.ascontiguousarray(np.asarray(Wv, np.float32)),
        "Wf": np.ascontiguousarray(np.asarray(Wf, np.float32)),
        "Wree": wree, "Wreo": wreo, "Wime": wime, "Wimo": wimo,
        "Cie": cie, "Sie": sie,
    }
    in_maps = []
    for c in range(NCORES):
        sl = slice(c * NB, (c + 1) * NB)
        in_maps.append({
            "query2": query[sl], "key2": key[sl], "value2": value[sl], **shared})

    from concourse.bass_utils import run_bass_kernel_spmd
    nc = _get_nc()
    res = run_bass_kernel_spmd(nc, in_maps, core_ids=list(range(NCORES)))
    _CACHE["last_results"] = res
    out = np.concatenate([res.results[c]["out2"] for c in range(NCORES)], axis=0)
    return out.astype(np.float32)


# revision 28
# speedup vs baseline: 1.1009x; 1.0123x over previous
"""AutoCorrelation Bass kernel, refinement architecture (stage 2).

Per batch: correlation pipeline (projections, fwd DFT, pointwise, inv DFT)
runs in float32r (tf32-grade, 1 cyc/row on PE) and is used ONLY to select
top-M=5 candidate lags per channel plus the softmax denominator. The top-3
selection and softmax weights then come from EXACT fp32 time-domain dots
a[tau] = (1/T) sum_t q2[t] k2[t-tau], with q2/k2 from fp32 matmuls and the
circular k-shifts gathered from a DRAM table via per-partition indirect DMA.
Value path (v-proj f32r, agg/E in bf16) only affects output values (2e-2 rel
gate; flips cost ~1e-2 each so refined selection must match fp32 reference).
"""
import numpy as np

import concourse.bass as bass
import concourse.tile as tile
from concourse import bacc, mybir

dt = mybir.dt
AF = mybir.ActivationFunctionType
OP = mybir.AluOpType

P = 128
B, C, T, K = 16, 512, 2048, 3
NB = 2
NCORES = 8
F = 1152
TC = T // P
CC = C // P
FC = F // P
NE = K * C // P
TE = 1152
TEC = TE // P
TO = 1024
TOC = TO // P
H = T // 2
M = 5                     # refinement candidates per channel

_CACHE = {}


def _dft_matrices():
    """Radix-split DFT matrices (fp64 -> fp32). Same as baseline."""
    t640 = np.arange(640.0)[:, None]
    t512 = np.arange(512.0)[:, None]
    ge = np.arange(640.0)[None, :]
    go = np.arange(512.0)[None, :]
    wree = np.where((t640 <= 512) & (ge <= 512),
                    np.cos(2 * np.pi * t640 * (2 * ge) / T), 0.0).astype(np.float32)
    wreo = np.cos(2 * np.pi * t512 * (2 * go + 1) / T).astype(np.float32)
    wime = np.where(ge <= 512,
                    -np.sin(2 * np.pi * t512 * (2 * ge) / T), 0.0).astype(np.float32)
    wimo = np.where(t640 <= 512,
                    -np.sin(2 * np.pi * t640 * (2 * go + 1) / T), 0.0).astype(np.float32)

    f64 = np.arange(F, dtype=np.float64)[None, :]
    livef = f64 <= H
    w = np.where((f64 == 0) | (f64 == H), 1.0, 2.0) * livef / (T * T)
    fc_ = f64.T
    tt = np.arange(TE, dtype=np.float64)[None, :]
    cie = np.where((fc_ <= H) & (tt <= H),
                   np.cos(2 * np.pi * fc_ * tt / T) * w.T, 0.0)
    tt2 = np.arange(TO, dtype=np.float64)[None, :]
    sie = np.where(fc_ <= H,
                   -np.sin(2 * np.pi * fc_ * tt2 / T) * w.T, 0.0)

    def permrows(m):
        out = np.zeros_like(m)
        out[0:513] = m[0:1025:2]
        out[640:1152] = m[1:1024:2]
        return out

    return (wree, wreo, wime, wimo,
            permrows(cie).astype(np.float32), permrows(sie).astype(np.float32))


def _build():
    nc = bacc.Bacc("TRN2", target_bir_lowering=False, debug=False,
                   num_devices=NCORES)

    query2 = nc.dram_tensor("query2", [NB, C, T], dt.float32, kind="ExternalInput").ap()
    key2 = nc.dram_tensor("key2", [NB, C, T], dt.float32, kind="ExternalInput").ap()
    value2 = nc.dram_tensor("value2", [NB, C, T], dt.float32r, kind="ExternalInput").ap()
    Wq = nc.dram_tensor("Wq", [C, C], dt.float32, kind="ExternalInput").ap()
    Wk = nc.dram_tensor("Wk", [C, C], dt.float32, kind="ExternalInput").ap()
    Wqr = nc.dram_tensor("Wqr", [C, C], dt.float32r, kind="ExternalInput").ap()
    Wkr = nc.dram_tensor("Wkr", [C, C], dt.float32r, kind="ExternalInput").ap()
    Wvr = nc.dram_tensor("Wvr", [C, C], dt.float32r, kind="ExternalInput").ap()
    Wf = nc.dram_tensor("Wf", [K * C, C], dt.float32, kind="ExternalInput").ap()
    Wree = nc.dram_tensor("Wree", [640, 640], dt.float32r, kind="ExternalInput").ap()
    Wreo = nc.dram_tensor("Wreo", [512, 512], dt.float32r, kind="ExternalInput").ap()
    Wime = nc.dram_tensor("Wime", [512, 640], dt.float32r, kind="ExternalInput").ap()
    Wimo = nc.dram_tensor("Wimo", [640, 512], dt.float32r, kind="ExternalInput").ap()
    Cie = nc.dram_tensor("Cie", [F, TE], dt.float32r, kind="ExternalInput").ap()
    Sie = nc.dram_tensor("Sie", [F, TO], dt.float32r, kind="ExternalInput").ap()
    out2 = nc.dram_tensor("out2", [NB, C, T], dt.float32, kind="ExternalOutput").ap()

    v2 = nc.dram_tensor("v2", [NB * C, 2 * T], dt.bfloat16).ap()     # rolled-v table
    k2d = nc.dram_tensor("k2d", [NB * C, 2 * T], dt.float32).ap()    # k2 gather table

    with tile.TileContext(nc) as tc:
        from contextlib import ExitStack

        # ---- P0: resident constants ----
        es_const = ExitStack()
        cpool = es_const.enter_context(tc.tile_pool(name="consts", bufs=1, side="left"))
        cie_sb = cpool.tile([P, FC, 640], dt.float32r, tag="cie_sb")
        sie_sb = cpool.tile([P, FC, 640], dt.float32r, tag="sie_sb")
        wv_sb = cpool.tile([P, CC, C], dt.float32r, tag="wv_sb")
        wqr_sb = cpool.tile([P, CC, C], dt.float32r, tag="wqr_sb")
        nc.sync.dma_start(wqr_sb[:], Wqr.rearrange("(n p) d -> p n d", p=P))
        wkr_sb = cpool.tile([P, CC, C], dt.float32r, tag="wkr_sb")
        nc.sync.dma_start(wkr_sb[:], Wkr.rearrange("(n p) d -> p n d", p=P))

        for b in range(NB):
            # ============ P1: radix splits + f32r split-proj + v ============
            es_qk = ExitStack()
            qk_pool = es_qk.enter_context(tc.tile_pool(name=f"qk{b}", bufs=1, side="right"))
            qTee = qk_pool.tile([P, 5, C], dt.float32r, tag="qTee")
            qTeo = qk_pool.tile([P, 4, C], dt.float32r, tag="qTeo")
            qToo = qk_pool.tile([P, 4, C], dt.float32r, tag="qToo")
            qToe = qk_pool.tile([P, 5, C], dt.float32r, tag="qToe")
            kTee = qk_pool.tile([P, 5, C], dt.float32r, tag="kTee")
            kTeo = qk_pool.tile([P, 4, C], dt.float32r, tag="kTeo")
            kToo = qk_pool.tile([P, 4, C], dt.float32r, tag="kToo")
            kToe = qk_pool.tile([P, 5, C], dt.float32r, tag="kToe")

            with tc.tile_pool(name=f"a{b}", bufs=1) as ap_, \
                 tc.tile_pool(name=f"axs{b}", bufs=2) as axs, \
                 tc.tile_pool(name=f"aps{b}", bufs=3, space="PSUM") as aps:
                for name, srcx, w_sb, dsts in (
                        ("q", query2, wqr_sb, (qTee, qTeo, qToo, qToe)),
                        ("k", key2, wkr_sb, (kTee, kTeo, kToo, kToe))):
                    dee, deo, doo, doe = dsts
                    xee = ap_.tile([P, CC, 640], dt.float32r, tag="xee")
                    xeo = ap_.tile([P, CC, 512], dt.float32r, tag="xeo")
                    xoo = ap_.tile([P, CC, 512], dt.float32r, tag="xoo")
                    xoe = ap_.tile([P, CC, 640], dt.float32r, tag="xoe")
                    for cc in range(CC):
                        x_cc = axs.tile([P, T], dt.float32, tag="x_cc")
                        nc.sync.dma_start(
                            x_cc[:],
                            srcx[b].rearrange("(n p) t -> p n t", p=P)[:, cc, :])
                        ab = axs.tile([P, 2, 511], dt.float32, tag="ab")
                        x = x_cc
                        # f32r memset hits a walrus codegen bug; zero via ACT
                        nc.scalar.activation(xee[:, cc, 513:640], x[:, 0:127],
                                             AF.Copy, bias=0.0, scale=0.0)
                        nc.scalar.activation(xoe[:, cc, 513:640], x[:, 0:127],
                                             AF.Copy, bias=0.0, scale=0.0)
                        nc.scalar.activation(xoe[:, cc, 0:1], x[:, 0:1],
                                             AF.Copy, bias=0.0, scale=0.0)
                        nc.scalar.activation(xoo[:, cc, 0:1], x[:, 0:1],
                                             AF.Copy, bias=0.0, scale=0.0)
                        nc.vector.tensor_tensor(out=ab[:, 0, :], in0=x[:, 1:512],
                                                in1=x[:, T - 1:1536:-1], op=OP.add)
                        nc.vector.tensor_tensor(out=ab[:, 1, :], in0=x[:, 1023:512:-1],
                                                in1=x[:, 1025:1536], op=OP.add)
                        nc.vector.tensor_tensor(out=xee[:, cc, 1:512], in0=ab[:, 0, :],
                                                in1=ab[:, 1, :], op=OP.add)
                        nc.vector.tensor_tensor(out=xeo[:, cc, 1:512], in0=ab[:, 0, :],
                                                in1=ab[:, 1, :], op=OP.subtract)
                        nc.vector.tensor_tensor(out=ab[:, 0, :], in0=x[:, 1:512],
                                                in1=x[:, T - 1:1536:-1], op=OP.subtract)
                        nc.vector.tensor_tensor(out=ab[:, 1, :], in0=x[:, 1023:512:-1],
                                                in1=x[:, 1025:1536], op=OP.subtract)
                        nc.vector.tensor_tensor(out=xoo[:, cc, 1:512], in0=ab[:, 0, :],
                                                in1=ab[:, 1, :], op=OP.subtract)
                        nc.vector.tensor_tensor(out=xoe[:, cc, 1:512], in0=ab[:, 0, :],
                                                in1=ab[:, 1, :], op=OP.add)
                        nc.vector.tensor_tensor(out=xee[:, cc, 0:1], in0=x[:, 0:1],
                                                in1=x[:, H:H + 1], op=OP.add)
                        nc.vector.tensor_tensor(out=xeo[:, cc, 0:1], in0=x[:, 0:1],
                                                in1=x[:, H:H + 1], op=OP.subtract)
                        nc.vector.tensor_tensor(out=xee[:, cc, 512:513], in0=x[:, 512:513],
                                                in1=x[:, 1536:1537], op=OP.add)
                        nc.vector.tensor_tensor(out=xoe[:, cc, 512:513], in0=x[:, 512:513],
                                                in1=x[:, 1536:1537], op=OP.subtract)
                    for st_, dst, nch in ((xee, dee, 5), (xeo, deo, 4),
                                          (xoo, doo, 4), (xoe, doe, 5)):
                        for i in range(nch):
                            ps = aps.tile([P, C], dt.float32, tag="proj_ps")
                            for cc in range(CC):
                                nc.tensor.matmul(
                                    ps[:], st_[:, cc, bass.ts(i, P)],
                                    w_sb[:, cc, :],
                                    start=(cc == 0), stop=(cc == CC - 1))
                            nc.scalar.activation(dst[:, i, :], ps[:], AF.Copy)

            if b == 0:
                # deferred const loads: issued after P1's input DMAs so the
                # head of the in-order DMA queue feeds the splits first
                nc.sync.dma_start(
                    cie_sb[:], Cie.rearrange("(n p) t -> p n t", p=P)[:, :, 0:640])
                nc.sync.dma_start(
                    sie_sb[:], Sie.rearrange("(n p) t -> p n t", p=P)[:, :, 0:640])
                nc.sync.dma_start(wv_sb[:], Wvr.rearrange("(n p) d -> p n d", p=P))
            # ============ P2: forward DFT (f32r) + pointwise ============
            es_p = ExitStack()
            p_pool = es_p.enter_context(tc.tile_pool(name=f"p{b}", bufs=1, side="left"))
            pre = p_pool.tile([P, FC, C], dt.float32r, tag="pre")
            pim = p_pool.tile([P, FC, C], dt.float32r, tag="pim")
            with tc.tile_pool(name=f"bmat{b}", bufs=2) as bmat, \
                 tc.tile_pool(name=f"bps{b}", bufs=2, space="PSUM") as bps, \
                 tc.tile_pool(name=f"btmp{b}", bufs=2) as btmp:
                wree_r = Wree.rearrange("(n p) f -> p n f", p=P)
                wreo_r = Wreo.rearrange("(n p) f -> p n f", p=P)
                wime_r = Wime.rearrange("(n p) f -> p n f", p=P)
                wimo_r = Wimo.rearrange("(n p) f -> p n f", p=P)
                for fc in range(FC):
                    even = fc < 5
                    fl = fc if even else fc - 5
                    ncos, nsin = (5, 4) if even else (4, 5)
                    cm = bmat.tile([P, 5, P], dt.float32r, tag="cm")
                    nc.sync.dma_start(
                        cm[:, 0:ncos, :],
                        (wree_r if even else wreo_r)[:, :, bass.ts(fl, P)])
                    sm = bmat.tile([P, 5, P], dt.float32r, tag="sm")
                    nc.sync.dma_start(
                        sm[:, 0:nsin, :],
                        (wime_r if even else wimo_r)[:, :, bass.ts(fl, P)])
                    qcos = qTee if even else qTeo
                    qsin = qToo if even else qToe
                    kcos = kTee if even else kTeo
                    ksin = kToo if even else kToe
                    acc = {}
                    for nm, mat, sig, nchunk in (
                            ("aq", cm, qcos, ncos), ("bq", sm, qsin, nsin),
                            ("ak", cm, kcos, ncos), ("bk", sm, ksin, nsin)):
                        ps = bps.tile([P, C], dt.float32, tag=nm, name=f"ps_{nm}",
                                      bufs=(1 if nm in ("ak", "bk") else 2))
                        for i in range(nchunk):
                            nc.tensor.matmul(
                                ps[:], mat[:, i, :], sig[:, i, :],
                                start=(i == 0), stop=(i == nchunk - 1))
                        acc[nm] = ps
                    aqs = btmp.tile([P, C], dt.float32, tag="aqs")
                    nc.scalar.activation(aqs[:], acc["aq"][:], AF.Copy)
                    bqs = btmp.tile([P, C], dt.float32, tag="bqs")
                    nc.scalar.activation(bqs[:], acc["bq"][:], AF.Copy)
                    tmp = btmp.tile([P, C], dt.float32, tag="tmp")
                    nc.vector.tensor_tensor(
                        out=pre[:, fc, :], in0=aqs[:], in1=acc["ak"][:], op=OP.mult)
                    nc.vector.tensor_tensor(
                        out=tmp[:], in0=bqs[:], in1=acc["bk"][:], op=OP.mult)
                    nc.vector.tensor_tensor(
                        out=pre[:, fc, :], in0=pre[:, fc, :], in1=tmp[:], op=OP.add)
                    nc.vector.tensor_tensor(
                        out=pim[:, fc, :], in0=bqs[:], in1=acc["ak"][:], op=OP.mult)
                    tmp2 = btmp.tile([P, C], dt.float32, tag="tmp2")
                    nc.vector.tensor_tensor(
                        out=tmp2[:], in0=aqs[:], in1=acc["bk"][:], op=OP.mult)
                    nc.vector.tensor_tensor(
                        out=pim[:, fc, :], in0=pim[:, fc, :], in1=tmp2[:], op=OP.subtract)
            es_qk.close()

            # ============ P3: inverse DFT (f32r) + top-8 + denominator ======
            es_sel = ExitStack()
            sel_pool = es_sel.enter_context(
                tc.tile_pool(name=f"sel{b}", bufs=1, side="right"))
            idx8 = [sel_pool.tile([P, 8], dt.uint32, tag=f"idx8_{dc}",
                                  name=f"idx8_{b}_{dc}") for dc in range(CC)]
            negm = [sel_pool.tile([P, 1], dt.float32, tag=f"negm_{dc}",
                                  name=f"negm_{b}_{dc}") for dc in range(CC)]
            rs = [sel_pool.tile([P, 1], dt.float32, tag=f"rs_{dc}",
                                name=f"rs_{b}_{dc}") for dc in range(CC)]
            with tc.tile_pool(name=f"cr{b}", bufs=2) as crp, \
                 tc.tile_pool(name=f"ctmp{b}", bufs=2) as ctmp, \
                 tc.tile_pool(name=f"cps{b}", bufs=1, space="PSUM") as cps:
                HB = H // 2
                for cc in range(CC):
                    rcE = cps.tile([P, HB], dt.float32, tag="rcE", name="ps_rcE", bufs=2)
                    rcE2 = cps.tile([P, P], dt.float32, tag="rcE2", name="ps_rcE2")
                    rcO = cps.tile([P, HB], dt.float32, tag="rcO", name="ps_rcO")
                    rsE = cps.tile([P, HB], dt.float32, tag="rsE", name="ps_rsE")
                    rsO = cps.tile([P, HB], dt.float32, tag="rsO", name="ps_rsO", bufs=2)
                    rsO2 = cps.tile([P, P], dt.float32, tag="rsO2", name="ps_rsO2")
                    for fc in range(5):
                        st, sp = (fc == 0), (fc == 4)
                        pre_l = pre[:, fc, bass.ts(cc, P)]
                        pim_l = pim[:, fc, bass.ts(cc, P)]
                        nc.tensor.matmul(rcE[:], pre_l, cie_sb[:, fc, 0:HB],
                                         start=st, stop=sp)
                        nc.tensor.matmul(rcE2[:], pre_l, cie_sb[:, fc, HB:HB + P],
                                         start=st, stop=sp)
                        nc.tensor.matmul(rsE[:], pim_l, sie_sb[:, fc, 0:HB],
                                         start=st, stop=sp)
                    for fc in range(5, FC):
                        st, sp = (fc == 5), (fc == FC - 1)
                        pre_l = pre[:, fc, bass.ts(cc, P)]
                        pim_l = pim[:, fc, bass.ts(cc, P)]
                        nc.tensor.matmul(rcO[:], pre_l, cie_sb[:, fc, 0:HB],
                                         start=st, stop=sp)
                        nc.tensor.matmul(rsO[:], pim_l, sie_sb[:, fc, 0:HB],
                                         start=st, stop=sp)
                        nc.tensor.matmul(rsO2[:], pim_l, sie_sb[:, fc, HB:HB + P],
                                         start=st, stop=sp)
                    rcO_sb = ctmp.tile([P, HB], dt.float32, tag="rcO_sb")
                    nc.scalar.activation(rcO_sb[:], rcO[:], AF.Copy)
                    rsE_sb = ctmp.tile([P, HB], dt.float32, tag="rsE_sb")
                    nc.scalar.activation(rsE_sb[:], rsE[:], AF.Copy)
                    rsO_sb = ctmp.tile([P, HB + 1], dt.float32, tag="rsO_sb")
                    nc.scalar.activation(rsO_sb[:, 0:HB], rsO[:], AF.Copy)
                    nc.scalar.activation(rsO_sb[:, HB:HB + 1], rsO2[:, 0:1], AF.Copy)
                    s1 = ctmp.tile([P, HB], dt.float32, tag="s1")
                    nc.vector.tensor_tensor(out=s1[:], in0=rcE[:], in1=rcO_sb[:],
                                            op=OP.add)
                    s2 = ctmp.tile([P, HB], dt.float32, tag="s2")
                    nc.vector.tensor_tensor(out=s2[:], in0=rcE[:], in1=rcO_sb[:],
                                            op=OP.subtract)
                    w1 = ctmp.tile([P, HB], dt.float32, tag="w1")
                    nc.vector.tensor_tensor(out=w1[:], in0=rsE_sb[:],
                                            in1=rsO_sb[:, 0:HB], op=OP.add)
                    w2 = ctmp.tile([P, HB], dt.float32, tag="w2")
                    nc.vector.tensor_tensor(out=w2[:], in0=rsO_sb[:, 0:HB],
                                            in1=rsE_sb[:], op=OP.subtract)
                    rt = crp.tile([P, T], dt.float32, tag="rt")
                    nc.vector.tensor_tensor(out=rt[:, 0:HB], in0=s1[:], in1=w1[:],
                                            op=OP.add)
                    nc.vector.tensor_tensor(out=rt[:, 1023:HB:-1], in0=s2[:, 1:HB],
                                            in1=w2[:, 1:HB], op=OP.add)
                    nc.vector.tensor_tensor(out=rt[:, 1025:1536], in0=s2[:, 1:HB],
                                            in1=w2[:, 1:HB], op=OP.subtract)
                    nc.vector.tensor_tensor(out=rt[:, T - 1:1536:-1], in0=s1[:, 1:HB],
                                            in1=w1[:, 1:HB], op=OP.subtract)
                    nc.vector.tensor_tensor(out=rt[:, HB:HB + 1], in0=rcE2[:, 0:1],
                                            in1=rsO_sb[:, HB:HB + 1], op=OP.add)
                    nc.vector.tensor_tensor(out=rt[:, H:H + 1], in0=rcE[:, 0:1],
                                            in1=rcO_sb[:, 0:1], op=OP.subtract)
                    nc.vector.tensor_tensor(out=rt[:, 1536:1537], in0=rcE2[:, 0:1],
                                            in1=rsO_sb[:, HB:HB + 1], op=OP.subtract)
                    # top-8 + softmax denominator on approx r
                    vals = ctmp.tile([P, 8], dt.float32, tag="vals")
                    nc.vector.max(vals[:], rt[:])
                    nc.vector.max_index(idx8[cc][:], vals[:], rt[:])
                    nc.scalar.activation(negm[cc][:], vals[:, 0:1],
                                         AF.Copy, bias=0.0, scale=-1.0)
                    esc = crp.tile([P, T], dt.float32, tag="esc")
                    s_col = ctmp.tile([P, 1], dt.float32, tag="s_col")
                    nc.scalar.activation(
                        esc[:], rt[:], AF.Exp,
                        bias=negm[cc][:, 0:1], scale=1.0,
                        accum_out=s_col[:, 0:1])
                    nc.vector.reciprocal(rs[cc][:], s_col[:])
            es_p.close()

            # ============ P4: fp32 q2/k2 + exact refinement ============
            w3_t = [sel_pool.tile([P, K], dt.float32, tag=f"w3_{dc}",
                                  name=f"w3_{b}_{dc}") for dc in range(CC)]
            gov = [sel_pool.tile([P, K], dt.uint32, tag=f"gov_{dc}",
                                 name=f"gov_{b}_{dc}") for dc in range(CC)]
            with tc.tile_pool(name=f"d{b}", bufs=1) as dp, \
                 tc.tile_pool(name=f"dsc{b}", bufs=2) as dsc, \
                 tc.tile_pool(name=f"dks{b}", bufs=3) as dks, \
                 tc.tile_pool(name=f"dk2{b}", bufs=1) as dk2, \
                 tc.tile_pool(name=f"dps{b}", bufs=2, space="PSUM") as dps:
                k2r = k2d.rearrange("(n p) w -> n p w", p=P)
                # P4a: k2 (fp32) -> DRAM doubled table
                xk_sb = dp.tile([P, CC, T], dt.float32, tag="x_p4s")
                nc.sync.dma_start(
                    xk_sb[:], key2[b].rearrange("(n p) t -> p n t", p=P))
                wk_sb = dp.tile([P, CC, C], dt.float32, tag="w_p4")
                nc.sync.dma_start(wk_sb[:], Wk.rearrange("(n p) d -> p n d", p=P))
                for dc in range(CC):
                    k2sb = dsc.tile([P, T], dt.float32, tag="k2sb")
                    for tb in range(4):
                        ps = dps.tile([P, T // 4], dt.float32, tag="p4ps")
                        for cc in range(CC):
                            nc.tensor.matmul(
                                ps[:], wk_sb[:, cc, bass.ts(dc, P)],
                                xk_sb[:, cc, bass.ts(tb, T // 4)],
                                start=(cc == 0), stop=(cc == CC - 1))
                        nc.scalar.activation(
                            k2sb[:, bass.ts(tb, T // 4)], ps[:], AF.Copy)
                    nc.scalar.dma_start(k2r[b * CC + dc, :, 0:T], k2sb[:])
                    nc.scalar.dma_start(k2r[b * CC + dc, :, T:2 * T], k2sb[:])
                # P4b: q2 per dc + gathers + dots + selection
                xq_sb = dp.tile([P, CC, T], dt.float32, tag="x_p4s")
                nc.sync.dma_start(
                    xq_sb[:], query2[b].rearrange("(n p) t -> p n t", p=P))
                wq_sb = dp.tile([P, CC, C], dt.float32, tag="w_p4")
                nc.sync.dma_start(wq_sb[:], Wq.rearrange("(n p) d -> p n d", p=P))
                for dc in range(CC):
                    q2sb = dsc.tile([P, T], dt.float32, tag="q2sb")
                    for tb in range(4):
                        ps = dps.tile([P, T // 4], dt.float32, tag="p4ps")
                        for cc in range(CC):
                            nc.tensor.matmul(
                                ps[:], wq_sb[:, cc, bass.ts(dc, P)],
                                xq_sb[:, cc, bass.ts(tb, T // 4)],
                                start=(cc == 0), stop=(cc == CC - 1))
                        nc.scalar.activation(
                            q2sb[:, bass.ts(tb, T // 4)], ps[:], AF.Copy)

                    # gather offsets for M candidates:
                    #   (b*C+dc*128+p)*2T + T - lag_m
                    iot = dsc.tile([P, 1], dt.int32, tag="iot")
                    nc.gpsimd.iota(
                        iot[:], pattern=[[0, 1]],
                        base=(b * C + dc * P) * (2 * T) + T,
                        channel_multiplier=2 * T)
                    iot_f = dsc.tile([P, 1], dt.float32, tag="iot_f")
                    nc.vector.tensor_copy(iot_f[:], iot[:])
                    idxm_f = dsc.tile([P, M], dt.float32, tag="idxm_f")
                    nc.vector.tensor_copy(idxm_f[:], idx8[dc][:, 0:M])
                    gom = dsc.tile([P, M], dt.float32, tag="gom")
                    nc.scalar.activation(gom[:], idxm_f[:],
                                         AF.Copy, bias=0.0, scale=-1.0)
                    nc.vector.tensor_scalar_add(gom[:], gom[:], iot_f[:, 0:1])
                    gou = dsc.tile([P, M], dt.uint32, tag="gou")
                    nc.vector.tensor_copy(gou[:], gom[:])

                    refined = dsc.tile([P, 8], dt.float32, tag="refined")
                    nc.gpsimd.memset(refined[:, M:8], -3.0e38)
                    for m in range(M):
                        ksh = dks.tile([P, T], dt.float32, tag="ksh")
                        nc.gpsimd.indirect_dma_start(
                            out=ksh[:], out_offset=None,
                            in_=k2d[:, :],
                            in_offset=bass.IndirectOffsetOnAxis(
                                ap=gou[:, m:m + 1], axis=1),
                            element_offset=0)
                        scr = dks.tile([P, T], dt.float32, tag="scr")
                        nc.vector.tensor_tensor(
                            out=scr[:], in0=q2sb[:], in1=ksh[:], op=OP.mult)
                        scr2 = dk2.tile([P, T], dt.float32, tag="scr2")
                        nc.scalar.activation(
                            scr2[:], scr[:], AF.Copy, scale=1.0 / T,
                            accum_out=refined[:, m:m + 1])

                    valr = dsc.tile([P, 8], dt.float32, tag="valr")
                    nc.vector.max(valr[:], refined[:])
                    pos8 = dsc.tile([P, 8], dt.uint32, tag="pos8")
                    nc.vector.max_index(pos8[:], valr[:], refined[:])
                    posf = dsc.tile([P, K], dt.float32, tag="posf")
                    nc.vector.tensor_copy(posf[:], pos8[:, 0:K])
                    # one-hot map: lag_sel[:, k] = sum_m idxm_f[:, m]*(posf==m)
                    lagf = dsc.tile([P, K], dt.float32, tag="lagf")
                    eqm = dsc.tile([P, K], dt.float32, tag="eqm")
                    contrib = dsc.tile([P, K], dt.float32, tag="contrib")
                    for m in range(M):
                        nc.vector.tensor_scalar(
                            out=eqm[:], in0=posf[:], scalar1=float(m),
                            scalar2=None, op0=OP.is_equal)
                        nc.vector.tensor_scalar_mul(
                            contrib[:], eqm[:], idxm_f[:, m:m + 1])
                        if m == 0:
                            nc.vector.tensor_copy(lagf[:], contrib[:])
                        else:
                            nc.vector.tensor_tensor(
                                out=lagf[:], in0=lagf[:], in1=contrib[:],
                                op=OP.add)
                    # weights: w3 = exp(valr[0:K] - m~) * rs
                    ew = dsc.tile([P, K], dt.float32, tag="ew")
                    nc.scalar.activation(ew[:], valr[:, 0:K],
                                         AF.Exp, bias=negm[dc][:, 0:1], scale=1.0)
                    nc.vector.tensor_scalar_mul(w3_t[dc][:], ew[:], rs[dc][:, 0:1])
                    # v2 gather offsets: rowbase + T - lag_sel
                    govf = dsc.tile([P, K], dt.float32, tag="govf")
                    nc.scalar.activation(govf[:], lagf[:],
                                         AF.Copy, bias=0.0, scale=-1.0)
                    nc.vector.tensor_scalar_add(govf[:], govf[:], iot_f[:, 0:1])
                    nc.vector.tensor_copy(gov[dc][:], govf[:])
                # P4v: v projection (f32r) -> v2 DRAM table (bf16, doubled)
                xv_sb = dp.tile([P, CC, T], dt.float32r, tag="x_p4s")
                nc.sync.dma_start(
                    xv_sb[:], value2[b].rearrange("(n p) t -> p n t", p=P))
                v2r = v2.rearrange("(n p) w -> n p w", p=P)
                for dc in range(CC):
                    v_sb = dsc.tile([P, T], dt.bfloat16, tag="v_sb")
                    for tb in range(4):
                        ps = dps.tile([P, T // 4], dt.float32, tag="p4ps")
                        for cc in range(CC):
                            nc.tensor.matmul(
                                ps[:], wv_sb[:, cc, bass.ts(dc, P)],
                                xv_sb[:, cc, bass.ts(tb, T // 4)],
                                start=(cc == 0), stop=(cc == CC - 1))
                        nc.scalar.activation(
                            v_sb[:, bass.ts(tb, T // 4)], ps[:], AF.Copy)
                    nc.sync.dma_start(v2r[b * CC + dc, :, 0:T], v_sb[:])
                    nc.sync.dma_start(v2r[b * CC + dc, :, T:2 * T], v_sb[:])
            es_sel.close_later = None  # keep sel_pool until P5 end

            # ============ P5: agg gathers (bf16) + scale + E ============
            with tc.tile_pool(name=f"e{b}", bufs=1) as ep, \
                 tc.tile_pool(name=f"eagg{b}", bufs=3) as eagg, \
                 tc.tile_pool(name=f"eps{b}", bufs=6, space="PSUM") as eps:
                wf16 = ep.tile([P, NE, C], dt.bfloat16, tag="wf16")
                with tc.tile_pool(name=f"wfload{b}", bufs=1) as wfl:
                    wf32 = wfl.tile([P, NE, C], dt.float32, tag="wf32")
                    nc.sync.dma_start(wf32[:], Wf.rearrange("(n p) d -> p n d", p=P))
                    for j in range(NE):
                        nc.scalar.activation(wf16[:, j, :], wf32[:, j, :], AF.Copy)
                TQ = T // 4
                for tb in range(4):
                    agg = eagg.tile([P, NE, TQ], dt.bfloat16, tag="aggs",
                                    name="aggs", bufs=3)
                    for k in range(K):
                        for dc in range(CC):
                            j = k * CC + dc
                            nc.gpsimd.indirect_dma_start(
                                out=agg[:, j, :], out_offset=None,
                                in_=v2[:, :],
                                in_offset=bass.IndirectOffsetOnAxis(
                                    ap=gov[dc][:, k:k + 1], axis=1),
                                element_offset=tb * TQ)
                            nc.vector.tensor_scalar_mul(
                                agg[:, j, :], agg[:, j, :],
                                w3_t[dc][:, k:k + 1])
                    for dco in range(CC):
                        ps = eps.tile([P, TQ], dt.float32, tag="out_ps")
                        for j in range(NE):
                            nc.tensor.matmul(
                                ps[:], wf16[:, j, bass.ts(dco, P)],
                                agg[:, j, :],
                                start=(j == 0), stop=(j == NE - 1))
                        o_sb = ep.tile([P, TQ], dt.float32, tag="o_sb")
                        nc.scalar.activation(o_sb[:], ps[:], AF.Copy)
                        nc.sync.dma_start(
                            out2[b, bass.ts(dco, P), bass.ts(tb, TQ)], o_sb[:])
            es_sel.close()

        es_const.close()

    nc.compile()
    return nc


def _get_nc():
    if "nc" not in _CACHE:
        _CACHE["nc"] = _build()
    return _CACHE["nc"]


def kernel(query, key, value, Wq, bq, Wk, bk, Wv, bv, Wf, bf):
    query = np.ascontiguousarray(np.asarray(query, dtype=np.float32))
    key = np.ascontiguousarray(np.asarray(key, dtype=np.float32))
    value = np.ascontiguousarray(np.asarray(value, dtype=np.float32))
    for bias in (bq, bk, bv, bf):
        assert np.all(np.asarray(bias) == 0.0), "nonzero biases unsupported"

    if "mats" not in _CACHE:
        _CACHE["mats"] = _dft_matrices()
    wree, wreo, wime, wimo, cie, sie = _CACHE["mats"]

    Wqc = np.ascontiguousarray(np.asarray(Wq, np.float32))
    Wkc = np.ascontiguousarray(np.asarray(Wk, np.float32))
    shared = {
        "Wq": Wqc, "Wk": Wkc, "Wqr": Wqc, "Wkr": Wkc,
        "Wvr": np.ascontiguousarray(np.asarray(Wv, np.float32)),
        "Wf": np.ascontiguousarray(np.asarray(Wf, np.float32)),
        "Wree": wree, "Wreo": wreo, "Wime": wime, "Wimo": wimo,
        "Cie": cie, "Sie": sie,
    }
    in_maps = []
    for c in range(NCORES):
        sl = slice(c * NB, (c + 1) * NB)
        in_maps.append({
            "query2": query[sl], "key2": key[sl], "value2": value[sl], **shared})

    from concourse.bass_utils import run_bass_kernel_spmd
    nc = _get_nc()
    res = run_bass_kernel_spmd(nc, in_maps, core_ids=list(range(NCORES)))
    _CACHE["last_results"] = res
    out = np.concatenate([res.results[c]["out2"] for c in range(NCORES)], axis=0)
    return out.astype(np.float32)
